# revision 16
# baseline (speedup 1.0000x reference)
"""Fused causal-transformer block (LN1 -> attn -> LN2 -> MLP, residuals) on
8 Trainium2 NeuronCores.

Sharding: 2 groups of 4 cores; group g handles batch element b=g (data
parallel).  Within a group:
  - Attention is Megatron head-parallel: core s owns 4 heads, computes
    partial y = attn(x) @ W_aproj_s for ALL tokens, chunked over four
    512-token chunks.  Each chunk's partials are summed with an in-group
    ReduceScatter, leaving core s with the summed attention output for
    token tile (chunk*4 + s) -- its 128-token slice of each chunk.
  - The MLP is token-parallel with REPLICATED weights: core s runs the
    full 4C-hidden MLP for its 4 owned token tiles (512 tokens total).
    No second collective is needed; the host reassembles token slices.
This cuts the collective count from 8 AllReduces to 4 ReduceScatters,
all hidden behind attention compute (the single CC core was the
bottleneck of the AllReduce design).

Compute dtype: bf16 matmul inputs, fp32 PSUM accumulation, fp32 residual
stream and softmax statistics.

Layouts (per core, all prepared host-side in kernel()):
  h1T/h2T  : [128, C/128, t]  activations transposed (contraction dim on
             partitions) produced on-device via PE transposes.
  qkT      : [128, H_core, T] rows = [q heads | k heads] * 64-dim each,
             two heads stacked per 128-partition tile.  Scores are
             computed directly in S^T [k, q] layout, so softmax
             normalization arrives as a PSUM row via a ones-column in v.
  v_aug    : [128, T/128, H_core, 65]  v token-major per head + ones col.
  w_fcT    : [128, C/128, 4C]  full W_fc^T (replicated), moving operand.
  w_mpT    : [128, 4C/128, C]  full W_mproj^T (replicated), moving.
"""

import contextlib
import ctypes
import math
import sys
import types

import numpy as np
import ml_dtypes

import bass_rust
import concourse.bass as bass
import concourse.mybir as mybir
import concourse.tile as tile
from concourse import library_config
from concourse.bass_utils import run_bass_kernel_spmd
from concourse.masks import make_identity
from concourse.tile import TileContext
from concourse.vector_clock import ScopedClock

# ---------------------------------------------------------------------------
# problem constants (hardcoded per the harness contract)
B, T, C, H = 2, 2048, 1024, 16
HD = C // H                 # 64
N_CORES = 8
TPG = 4                     # tensor-parallel group size
H_CORE = H // TPG           # heads per core = 4
DH = H_CORE * HD            # per-core attention dim = 256
FH = 4 * C                  # full MLP hidden (replicated) = 4096
P = 128
EPS = 1e-5
QCH = 512                   # q-chunk width
GROUPS = [[0, 1, 2, 3], [4, 5, 6, 7]]

F32 = mybir.dt.float32
BF16 = mybir.dt.bfloat16

# ---------------------------------------------------------------------------
# workaround 1: the container's walrus accepts a single sync-wait command per
# instruction; move extra semaphore waits onto inserted EventSemaphore
# instructions on the same engine (program order preserves semantics).

_waitfix_counter = [0]


def _legalize_waits(nc, cap=1):
    fn = nc.m.functions[0]
    n_split = 0
    for bb in fn.blocks:
        out = []
        changed = False
        for inst in bb.instructions:
            si = inst.sync_info
            waits = list(si.on_wait) if si is not None else []
            if len(waits) > cap:
                movable = [w for w in waits if w.sync_type == "semaphore"]
                fixed = [w for w in waits if w.sync_type != "semaphore"]
                n_keep = max(cap - len(fixed), 0)
                keep = fixed + (movable[len(movable) - n_keep:] if n_keep else [])
                extra = movable[: len(movable) - n_keep] if n_keep else movable
                for w in extra:
                    _waitfix_counter[0] += 1
                    ev = mybir.InstEventSemaphore(
                        name=f"I-waitfix-{_waitfix_counter[0]}",
                        engine=inst.engine,
                        ins=[],
                        outs=[],
                        sync_info=bass_rust.SyncInfo(on_wait=[w], on_update=[]),
                    )
                    out.append(ev)
                    n_split += 1
                inst.sync_info = bass_rust.SyncInfo(
                    on_wait=keep, on_update=list(si.on_update)
                )
                changed = True
            out.append(inst)
        if changed:
            bb.instructions = out
    return n_split


# workaround 2: same issue for the Tile kernel-tail Drain — emit one wait-nop
# per live proc ahead of a wait-less drain instead of stacking waits on it.


def _drain_and_barrier_split(self, tick_clock, wait_clock):
    gc = tick_clock.global_clock
    sems_alloc = wait_clock.sems.allocated()
    for proc in sorted(sems_alloc):
        tick = gc.peek_next(proc) - 1
        if tick <= 0:
            continue
        vc1 = bass_rust.VectorClock()
        vc1.require_at_least(proc, tick)
        nop = self.nc.sync.nop()
        wait_clock.add_sem_waits(nop.ins, ScopedClock({None: vc1}))
    self.nc.sync.drain()
    self.nc.all_engine_barrier()
    assert self.sems is not None
    popped = self.nc._tile_sem_poison_stack.pop()
    assert popped is self._sem_poison
    self.nc.clear_and_free_semaphores(list(self.sems.allocated().values()))
    self.nc.all_engine_barrier()


TileContext._drain_and_barrier = _drain_and_barrier_split


# workaround 3 (profiling only): register the NTFF hook the trimmed antenv
# lacks so run_bass_kernel_spmd(trace=True) works under axon.


def _install_prof_hook():
    if "antenv.axon_hooks" in sys.modules:
        return
    so_path = "/opt/axon/libaxon_pjrt.so"
    hook = None
    try:
        lib = ctypes.CDLL(so_path)
        if hasattr(lib, "axon_start_nrt_profile"):
            lib.axon_start_nrt_profile.argtypes = [
                ctypes.POINTER(ctypes.c_int64),
                ctypes.c_size_t,
            ]
            lib.axon_start_nrt_profile.restype = ctypes.c_int64
            lib.axon_stop_nrt_profile.argtypes = [ctypes.c_char_p]
            lib.axon_stop_nrt_profile.restype = ctypes.c_int64

            @contextlib.contextmanager
            def _hook_cm(output_dir, device_ids):
                import jax

                jax.devices()
                if device_ids:
                    ids = (ctypes.c_int64 * len(device_ids))(*device_ids)
                    rc = lib.axon_start_nrt_profile(ids, len(device_ids))
                else:
                    rc = lib.axon_start_nrt_profile(None, 0)
                if rc != 0:
                    raise RuntimeError(f"axon_start_nrt_profile rc={rc}")
                try:
                    yield
                finally:
                    n = lib.axon_stop_nrt_profile(str(output_dir).encode())
                    if n < 0:
                        raise RuntimeError(f"axon_stop_nrt_profile rc={n}")

            hook = _hook_cm
    except OSError:
        pass
    mod = types.ModuleType("antenv.axon_hooks")
    mod.get_axon_ntff_profile_hook = lambda: hook
    mod.set_axon_ntff_profile_hook = lambda h: None
    sys.modules["antenv.axon_hooks"] = mod
    from concourse import bass_utils

    bass_utils.upload_artifacts = lambda tmpdir: tmpdir


# ---------------------------------------------------------------------------
# device kernel builder


def build_module(
    t_len=T,
    c_dim=C,
    h_core=H_CORE,
    fh=FH,
    flags=frozenset(),
    replica_groups=GROUPS,
    local_reduce=False,
    legalize=True,
):
    """Build the per-core SPMD Bass module.

    flags: subset of {"ln1_g","ln1_b","ln2_g","ln2_b","b_qk","b_v","b_ap",
    "b_fc","b_mp"} enabling the non-trivial affine/bias paths.
    local_reduce: replace the in-group ReduceScatter with a local strided
    copy (single core test mode: takes this rank-0 slice).
    """
    KO = c_dim // P             # c-tiles
    NT = t_len // P             # token tiles
    NQC = t_len // QCH          # q chunks
    KPQ = QCH // P              # token tiles per chunk (= group size 4)
    QK = h_core * P             # stacked q+k dims
    MQK = h_core                # m-tiles of qkT
    DKO = (h_core * HD) // P    # d-tiles of y/aproj  (h_core/2)
    FKO = fh // P               # hidden tiles (32)
    NPAIR = h_core // 2
    NCC = c_dim // QCH          # 512-chunks of C
    NHC = fh // QCH             # 512-chunks of hidden (8)
    assert h_core % 2 == 0 and c_dim % P == 0 and t_len % QCH == 0

    nc = bass.Bass(num_devices=N_CORES)

    x_tm = nc.dram_tensor("x_tm", (t_len, c_dim), F32, kind="ExternalInput")
    x_own = nc.dram_tensor("x_own", (NQC * P, c_dim), F32, kind="ExternalInput")
    w_qk = nc.dram_tensor("w_qk", (P, KO, QK), BF16, kind="ExternalInput")
    w_v = nc.dram_tensor("w_v", (P, KO, h_core * HD), BF16, kind="ExternalInput")
    w_ap = nc.dram_tensor("w_ap", (P, DKO, c_dim), BF16, kind="ExternalInput")
    w_fcT = nc.dram_tensor("w_fcT", (P, KO, fh), BF16, kind="ExternalInput")
    w_mpT = nc.dram_tensor("w_mpT", (P, FKO, c_dim), BF16, kind="ExternalInput")
    tri = nc.dram_tensor("tri", (P, P), F32, kind="ExternalInput")
    opt_in = {}
    for name, shape in [
        ("ln1_g", (1, c_dim)), ("ln1_b", (1, c_dim)),
        ("ln2_g", (1, c_dim)), ("ln2_b", (1, c_dim)),
        ("b_qk", (P, MQK)), ("b_v", (1, h_core * HD)), ("b_ap", (1, c_dim)),
        ("b_fc", (1, fh)), ("b_mp", (1, c_dim)),
    ]:
        if name in flags:
            opt_in[name] = nc.dram_tensor(name, shape, F32, kind="ExternalInput")

    # per-core output: its 4 owned token tiles, row qc*128+p = token
    # tile (qc*4 + rank), host reassembles.
    out_y = nc.dram_tensor("out", (NQC * P, c_dim), F32, kind="ExternalOutput")

    # collective payloads travel in bf16: halves the wire time; the partial
    # projections are O(1)-magnitude so the rounding is ~1e-3 relative.
    ARDT = BF16
    rs_in = [nc.dram_tensor(f"rs_in{i}", (QCH, c_dim), ARDT) for i in range(NQC)]
    rs_out = [nc.dram_tensor(f"rs_out{i}", (P, c_dim), ARDT) for i in range(NQC)]
    # DRAM bounce rows for the softmax-denominator partition broadcast
    recip_d = nc.dram_tensor("recip_d", (NQC * h_core, QCH), F32)

    with TileContext(nc) as tc, contextlib.ExitStack() as ctx:
        const = ctx.enter_context(tc.tile_pool(name="const", bufs=1))
        workb = ctx.enter_context(tc.tile_pool(name="workb", bufs=2))
        works = ctx.enter_context(tc.tile_pool(name="works", bufs=3))
        stats = ctx.enter_context(tc.tile_pool(name="stats", bufs=6))

        ident = const.tile([P, P], BF16)
        make_identity(nc, ident)
        eps_t = const.tile([P, 1], F32)
        nc.vector.memset(eps_t[:], EPS)
        tri_sb = const.tile([P, P], F32)
        nc.sync.dma_start(tri_sb[:], tri[:])

        # optional affine operands, broadcast to 128 partitions once
        def _bcast_row(name, width):
            if name not in opt_in:
                return None
            bc = const.tile([P, width], F32, name=f"bc_{name}", tag=f"bc_{name}")
            nc.sync.dma_start(bc[:], opt_in[name][:].to_broadcast((P, width)))
            return bc

        def _col(name):
            if name not in opt_in:
                return None
            t_ = const.tile(list(opt_in[name].shape), F32, name=f"col_{name}", tag=f"col_{name}")
            nc.sync.dma_start(t_[:], opt_in[name][:])
            return t_

        ln1_g_bc = _bcast_row("ln1_g", c_dim)
        ln1_b_bc = _bcast_row("ln1_b", c_dim)
        ln2_g_bc = _bcast_row("ln2_g", c_dim)
        ln2_b_bc = _bcast_row("ln2_b", c_dim)
        b_v_bc = _bcast_row("b_v", h_core * HD)
        b_ap_bc = _bcast_row("b_ap", c_dim)
        b_fc_bc = _bcast_row("b_fc", fh)
        b_mp_bc = _bcast_row("b_mp", c_dim)
        b_qk_col = _col("b_qk")

        ps_tr = ctx.enter_context(tc.tile_pool(name="ps_tr", bufs=2, space="PSUM"))
        ps_mm = ctx.enter_context(tc.tile_pool(name="ps_mm", bufs=4, space="PSUM"))
        ps_y = ctx.enter_context(tc.tile_pool(name="ps_y", bufs=2, space="PSUM"))

        def ln_tile(x_f32, g_bc, b_bc):
            """LayerNorm of a [P, c_dim] fp32 AP -> new [P, c_dim] bf16 tile.
            rsqrt via Ln+Exp: shares the Scalar activation table with the
            softmax Exp, so no table reloads between LN and attention."""
            nsub = c_dim // 512
            st = stats.tile([P, nsub, 6], F32)
            for j in range(nsub):
                nc.vector.bn_stats(st[:, j, :], x_f32[:, j * 512:(j + 1) * 512])
            mv = stats.tile([P, 2], F32)
            nc.vector.bn_aggr(mv[:], st[:])
            r = stats.tile([P, 1], F32)
            nc.scalar.activation(
                r[:], mv[:, 1:2], mybir.ActivationFunctionType.Ln, bias=eps_t[:]
            )
            nc.scalar.activation(
                r[:], r[:], mybir.ActivationFunctionType.Exp, scale=-0.5
            )
            if g_bc is None and b_bc is None:
                h_bf = works.tile([P, c_dim], BF16, tag="ln_out", name="h_bf")
                nc.vector.tensor_scalar(
                    out=h_bf[:], in0=x_f32[:], scalar1=mv[:, 0:1], scalar2=r[:],
                    op0=mybir.AluOpType.subtract, op1=mybir.AluOpType.mult,
                )
            else:
                h_f = workb.tile([P, c_dim], F32, tag="ln_f32", name="h_f")
                nc.vector.tensor_scalar(
                    out=h_f[:], in0=x_f32[:], scalar1=mv[:, 0:1], scalar2=r[:],
                    op0=mybir.AluOpType.subtract, op1=mybir.AluOpType.mult,
                )
                if g_bc is not None:
                    nc.vector.tensor_mul(h_f[:], h_f[:], g_bc[:])
                if b_bc is not None:
                    nc.vector.tensor_add(h_f[:], h_f[:], b_bc[:])
                h_bf = works.tile([P, c_dim], BF16, tag="ln_out", name="h_bf")
                nc.vector.tensor_copy(h_bf[:], h_f[:])
            return h_bf

        def transpose_into(dstT, src_bf, tl, n_k):
            """PE-transpose [P, n_k*128] bf16 into dstT[:, :, tl*P:(tl+1)*P].
            Four 128x128 transposes share one PSUM bank so a single DVE copy
            evacuates them."""
            for kg in range(0, n_k, 4):
                nb = min(4, n_k - kg)
                pst = ps_tr.tile([P, 4 * P], BF16, tag="pst", name="pst")
                for j in range(nb):
                    nc.tensor.transpose(
                        pst[:, j * P:(j + 1) * P],
                        src_bf[:, (kg + j) * P:(kg + j + 1) * P],
                        ident[:],
                    )
                nc.vector.tensor_copy(
                    dstT[:, kg:kg + nb, tl * P:(tl + 1) * P],
                    pst[:, 0:nb * P].rearrange("p (a b) -> p a b", a=nb),
                )

        # =============== the MLP weight pools (persistent) =================
        # w_fcT preloads during attention (SBUF has room); w_mpT loads into
        # the space the attention pools free, overlapping the fc stages.
        wfc_pool = ctx.enter_context(tc.tile_pool(name="wfc", bufs=1))
        h2p = ctx.enter_context(tc.tile_pool(name="h2p", bufs=4))

        # ======================= attention phase ===========================
        with contextlib.ExitStack() as attn_ctx:
            # weights go on the Activation engine's DMA queue so the big
            # transfers never head-of-line-block the latency-critical
            # activation loads on the sync queue.
            wa = attn_ctx.enter_context(tc.tile_pool(name="wa", bufs=1))
            w_qk_sb = wa.tile([P, KO, QK], BF16)
            nc.scalar.dma_start(w_qk_sb[:], w_qk[:])
            w_v_sb = wa.tile([P, KO, h_core * HD], BF16)
            nc.scalar.dma_start(w_v_sb[:], w_v[:])
            w_ap_sb = wa.tile([P, DKO, c_dim], BF16)
            nc.scalar.dma_start(w_ap_sb[:], w_ap[:])

            big = attn_ctx.enter_context(tc.tile_pool(name="big", bufs=1))
            qkT = big.tile([P, MQK, t_len], BF16)
            vaug = big.tile([P, NT, h_core, 65], BF16)
            nc.vector.memset(vaug[:, :, :, 64:65], 1.0)

            h1p = attn_ctx.enter_context(tc.tile_pool(name="h1p", bufs=2))
            yp = attn_ctx.enter_context(tc.tile_pool(name="yp", bufs=2))
            pt_pool = attn_ctx.enter_context(tc.tile_pool(name="pt", bufs=6))
            rowp = attn_ctx.enter_context(tc.tile_pool(name="rows", bufs=2))
            arp = attn_ctx.enter_context(tc.tile_pool(name="arp", bufs=2))

            w_fc_sb = wfc_pool.tile([P, KO, fh], BF16)

            def stage_A_pre(qc):
                """LN1 + transposed activations for chunk qc (DVE/Scalar
                heavy; emitted a chunk ahead so the PE never waits on it)."""
                h1T = h1p.tile([P, KO, QCH], BF16, tag="h1T", name="h1T")
                for tl in range(KPQ):
                    tt = qc * KPQ + tl
                    xt = workb.tile([P, c_dim], F32, tag="x_in", name="xt")
                    nc.sync.dma_start(xt[:], x_tm[tt * P:(tt + 1) * P, :])
                    h_bf = ln_tile(xt, ln1_g_bc, ln1_b_bc)
                    transpose_into(h1T, h_bf, tl, KO)
                return h1T

            def stage_A(qc, h1T):
                # qkT chunk (transposed-output form)
                for mt in range(MQK):
                    ps = ps_mm.tile([P, QCH], F32, tag="ps", name="ps")
                    for ko in range(KO):
                        nc.tensor.matmul(
                            ps[:],
                            w_qk_sb[:, ko, mt * P:(mt + 1) * P],
                            h1T[:, ko, :],
                            start=(ko == 0),
                            stop=(ko == KO - 1),
                        )
                    dst = qkT[:, mt, qc * QCH:(qc + 1) * QCH]
                    if b_qk_col is not None:
                        nc.vector.tensor_scalar_add(dst, ps[:], b_qk_col[:, mt:mt + 1])
                    else:
                        nc.any.tensor_copy(out=dst, in_=ps[:])

                # v token-major for this chunk's tiles
                for tl in range(KPQ):
                    tt = qc * KPQ + tl
                    ps = ps_mm.tile([P, QCH], F32, tag="ps", name="ps")
                    for ko in range(KO):
                        nc.tensor.matmul(
                            ps[:, 0:h_core * HD],
                            h1T[:, ko, tl * P:(tl + 1) * P],
                            w_v_sb[:, ko, :],
                            start=(ko == 0),
                            stop=(ko == KO - 1),
                        )
                    if b_v_bc is not None:
                        nc.vector.tensor_add(
                            ps[:, 0:h_core * HD], ps[:, 0:h_core * HD], b_v_bc[:]
                        )
                    for h in range(h_core):
                        nc.any.tensor_copy(
                            out=vaug[:, tt, h, 0:64], in_=ps[:, h * HD:(h + 1) * HD]
                        )

                # causal attention, one head-pair at a time
                yT = yp.tile([P, DKO, QCH], BF16, tag="yT", name="yT")
                for pr in range(NPAIR):
                    heads = ((0, 2 * pr), (64, 2 * pr + 1))
                    psy = {}
                    for sub, h in heads:
                        psy[h] = ps_y.tile([P, QCH], F32, tag="psy", name=f"psy_{h}")
                    nkt = (qc + 1) * KPQ
                    pts = {}

                    def emit_s_exp(kt, heads=heads, pts=pts, qc=qc, pr=pr):
                        i = kt - qc * KPQ  # >=0 on the diagonal band
                        for sub, h in heads:
                            pss = ps_mm.tile([P, QCH], F32, tag="ps", name=f"pss_{h}")
                            nc.tensor.matmul(
                                pss[:],
                                qkT[sub:sub + 64, DKO + pr, kt * P:(kt + 1) * P],
                                qkT[sub:sub + 64, pr, qc * QCH:(qc + 1) * QCH],
                                start=True,
                                stop=True,
                            )
                            pt = pt_pool.tile([P, QCH], BF16, tag="pt", name=f"pt_{h}")
                            if i >= 0:
                                if i > 0:
                                    nc.vector.memset(pt[:, 0:i * P], 0.0)
                                nc.vector.tensor_add(
                                    pss[:, i * P:(i + 1) * P],
                                    pss[:, i * P:(i + 1) * P],
                                    tri_sb[:],
                                )
                                nc.scalar.activation(
                                    pt[:, i * P:QCH],
                                    pss[:, i * P:QCH],
                                    mybir.ActivationFunctionType.Exp,
                                )
                            else:
                                nc.scalar.activation(
                                    pt[:], pss[:], mybir.ActivationFunctionType.Exp
                                )
                            pts[(kt, h)] = pt

                    def emit_pv(kt, heads=heads, psy=psy, pts=pts, nkt=nkt):
                        for sub, h in heads:
                            nc.tensor.matmul(
                                psy[h][0:65, :],
                                vaug[:, kt, h, :],
                                pts.pop((kt, h))[:],
                                start=(kt == 0),
                                stop=(kt == nkt - 1),
                            )

                    for kt in range(nkt):
                        emit_s_exp(kt)
                        if kt > 0:
                            emit_pv(kt - 1)
                    emit_pv(nkt - 1)
                    for sub, h in heads:
                        row = rowp.tile([1, QCH], F32, tag="rec", name="row")
                        nc.scalar.copy(row[:], psy[h][64:65, :])
                        bc_sb = rowp.tile([64, QCH], F32, tag="bc_sb", name="bc_sb")
                        rd = recip_d[qc * h_core + h:qc * h_core + h + 1, :]
                        nc.sync.dma_start(rd, row[:])
                        nc.sync.dma_start(bc_sb[:], rd.to_broadcast((64, QCH)))
                        # reciprocal on the 64-partition broadcast: ~25x
                        # faster on DVE than on the 1-partition row
                        nc.vector.reciprocal(bc_sb[:], bc_sb[:])
                        nc.vector.tensor_tensor(
                            yT[sub:sub + 64, pr, :],
                            psy[h][0:64, :],
                            bc_sb[:],
                            mybir.AluOpType.mult,
                        )

                # out-projection partials -> one staging tile -> one DMA
                ar_st = arp.tile([P, KPQ, c_dim], ARDT, tag="ar_st", name="ar_st")
                for tl in range(KPQ):
                    for nch in range(NCC):
                        ps = ps_mm.tile([P, QCH], F32, tag="ps", name="ps")
                        for dk in range(DKO):
                            nc.tensor.matmul(
                                ps[:],
                                yT[:, dk, tl * P:(tl + 1) * P],
                                w_ap_sb[:, dk, nch * QCH:(nch + 1) * QCH],
                                start=(dk == 0),
                                stop=(dk == DKO - 1),
                            )
                        dst = ar_st[:, tl, nch * QCH:(nch + 1) * QCH]
                        if b_ap_bc is not None:
                            nc.vector.tensor_add(
                                dst, ps[:], b_ap_bc[:, nch * QCH:(nch + 1) * QCH]
                            )
                        else:
                            nc.any.tensor_copy(out=dst, in_=ps[:])
                nc.sync.dma_start(
                    rs_in[qc][:].rearrange("(a p) c -> p a c", p=P), ar_st[:]
                )
                if local_reduce:
                    nc.sync.dma_start(rs_out[qc][:], rs_in[qc][0:P, :])
                else:
                    nc.gpsimd.collective_compute(
                        "ReduceScatter",
                        mybir.AluOpType.add,
                        replica_groups=replica_groups,
                        ins=[rs_in[qc][:]],
                        outs=[rs_out[qc][:]],
                    )

            def x1_tile(qc):
                """x1 = x_own + attn for owned tile of chunk qc, fp32.
                Reads go on the gpsimd software-DGE queue: they depend on the
                ReduceScatter, and on the in-order sync queue they would
                head-of-line-block the attention x loads behind them."""
                xt = workb.tile([P, c_dim], F32, tag="x1f", name="xt")
                nc.gpsimd.dma_start(xt[:], x_own[qc * P:(qc + 1) * P, :])
                at = workb.tile([P, c_dim], ARDT, tag="ar_rd", name="at")
                nc.gpsimd.dma_start(at[:], rs_out[qc][:])
                nc.vector.tensor_add(xt[:], xt[:], at[:])
                return xt

            def stage_F_pre(qc):
                """x1 + LN2 + transposed h2 for the owned tile of chunk qc;
                emitted inside the attention pipeline right after the
                chunk's ReduceScatter result lands."""
                x1 = x1_tile(qc)
                h_bf = ln_tile(x1, ln2_g_bc, ln2_b_bc)
                h2T = h2p.tile([P, KO, P], BF16, tag="h2T", name="h2T")
                transpose_into(h2T, h_bf, 0, KO)
                return h2T

            h1s = {}
            h2s = {}
            h1s[0] = stage_A_pre(0)
            h1s[1] = stage_A_pre(1)
            stage_A(0, h1s.pop(0))
            h1s[2] = stage_A_pre(2)
            stage_A(1, h1s.pop(1))
            # preload the full fc weight mid-attention: late enough not to
            # contend for HBM with the startup x/weight loads, early enough
            # to land long before the fc stages need it.
            nc.scalar.dma_start(w_fc_sb[:], w_fcT[:])
            h2s[0] = stage_F_pre(0)
            h1s[3] = stage_A_pre(3)
            stage_A(2, h1s.pop(2))
            h2s[1] = stage_F_pre(1)
            stage_A(3, h1s.pop(3))
            h2s[2] = stage_F_pre(2)
            h2s[3] = stage_F_pre(3)

        # ======================== MLP phase ================================
        # token-parallel over the 4 owned tiles; full weights, no collective
        wmp_pool = ctx.enter_context(tc.tile_pool(name="wmp", bufs=1))
        w_mp_sb = wmp_pool.tile([P, FKO, c_dim], BF16)
        nc.scalar.dma_start(w_mp_sb[:], w_mpT[:])

        gsp = ctx.enter_context(tc.tile_pool(name="gsp", bufs=2))
        gtp = ctx.enter_context(tc.tile_pool(name="gtp", bufs=2))

        def stage_F(qc, h2T):
            """fc + gelu + transpose for the owned tile of chunk qc."""
            g_s = gsp.tile([P, NHC, QCH], BF16, tag="g_s", name="g_s")
            # two PSUM half-rounds of 4 h-chunks: stationary h2T[ko] is
            # amortized over 4 moving-512 matmuls per load
            for half in range(2):
                pss = [
                    ps_mm.tile([P, QCH], F32, tag="ps", name=f"psf_{i}")
                    for i in range(4)
                ]
                for ko in range(KO):
                    for i in range(4):
                        hc = half * 4 + i
                        nc.tensor.matmul(
                            pss[i][:],
                            h2T[:, ko, :],
                            w_fc_sb[:, ko, hc * QCH:(hc + 1) * QCH],
                            start=(ko == 0),
                            stop=(ko == KO - 1),
                        )
                for i in range(4):
                    hc = half * 4 + i
                    if b_fc_bc is not None:
                        nc.vector.tensor_add(
                            pss[i][:], pss[i][:],
                            b_fc_bc[:, hc * QCH:(hc + 1) * QCH],
                        )
                    nc.scalar.activation(
                        g_s[:, hc, :], pss[i][:],
                        mybir.ActivationFunctionType.Gelu_apprx_tanh,
                    )

            gT = gtp.tile([P, FKO, P], BF16, tag="gT", name="gT")
            for hc in range(NHC):
                transpose_into(gT[:, hc * 4:(hc + 1) * 4, :], g_s[:, hc, :], 0, 4)
            return gT

        def stage_M(qc, gT):
            """Down-projection + final residual + store for chunk qc."""
            x1 = x1_tile(qc)
            for nch in range(NCC):
                ps = ps_mm.tile([P, QCH], F32, tag="ps", name="ps")
                for hk in range(FKO):
                    nc.tensor.matmul(
                        ps[:],
                        gT[:, hk, :],
                        w_mp_sb[:, hk, nch * QCH:(nch + 1) * QCH],
                        start=(hk == 0),
                        stop=(hk == FKO - 1),
                    )
                if b_mp_bc is not None:
                    nc.vector.tensor_add(
                        ps[:], ps[:], b_mp_bc[:, nch * QCH:(nch + 1) * QCH]
                    )
                ev = works.tile([P, QCH], F32, tag="evac", name="ev")
                nc.vector.tensor_tensor(
                    ev[:], ps[:], x1[:, nch * QCH:(nch + 1) * QCH],
                    mybir.AluOpType.add,
                )
                nc.sync.dma_start(
                    out_y[qc * P:(qc + 1) * P, nch * QCH:(nch + 1) * QCH], ev[:]
                )

        gts = {}
        gts[0] = stage_F(0, h2s.pop(0))
        gts[1] = stage_F(1, h2s.pop(1))
        stage_M(0, gts.pop(0))
        gts[2] = stage_F(2, h2s.pop(2))
        stage_M(1, gts.pop(1))
        gts[3] = stage_F(3, h2s.pop(3))
        stage_M(2, gts.pop(2))
        stage_M(3, gts.pop(3))

    if legalize:
        _legalize_waits(nc)
    return nc


# ---------------------------------------------------------------------------
# host-side sharding / layout prep


def _tile_k(arr, width):
    """[K, M] -> [128, K//128, M] (contraction dim inner on partitions)."""
    k, m = arr.shape
    assert m == width and k % P == 0
    return np.ascontiguousarray(
        arr.reshape(k // P, P, m).transpose(1, 0, 2)
    )


def _bf(arr):
    return arr.astype(ml_dtypes.bfloat16)


def make_core_inputs(inputs, t_len=T, c_dim=C, h_core=H_CORE, fh=FH,
                     n_groups=len(GROUPS), tpg=TPG):
    """Shard + lay out the full inputs into per-core input dicts and the
    active-flag set."""
    f32 = np.float32
    x = np.asarray(inputs["x"], f32)
    W_attn = np.asarray(inputs["W_attn"], f32)
    W_aproj = np.asarray(inputs["W_aproj"], f32)
    W_fc = np.asarray(inputs["W_fc"], f32)
    W_mproj = np.asarray(inputs["W_mproj"], f32)
    ln1_g = np.asarray(inputs["ln1_g"], f32)
    ln1_b = np.asarray(inputs["ln1_b"], f32)
    ln2_g = np.asarray(inputs["ln2_g"], f32)
    ln2_b = np.asarray(inputs["ln2_b"], f32)
    b_attn = np.asarray(inputs["b_attn"], f32)
    b_aproj = np.asarray(inputs["b_aproj"], f32)
    b_fc = np.asarray(inputs["b_fc"], f32)
    b_mproj = np.asarray(inputs["b_mproj"], f32)

    Wq, Wk, Wv = W_attn[:c_dim], W_attn[c_dim:2 * c_dim], W_attn[2 * c_dim:]
    bq, bk, bv = b_attn[:c_dim], b_attn[c_dim:2 * c_dim], b_attn[2 * c_dim:]
    scale = 1.0 / math.sqrt(HD)

    flags = set()
    if not np.all(ln1_g == 1.0):
        flags.add("ln1_g")
    if np.any(ln1_b):
        flags.add("ln1_b")
    if not np.all(ln2_g == 1.0):
        flags.add("ln2_g")
    if np.any(ln2_b):
        flags.add("ln2_b")
    if np.any(b_attn[:2 * c_dim]):
        flags.add("b_qk")
    if np.any(bv):
        flags.add("b_v")
    if np.any(b_aproj):
        flags.add("b_ap")
    if np.any(b_fc):
        flags.add("b_fc")
    if np.any(b_mproj):
        flags.add("b_mp")

    tri = np.where(
        np.arange(P)[:, None] > np.arange(P)[None, :], f32(-1e30), f32(0.0)
    ).astype(f32)

    # replicated full MLP weights, transposed layouts (contraction inner)
    w_fcT_full = _tile_k(_bf(np.ascontiguousarray(W_fc.T)), fh)
    w_mpT_full = _tile_k(_bf(np.ascontiguousarray(W_mproj.T)), c_dim)

    NQC = t_len // QCH

    in_maps = []
    for core in range(n_groups * tpg):
        g, s = core // tpg, core % tpg
        heads = range(s * h_core, (s + 1) * h_core)
        # stacked [q heads | k heads] output dims, q pre-scaled by 1/sqrt(hd)
        w_qk_rows = np.concatenate(
            [Wq[h * HD:(h + 1) * HD] * scale for h in heads]
            + [Wk[h * HD:(h + 1) * HD] for h in heads], axis=0
        )  # [QK, C]
        w_v_rows = np.concatenate(
            [Wv[h * HD:(h + 1) * HD] for h in heads], axis=0
        )  # [DH, C]
        dsl = slice(s * h_core * HD, (s + 1) * h_core * HD)
        xg = x[g % x.shape[0]]
        x_own = np.concatenate(
            [xg[(qc * tpg + s) * P:(qc * tpg + s + 1) * P] for qc in range(NQC)],
            axis=0,
        )
        m = {
            "x_tm": np.ascontiguousarray(xg),
            "x_own": np.ascontiguousarray(x_own),
            "w_qk": _tile_k(_bf(w_qk_rows.T), h_core * P),
            "w_v": _tile_k(_bf(w_v_rows.T), h_core * HD),
            "w_ap": _tile_k(_bf(W_aproj[:, dsl].T.copy()), c_dim),
            "w_fcT": w_fcT_full,
            "w_mpT": w_mpT_full,
            "tri": tri,
        }
        if "ln1_g" in flags:
            m["ln1_g"] = ln1_g.reshape(1, -1).copy()
        if "ln1_b" in flags:
            m["ln1_b"] = ln1_b.reshape(1, -1).copy()
        if "ln2_g" in flags:
            m["ln2_g"] = ln2_g.reshape(1, -1).copy()
        if "ln2_b" in flags:
            m["ln2_b"] = ln2_b.reshape(1, -1).copy()
        if "b_qk" in flags:
            b_qk_rows = np.concatenate(
                [bq[h * HD:(h + 1) * HD] * scale for h in heads]
                + [bk[h * HD:(h + 1) * HD] for h in heads]
            )  # [QK] along partitions: [P, MQK]
            m["b_qk"] = np.ascontiguousarray(
                b_qk_rows.reshape(h_core, P).T
            )
        if "b_v" in flags:
            m["b_v"] = np.concatenate(
                [bv[h * HD:(h + 1) * HD] for h in heads]
            ).reshape(1, -1).copy()
        if "b_ap" in flags:
            m["b_ap"] = (b_aproj / tpg).reshape(1, -1).copy()
        if "b_fc" in flags:
            m["b_fc"] = b_fc.reshape(1, -1).copy()
        if "b_mp" in flags:
            m["b_mp"] = b_mproj.reshape(1, -1).copy()
        in_maps.append(m)
    return in_maps, frozenset(flags)


# ---------------------------------------------------------------------------
# runner

_module_cache = {}


def run(inputs, trace=False, trace_kwargs=None, tmpdir=None):
    in_maps, flags = make_core_inputs(inputs)
    key = (flags, trace)
    if key not in _module_cache:
        _module_cache[key] = build_module(flags=flags)
    nc = _module_cache[key]
    if trace:
        _install_prof_hook()
    res = run_bass_kernel_spmd(
        nc,
        in_maps,
        core_ids=list(range(N_CORES)),
        trace=trace,
        tmpdir=tmpdir,
        **(trace_kwargs or {}),
    )
    # reassemble: core g*TPG+s provides token tiles (qc*TPG + s) of batch g
    NQC = T // QCH
    out = np.empty((B, T, C), np.float32)
    for g in range(len(GROUPS)):
        for s in range(TPG):
            o = res.results[g * TPG + s]["out"]
            for qc in range(NQC):
                tt = qc * TPG + s
                out[g, tt * P:(tt + 1) * P, :] = o[qc * P:(qc + 1) * P, :]
    return out, res


def kernel(**inputs) -> np.ndarray:
    out, _ = run(inputs, trace=False)
    return out


# revision 18
# speedup vs baseline: 1.0383x; 1.0383x over previous
"""Fused causal-transformer block (LN1 -> attn -> LN2 -> MLP, residuals) on
8 Trainium2 NeuronCores.

Sharding: 2 groups of 4 cores; group g handles batch element b=g (data
parallel).  Within a group:
  - Attention is Megatron head-parallel: core s owns 4 heads, computes
    partial y = attn(x) @ W_aproj_s for ALL tokens, chunked over four
    512-token chunks.  Each chunk's partials are summed with an in-group
    ReduceScatter, leaving core s with the summed attention output for
    token tile (chunk*4 + s) -- its 128-token slice of each chunk.
  - The MLP is token-parallel with REPLICATED weights: core s runs the
    full 4C-hidden MLP for its 4 owned token tiles (512 tokens total).
    No second collective is needed; the host reassembles token slices.
This cuts the collective count from 8 AllReduces to 4 ReduceScatters,
all hidden behind attention compute (the single CC core was the
bottleneck of the AllReduce design).

Compute dtype: bf16 matmul inputs, fp32 PSUM accumulation, fp32 residual
stream and softmax statistics.

Layouts (per core, all prepared host-side in kernel()):
  h1T/h2T  : [128, C/128, t]  activations transposed (contraction dim on
             partitions) produced on-device via PE transposes.
  qkT      : [128, H_core, T] rows = [q heads | k heads] * 64-dim each,
             two heads stacked per 128-partition tile.  Scores are
             computed directly in S^T [k, q] layout, so softmax
             normalization arrives as a PSUM row via a ones-column in v.
  v_aug    : [128, T/128, H_core, 65]  v token-major per head + ones col.
  w_fcT    : [128, C/128, 4C]  full W_fc^T (replicated), moving operand.
  w_mpT    : [128, 4C/128, C]  full W_mproj^T (replicated), moving.
"""

import contextlib
import ctypes
import math
import sys
import types

import numpy as np
import ml_dtypes

import bass_rust
import concourse.bass as bass
import concourse.mybir as mybir
import concourse.tile as tile
from concourse import library_config
from concourse.bass_utils import run_bass_kernel_spmd
from concourse.masks import make_identity
from concourse.tile import TileContext
from concourse.vector_clock import ScopedClock

# ---------------------------------------------------------------------------
# problem constants (hardcoded per the harness contract)
B, T, C, H = 2, 2048, 1024, 16
HD = C // H                 # 64
N_CORES = 8
TPG = 4                     # tensor-parallel group size
H_CORE = H // TPG           # heads per core = 4
DH = H_CORE * HD            # per-core attention dim = 256
FH = 4 * C                  # full MLP hidden (replicated) = 4096
P = 128
EPS = 1e-5
QCH = 512                   # q-chunk width
GROUPS = [[0, 1, 2, 3], [4, 5, 6, 7]]

F32 = mybir.dt.float32
BF16 = mybir.dt.bfloat16

# ---------------------------------------------------------------------------
# workaround 1: the container's walrus accepts a single sync-wait command per
# instruction; move extra semaphore waits onto inserted EventSemaphore
# instructions on the same engine (program order preserves semantics).

_waitfix_counter = [0]


def _legalize_waits(nc, cap=1):
    fn = nc.m.functions[0]
    n_split = 0
    for bb in fn.blocks:
        out = []
        changed = False
        for inst in bb.instructions:
            si = inst.sync_info
            waits = list(si.on_wait) if si is not None else []
            if len(waits) > cap:
                movable = [w for w in waits if w.sync_type == "semaphore"]
                fixed = [w for w in waits if w.sync_type != "semaphore"]
                n_keep = max(cap - len(fixed), 0)
                keep = fixed + (movable[len(movable) - n_keep:] if n_keep else [])
                extra = movable[: len(movable) - n_keep] if n_keep else movable
                for w in extra:
                    _waitfix_counter[0] += 1
                    ev = mybir.InstEventSemaphore(
                        name=f"I-waitfix-{_waitfix_counter[0]}",
                        engine=inst.engine,
                        ins=[],
                        outs=[],
                        sync_info=bass_rust.SyncInfo(on_wait=[w], on_update=[]),
                    )
                    out.append(ev)
                    n_split += 1
                inst.sync_info = bass_rust.SyncInfo(
                    on_wait=keep, on_update=list(si.on_update)
                )
                changed = True
            out.append(inst)
        if changed:
            bb.instructions = out
    return n_split


# workaround 2: same issue for the Tile kernel-tail Drain — emit one wait-nop
# per live proc ahead of a wait-less drain instead of stacking waits on it.


def _drain_and_barrier_split(self, tick_clock, wait_clock):
    gc = tick_clock.global_clock
    sems_alloc = wait_clock.sems.allocated()
    for proc in sorted(sems_alloc):
        tick = gc.peek_next(proc) - 1
        if tick <= 0:
            continue
        vc1 = bass_rust.VectorClock()
        vc1.require_at_least(proc, tick)
        nop = self.nc.sync.nop()
        wait_clock.add_sem_waits(nop.ins, ScopedClock({None: vc1}))
    self.nc.sync.drain()
    self.nc.all_engine_barrier()
    assert self.sems is not None
    popped = self.nc._tile_sem_poison_stack.pop()
    assert popped is self._sem_poison
    self.nc.clear_and_free_semaphores(list(self.sems.allocated().values()))
    self.nc.all_engine_barrier()


TileContext._drain_and_barrier = _drain_and_barrier_split


# workaround 3 (profiling only): register the NTFF hook the trimmed antenv
# lacks so run_bass_kernel_spmd(trace=True) works under axon.


def _install_prof_hook():
    if "antenv.axon_hooks" in sys.modules:
        return
    so_path = "/opt/axon/libaxon_pjrt.so"
    hook = None
    try:
        lib = ctypes.CDLL(so_path)
        if hasattr(lib, "axon_start_nrt_profile"):
            lib.axon_start_nrt_profile.argtypes = [
                ctypes.POINTER(ctypes.c_int64),
                ctypes.c_size_t,
            ]
            lib.axon_start_nrt_profile.restype = ctypes.c_int64
            lib.axon_stop_nrt_profile.argtypes = [ctypes.c_char_p]
            lib.axon_stop_nrt_profile.restype = ctypes.c_int64

            @contextlib.contextmanager
            def _hook_cm(output_dir, device_ids):
                import jax

                jax.devices()
                if device_ids:
                    ids = (ctypes.c_int64 * len(device_ids))(*device_ids)
                    rc = lib.axon_start_nrt_profile(ids, len(device_ids))
                else:
                    rc = lib.axon_start_nrt_profile(None, 0)
                if rc != 0:
                    raise RuntimeError(f"axon_start_nrt_profile rc={rc}")
                try:
                    yield
                finally:
                    n = lib.axon_stop_nrt_profile(str(output_dir).encode())
                    if n < 0:
                        raise RuntimeError(f"axon_stop_nrt_profile rc={n}")

            hook = _hook_cm
    except OSError:
        pass
    mod = types.ModuleType("antenv.axon_hooks")
    mod.get_axon_ntff_profile_hook = lambda: hook
    mod.set_axon_ntff_profile_hook = lambda h: None
    sys.modules["antenv.axon_hooks"] = mod
    from concourse import bass_utils

    bass_utils.upload_artifacts = lambda tmpdir: tmpdir


# ---------------------------------------------------------------------------
# device kernel builder


def build_module(
    t_len=T,
    c_dim=C,
    h_core=H_CORE,
    fh=FH,
    flags=frozenset(),
    replica_groups=GROUPS,
    local_reduce=False,
    legalize=True,
):
    """Build the per-core SPMD Bass module.

    flags: subset of {"ln1_g","ln1_b","ln2_g","ln2_b","b_qk","b_v","b_ap",
    "b_fc","b_mp"} enabling the non-trivial affine/bias paths.
    local_reduce: replace the in-group ReduceScatter with a local strided
    copy (single core test mode: takes this rank-0 slice).
    """
    KO = c_dim // P             # c-tiles
    NT = t_len // P             # token tiles
    NQC = t_len // QCH          # q chunks
    KPQ = QCH // P              # token tiles per chunk (= group size 4)
    QK = h_core * P             # stacked q+k dims
    MQK = h_core                # m-tiles of qkT
    DKO = (h_core * HD) // P    # d-tiles of y/aproj  (h_core/2)
    FKO = fh // P               # hidden tiles (32)
    NPAIR = h_core // 2
    NCC = c_dim // QCH          # 512-chunks of C
    NHC = fh // QCH             # 512-chunks of hidden (8)
    assert h_core % 2 == 0 and c_dim % P == 0 and t_len % QCH == 0

    nc = bass.Bass(num_devices=N_CORES)

    x_tm = nc.dram_tensor("x_tm", (t_len, c_dim), F32, kind="ExternalInput")
    x_own = nc.dram_tensor("x_own", (NQC * P, c_dim), F32, kind="ExternalInput")
    w_qk = nc.dram_tensor("w_qk", (P, KO, QK), BF16, kind="ExternalInput")
    w_v = nc.dram_tensor("w_v", (P, KO, h_core * HD), BF16, kind="ExternalInput")
    w_ap = nc.dram_tensor("w_ap", (P, DKO, c_dim), BF16, kind="ExternalInput")
    w_fcT = nc.dram_tensor("w_fcT", (P, KO, fh), BF16, kind="ExternalInput")
    w_mpT = nc.dram_tensor("w_mpT", (P, FKO, c_dim), BF16, kind="ExternalInput")
    tri = nc.dram_tensor("tri", (P, P), F32, kind="ExternalInput")
    opt_in = {}
    for name, shape in [
        ("ln1_g", (1, c_dim)), ("ln1_b", (1, c_dim)),
        ("ln2_g", (1, c_dim)), ("ln2_b", (1, c_dim)),
        ("b_qk", (P, MQK)), ("b_v", (1, h_core * HD)), ("b_ap", (1, c_dim)),
        ("b_fc", (1, fh)), ("b_mp", (1, c_dim)),
    ]:
        if name in flags:
            opt_in[name] = nc.dram_tensor(name, shape, F32, kind="ExternalInput")

    # per-core output: its 4 owned token tiles, row qc*128+p = token
    # tile (qc*4 + rank), host reassembles.
    out_y = nc.dram_tensor("out", (NQC * P, c_dim), F32, kind="ExternalOutput")

    # collective payloads travel in bf16: halves the wire time; the partial
    # projections are O(1)-magnitude so the rounding is ~1e-3 relative.
    ARDT = BF16
    rs_in = [nc.dram_tensor(f"rs_in{i}", (QCH, c_dim), ARDT) for i in range(NQC)]
    rs_out = [nc.dram_tensor(f"rs_out{i}", (P, c_dim), ARDT) for i in range(NQC)]
    # DRAM bounce rows for the softmax-denominator partition broadcast
    recip_d = nc.dram_tensor("recip_d", (NQC * h_core, QCH), F32)

    with TileContext(nc) as tc, contextlib.ExitStack() as ctx:
        const = ctx.enter_context(tc.tile_pool(name="const", bufs=1))
        workb = ctx.enter_context(tc.tile_pool(name="workb", bufs=2))
        works = ctx.enter_context(tc.tile_pool(name="works", bufs=3))
        stats = ctx.enter_context(tc.tile_pool(name="stats", bufs=6))

        ident = const.tile([P, P], BF16)
        make_identity(nc, ident)
        eps_t = const.tile([P, 1], F32)
        nc.vector.memset(eps_t[:], EPS)
        tri_sb = const.tile([P, P], F32)
        nc.sync.dma_start(tri_sb[:], tri[:])

        # optional affine operands, broadcast to 128 partitions once
        def _bcast_row(name, width):
            if name not in opt_in:
                return None
            bc = const.tile([P, width], F32, name=f"bc_{name}", tag=f"bc_{name}")
            nc.sync.dma_start(bc[:], opt_in[name][:].to_broadcast((P, width)))
            return bc

        def _col(name):
            if name not in opt_in:
                return None
            t_ = const.tile(list(opt_in[name].shape), F32, name=f"col_{name}", tag=f"col_{name}")
            nc.sync.dma_start(t_[:], opt_in[name][:])
            return t_

        ln1_g_bc = _bcast_row("ln1_g", c_dim)
        ln1_b_bc = _bcast_row("ln1_b", c_dim)
        ln2_g_bc = _bcast_row("ln2_g", c_dim)
        ln2_b_bc = _bcast_row("ln2_b", c_dim)
        b_v_bc = _bcast_row("b_v", h_core * HD)
        b_ap_bc = _bcast_row("b_ap", c_dim)
        b_fc_bc = _bcast_row("b_fc", fh)
        b_mp_bc = _bcast_row("b_mp", c_dim)
        b_qk_col = _col("b_qk")

        ps_tr = ctx.enter_context(tc.tile_pool(name="ps_tr", bufs=2, space="PSUM"))
        ps_mm = ctx.enter_context(tc.tile_pool(name="ps_mm", bufs=4, space="PSUM"))
        ps_y = ctx.enter_context(tc.tile_pool(name="ps_y", bufs=2, space="PSUM"))

        def ln_tile(x_f32, g_bc, b_bc):
            """LayerNorm of a [P, c_dim] fp32 AP -> new [P, c_dim] bf16 tile.
            rsqrt via Ln+Exp: shares the Scalar activation table with the
            softmax Exp, so no table reloads between LN and attention."""
            nsub = c_dim // 512
            st = stats.tile([P, nsub, 6], F32)
            for j in range(nsub):
                nc.vector.bn_stats(st[:, j, :], x_f32[:, j * 512:(j + 1) * 512])
            mv = stats.tile([P, 2], F32)
            nc.vector.bn_aggr(mv[:], st[:])
            r = stats.tile([P, 1], F32)
            nc.scalar.activation(
                r[:], mv[:, 1:2], mybir.ActivationFunctionType.Ln, bias=eps_t[:]
            )
            nc.scalar.activation(
                r[:], r[:], mybir.ActivationFunctionType.Exp, scale=-0.5
            )
            if g_bc is None and b_bc is None:
                h_bf = works.tile([P, c_dim], BF16, tag="ln_out", name="h_bf")
                nc.vector.tensor_scalar(
                    out=h_bf[:], in0=x_f32[:], scalar1=mv[:, 0:1], scalar2=r[:],
                    op0=mybir.AluOpType.subtract, op1=mybir.AluOpType.mult,
                )
            else:
                h_f = workb.tile([P, c_dim], F32, tag="ln_f32", name="h_f")
                nc.vector.tensor_scalar(
                    out=h_f[:], in0=x_f32[:], scalar1=mv[:, 0:1], scalar2=r[:],
                    op0=mybir.AluOpType.subtract, op1=mybir.AluOpType.mult,
                )
                if g_bc is not None:
                    nc.vector.tensor_mul(h_f[:], h_f[:], g_bc[:])
                if b_bc is not None:
                    nc.vector.tensor_add(h_f[:], h_f[:], b_bc[:])
                h_bf = works.tile([P, c_dim], BF16, tag="ln_out", name="h_bf")
                nc.vector.tensor_copy(h_bf[:], h_f[:])
            return h_bf

        def transpose_into(dstT, src_bf, tl, n_k):
            """PE-transpose [P, n_k*128] bf16 into dstT[:, :, tl*P:(tl+1)*P].
            Four 128x128 transposes share one PSUM bank so a single DVE copy
            evacuates them."""
            for kg in range(0, n_k, 4):
                nb = min(4, n_k - kg)
                pst = ps_tr.tile([P, 4 * P], BF16, tag="pst", name="pst")
                for j in range(nb):
                    nc.tensor.transpose(
                        pst[:, j * P:(j + 1) * P],
                        src_bf[:, (kg + j) * P:(kg + j + 1) * P],
                        ident[:],
                    )
                nc.vector.tensor_copy(
                    dstT[:, kg:kg + nb, tl * P:(tl + 1) * P],
                    pst[:, 0:nb * P].rearrange("p (a b) -> p a b", a=nb),
                )

        # =============== the MLP weight pools (persistent) =================
        # w_fcT preloads during attention (SBUF has room); w_mpT loads into
        # the space the attention pools free, overlapping the fc stages.
        wfc_pool = ctx.enter_context(tc.tile_pool(name="wfc", bufs=1))
        h2p = ctx.enter_context(tc.tile_pool(name="h2p", bufs=4))

        # ======================= attention phase ===========================
        with contextlib.ExitStack() as attn_ctx:
            # weights go on the Activation engine's DMA queue so the big
            # transfers never head-of-line-block the latency-critical
            # activation loads on the sync queue.
            wa = attn_ctx.enter_context(tc.tile_pool(name="wa", bufs=1))
            w_qk_sb = wa.tile([P, KO, QK], BF16)
            nc.scalar.dma_start(w_qk_sb[:], w_qk[:])
            w_v_sb = wa.tile([P, KO, h_core * HD], BF16)
            nc.scalar.dma_start(w_v_sb[:], w_v[:])
            w_ap_sb = wa.tile([P, DKO, c_dim], BF16)
            nc.scalar.dma_start(w_ap_sb[:], w_ap[:])

            big = attn_ctx.enter_context(tc.tile_pool(name="big", bufs=1))
            qkT = big.tile([P, MQK, t_len], BF16)
            vaug = big.tile([P, NT, h_core, 65], BF16)
            nc.vector.memset(vaug[:, :, :, 64:65], 1.0)

            h1p = attn_ctx.enter_context(tc.tile_pool(name="h1p", bufs=2))
            yp = attn_ctx.enter_context(tc.tile_pool(name="yp", bufs=2))
            pt_pool = attn_ctx.enter_context(tc.tile_pool(name="pt", bufs=6))
            rowp = attn_ctx.enter_context(tc.tile_pool(name="rows", bufs=2))
            arp = attn_ctx.enter_context(tc.tile_pool(name="arp", bufs=2))

            w_fc_sb = wfc_pool.tile([P, KO, fh], BF16)

            def stage_A_pre(qc):
                """LN1 + transposed activations for chunk qc (DVE/Scalar
                heavy; emitted a chunk ahead so the PE never waits on it)."""
                h1T = h1p.tile([P, KO, QCH], BF16, tag="h1T", name="h1T")
                for tl in range(KPQ):
                    tt = qc * KPQ + tl
                    xt = workb.tile([P, c_dim], F32, tag="x_in", name="xt")
                    nc.sync.dma_start(xt[:], x_tm[tt * P:(tt + 1) * P, :])
                    h_bf = ln_tile(xt, ln1_g_bc, ln1_b_bc)
                    transpose_into(h1T, h_bf, tl, KO)
                return h1T

            def stage_A(qc, h1T):
                # qkT chunk (transposed-output form)
                for mt in range(MQK):
                    ps = ps_mm.tile([P, QCH], F32, tag="ps", name="ps")
                    for ko in range(KO):
                        nc.tensor.matmul(
                            ps[:],
                            w_qk_sb[:, ko, mt * P:(mt + 1) * P],
                            h1T[:, ko, :],
                            start=(ko == 0),
                            stop=(ko == KO - 1),
                        )
                    dst = qkT[:, mt, qc * QCH:(qc + 1) * QCH]
                    if b_qk_col is not None:
                        nc.vector.tensor_scalar_add(dst, ps[:], b_qk_col[:, mt:mt + 1])
                    else:
                        nc.any.tensor_copy(out=dst, in_=ps[:])

                # v token-major for this chunk's tiles
                for tl in range(KPQ):
                    tt = qc * KPQ + tl
                    ps = ps_mm.tile([P, QCH], F32, tag="ps", name="ps")
                    for ko in range(KO):
                        nc.tensor.matmul(
                            ps[:, 0:h_core * HD],
                            h1T[:, ko, tl * P:(tl + 1) * P],
                            w_v_sb[:, ko, :],
                            start=(ko == 0),
                            stop=(ko == KO - 1),
                        )
                    if b_v_bc is not None:
                        nc.vector.tensor_add(
                            ps[:, 0:h_core * HD], ps[:, 0:h_core * HD], b_v_bc[:]
                        )
                    for h in range(h_core):
                        nc.any.tensor_copy(
                            out=vaug[:, tt, h, 0:64], in_=ps[:, h * HD:(h + 1) * HD]
                        )

                # causal attention, one head-pair at a time
                yT = yp.tile([P, DKO, QCH], BF16, tag="yT", name="yT")
                for pr in range(NPAIR):
                    heads = ((0, 2 * pr), (64, 2 * pr + 1))
                    psy = {}
                    for sub, h in heads:
                        psy[h] = ps_y.tile([P, QCH], F32, tag="psy", name=f"psy_{h}")
                    nkt = (qc + 1) * KPQ
                    pts = {}

                    def emit_s_exp(kt, heads=heads, pts=pts, qc=qc, pr=pr):
                        i = kt - qc * KPQ  # >=0 on the diagonal band
                        for sub, h in heads:
                            pss = ps_mm.tile([P, QCH], F32, tag="ps", name=f"pss_{h}")
                            nc.tensor.matmul(
                                pss[:],
                                qkT[sub:sub + 64, DKO + pr, kt * P:(kt + 1) * P],
                                qkT[sub:sub + 64, pr, qc * QCH:(qc + 1) * QCH],
                                start=True,
                                stop=True,
                            )
                            pt = pt_pool.tile([P, QCH], BF16, tag="pt", name=f"pt_{h}")
                            if i >= 0:
                                if i > 0:
                                    nc.vector.memset(pt[:, 0:i * P], 0.0)
                                nc.vector.tensor_add(
                                    pss[:, i * P:(i + 1) * P],
                                    pss[:, i * P:(i + 1) * P],
                                    tri_sb[:],
                                )
                                nc.scalar.activation(
                                    pt[:, i * P:QCH],
                                    pss[:, i * P:QCH],
                                    mybir.ActivationFunctionType.Exp,
                                )
                            else:
                                nc.scalar.activation(
                                    pt[:], pss[:], mybir.ActivationFunctionType.Exp
                                )
                            pts[(kt, h)] = pt

                    def emit_pv(kt, heads=heads, psy=psy, pts=pts, nkt=nkt):
                        for sub, h in heads:
                            nc.tensor.matmul(
                                psy[h][0:65, :],
                                vaug[:, kt, h, :],
                                pts.pop((kt, h))[:],
                                start=(kt == 0),
                                stop=(kt == nkt - 1),
                            )

                    for kt in range(nkt):
                        emit_s_exp(kt)
                        if kt > 0:
                            emit_pv(kt - 1)
                    emit_pv(nkt - 1)
                    for sub, h in heads:
                        row = rowp.tile([1, QCH], F32, tag="rec", name="row")
                        nc.scalar.copy(row[:], psy[h][64:65, :])
                        bc_sb = rowp.tile([64, QCH], F32, tag="bc_sb", name="bc_sb")
                        rd = recip_d[qc * h_core + h:qc * h_core + h + 1, :]
                        nc.sync.dma_start(rd, row[:])
                        nc.sync.dma_start(bc_sb[:], rd.to_broadcast((64, QCH)))
                        # reciprocal on the 64-partition broadcast: ~25x
                        # faster on DVE than on the 1-partition row
                        nc.vector.reciprocal(bc_sb[:], bc_sb[:])
                        nc.vector.tensor_tensor(
                            yT[sub:sub + 64, pr, :],
                            psy[h][0:64, :],
                            bc_sb[:],
                            mybir.AluOpType.mult,
                        )

                # out-projection partials -> one staging tile -> one DMA
                ar_st = arp.tile([P, KPQ, c_dim], ARDT, tag="ar_st", name="ar_st")
                for tl in range(KPQ):
                    for nch in range(NCC):
                        ps = ps_mm.tile([P, QCH], F32, tag="ps", name="ps")
                        for dk in range(DKO):
                            nc.tensor.matmul(
                                ps[:],
                                yT[:, dk, tl * P:(tl + 1) * P],
                                w_ap_sb[:, dk, nch * QCH:(nch + 1) * QCH],
                                start=(dk == 0),
                                stop=(dk == DKO - 1),
                            )
                        dst = ar_st[:, tl, nch * QCH:(nch + 1) * QCH]
                        if b_ap_bc is not None:
                            nc.vector.tensor_add(
                                dst, ps[:], b_ap_bc[:, nch * QCH:(nch + 1) * QCH]
                            )
                        else:
                            nc.any.tensor_copy(out=dst, in_=ps[:])
                nc.sync.dma_start(
                    rs_in[qc][:].rearrange("(a p) c -> p a c", p=P), ar_st[:]
                )
                if local_reduce:
                    nc.sync.dma_start(rs_out[qc][:], rs_in[qc][0:P, :])
                else:
                    nc.gpsimd.collective_compute(
                        "ReduceScatter",
                        mybir.AluOpType.add,
                        replica_groups=replica_groups,
                        ins=[rs_in[qc][:]],
                        outs=[rs_out[qc][:]],
                    )

            def x1_tile(qc):
                """x1 = x_own + attn for owned tile of chunk qc, fp32.
                Only emitted in the MLP phase: the rs_out read depends on the
                ReduceScatter, and on the in-order sync queue it would
                head-of-line-block any attention x loads behind it."""
                xt = workb.tile([P, c_dim], F32, tag="x1f", name="xt")
                nc.sync.dma_start(xt[:], x_own[qc * P:(qc + 1) * P, :])
                at = workb.tile([P, c_dim], ARDT, tag="ar_rd", name="at")
                nc.sync.dma_start(at[:], rs_out[qc][:])
                nc.vector.tensor_add(xt[:], xt[:], at[:])
                return xt

            def stage_F_pre(qc):
                """x1 + LN2 + transposed h2 for the owned tile of chunk qc."""
                x1 = x1_tile(qc)
                h_bf = ln_tile(x1, ln2_g_bc, ln2_b_bc)
                h2T = h2p.tile([P, KO, P], BF16, tag="h2T", name="h2T")
                transpose_into(h2T, h_bf, 0, KO)
                return h2T

            h1s = {}
            h1s[0] = stage_A_pre(0)
            h1s[1] = stage_A_pre(1)
            stage_A(0, h1s.pop(0))
            h1s[2] = stage_A_pre(2)
            stage_A(1, h1s.pop(1))
            # preload the full fc weight mid-attention: late enough not to
            # contend for HBM with the startup x/weight loads, early enough
            # to land long before the fc stages need it.
            nc.scalar.dma_start(w_fc_sb[:], w_fcT[:])
            h1s[3] = stage_A_pre(3)
            stage_A(2, h1s.pop(2))
            stage_A(3, h1s.pop(3))

        # ======================== MLP phase ================================
        # token-parallel over the 4 owned tiles; full weights, no collective
        wmp_pool = ctx.enter_context(tc.tile_pool(name="wmp", bufs=1))
        w_mp_sb = wmp_pool.tile([P, FKO, c_dim], BF16)
        nc.scalar.dma_start(w_mp_sb[:], w_mpT[:])

        gsp = ctx.enter_context(tc.tile_pool(name="gsp", bufs=2))
        gtp = ctx.enter_context(tc.tile_pool(name="gtp", bufs=2))

        def stage_F(qc, h2T):
            """fc + gelu + transpose for the owned tile of chunk qc."""
            g_s = gsp.tile([P, NHC, QCH], BF16, tag="g_s", name="g_s")
            # two PSUM half-rounds of 4 h-chunks: stationary h2T[ko] is
            # amortized over 4 moving-512 matmuls per load
            for half in range(2):
                pss = [
                    ps_mm.tile([P, QCH], F32, tag="ps", name=f"psf_{i}")
                    for i in range(4)
                ]
                for ko in range(KO):
                    for i in range(4):
                        hc = half * 4 + i
                        nc.tensor.matmul(
                            pss[i][:],
                            h2T[:, ko, :],
                            w_fc_sb[:, ko, hc * QCH:(hc + 1) * QCH],
                            start=(ko == 0),
                            stop=(ko == KO - 1),
                        )
                for i in range(4):
                    hc = half * 4 + i
                    if b_fc_bc is not None:
                        nc.vector.tensor_add(
                            pss[i][:], pss[i][:],
                            b_fc_bc[:, hc * QCH:(hc + 1) * QCH],
                        )
                    nc.scalar.activation(
                        g_s[:, hc, :], pss[i][:],
                        mybir.ActivationFunctionType.Gelu_apprx_tanh,
                    )

            gT = gtp.tile([P, FKO, P], BF16, tag="gT", name="gT")
            for hc in range(NHC):
                transpose_into(gT[:, hc * 4:(hc + 1) * 4, :], g_s[:, hc, :], 0, 4)
            return gT

        def stage_M(qc, gT):
            """Down-projection + final residual + store for chunk qc."""
            x1 = x1_tile(qc)
            for nch in range(NCC):
                ps = ps_mm.tile([P, QCH], F32, tag="ps", name="ps")
                for hk in range(FKO):
                    nc.tensor.matmul(
                        ps[:],
                        gT[:, hk, :],
                        w_mp_sb[:, hk, nch * QCH:(nch + 1) * QCH],
                        start=(hk == 0),
                        stop=(hk == FKO - 1),
                    )
                if b_mp_bc is not None:
                    nc.vector.tensor_add(
                        ps[:], ps[:], b_mp_bc[:, nch * QCH:(nch + 1) * QCH]
                    )
                ev = works.tile([P, QCH], F32, tag="evac", name="ev")
                nc.vector.tensor_tensor(
                    ev[:], ps[:], x1[:, nch * QCH:(nch + 1) * QCH],
                    mybir.AluOpType.add,
                )
                nc.sync.dma_start(
                    out_y[qc * P:(qc + 1) * P, nch * QCH:(nch + 1) * QCH], ev[:]
                )

        h2s = {}
        gts = {}
        h2s[0] = stage_F_pre(0)
        h2s[1] = stage_F_pre(1)
        gts[0] = stage_F(0, h2s.pop(0))
        h2s[2] = stage_F_pre(2)
        gts[1] = stage_F(1, h2s.pop(1))
        stage_M(0, gts.pop(0))
        h2s[3] = stage_F_pre(3)
        gts[2] = stage_F(2, h2s.pop(2))
        stage_M(1, gts.pop(1))
        gts[3] = stage_F(3, h2s.pop(3))
        stage_M(2, gts.pop(2))
        stage_M(3, gts.pop(3))

    if legalize:
        _legalize_waits(nc)
    return nc


# ---------------------------------------------------------------------------
# host-side sharding / layout prep


def _tile_k(arr, width):
    """[K, M] -> [128, K//128, M] (contraction dim inner on partitions)."""
    k, m = arr.shape
    assert m == width and k % P == 0
    return np.ascontiguousarray(
        arr.reshape(k // P, P, m).transpose(1, 0, 2)
    )


def _bf(arr):
    return arr.astype(ml_dtypes.bfloat16)


def make_core_inputs(inputs, t_len=T, c_dim=C, h_core=H_CORE, fh=FH,
                     n_groups=len(GROUPS), tpg=TPG):
    """Shard + lay out the full inputs into per-core input dicts and the
    active-flag set."""
    f32 = np.float32
    x = np.asarray(inputs["x"], f32)
    W_attn = np.asarray(inputs["W_attn"], f32)
    W_aproj = np.asarray(inputs["W_aproj"], f32)
    W_fc = np.asarray(inputs["W_fc"], f32)
    W_mproj = np.asarray(inputs["W_mproj"], f32)
    ln1_g = np.asarray(inputs["ln1_g"], f32)
    ln1_b = np.asarray(inputs["ln1_b"], f32)
    ln2_g = np.asarray(inputs["ln2_g"], f32)
    ln2_b = np.asarray(inputs["ln2_b"], f32)
    b_attn = np.asarray(inputs["b_attn"], f32)
    b_aproj = np.asarray(inputs["b_aproj"], f32)
    b_fc = np.asarray(inputs["b_fc"], f32)
    b_mproj = np.asarray(inputs["b_mproj"], f32)

    Wq, Wk, Wv = W_attn[:c_dim], W_attn[c_dim:2 * c_dim], W_attn[2 * c_dim:]
    bq, bk, bv = b_attn[:c_dim], b_attn[c_dim:2 * c_dim], b_attn[2 * c_dim:]
    scale = 1.0 / math.sqrt(HD)

    flags = set()
    if not np.all(ln1_g == 1.0):
        flags.add("ln1_g")
    if np.any(ln1_b):
        flags.add("ln1_b")
    if not np.all(ln2_g == 1.0):
        flags.add("ln2_g")
    if np.any(ln2_b):
        flags.add("ln2_b")
    if np.any(b_attn[:2 * c_dim]):
        flags.add("b_qk")
    if np.any(bv):
        flags.add("b_v")
    if np.any(b_aproj):
        flags.add("b_ap")
    if np.any(b_fc):
        flags.add("b_fc")
    if np.any(b_mproj):
        flags.add("b_mp")

    tri = np.where(
        np.arange(P)[:, None] > np.arange(P)[None, :], f32(-1e30), f32(0.0)
    ).astype(f32)

    # replicated full MLP weights, transposed layouts (contraction inner)
    w_fcT_full = _tile_k(_bf(np.ascontiguousarray(W_fc.T)), fh)
    w_mpT_full = _tile_k(_bf(np.ascontiguousarray(W_mproj.T)), c_dim)

    NQC = t_len // QCH

    in_maps = []
    for core in range(n_groups * tpg):
        g, s = core // tpg, core % tpg
        heads = range(s * h_core, (s + 1) * h_core)
        # stacked [q heads | k heads] output dims, q pre-scaled by 1/sqrt(hd)
        w_qk_rows = np.concatenate(
            [Wq[h * HD:(h + 1) * HD] * scale for h in heads]
            + [Wk[h * HD:(h + 1) * HD] for h in heads], axis=0
        )  # [QK, C]
        w_v_rows = np.concatenate(
            [Wv[h * HD:(h + 1) * HD] for h in heads], axis=0
        )  # [DH, C]
        dsl = slice(s * h_core * HD, (s + 1) * h_core * HD)
        xg = x[g % x.shape[0]]
        x_own = np.concatenate(
            [xg[(qc * tpg + s) * P:(qc * tpg + s + 1) * P] for qc in range(NQC)],
            axis=0,
        )
        m = {
            "x_tm": np.ascontiguousarray(xg),
            "x_own": np.ascontiguousarray(x_own),
            "w_qk": _tile_k(_bf(w_qk_rows.T), h_core * P),
            "w_v": _tile_k(_bf(w_v_rows.T), h_core * HD),
            "w_ap": _tile_k(_bf(W_aproj[:, dsl].T.copy()), c_dim),
            "w_fcT": w_fcT_full,
            "w_mpT": w_mpT_full,
            "tri": tri,
        }
        if "ln1_g" in flags:
            m["ln1_g"] = ln1_g.reshape(1, -1).copy()
        if "ln1_b" in flags:
            m["ln1_b"] = ln1_b.reshape(1, -1).copy()
        if "ln2_g" in flags:
            m["ln2_g"] = ln2_g.reshape(1, -1).copy()
        if "ln2_b" in flags:
            m["ln2_b"] = ln2_b.reshape(1, -1).copy()
        if "b_qk" in flags:
            b_qk_rows = np.concatenate(
                [bq[h * HD:(h + 1) * HD] * scale for h in heads]
                + [bk[h * HD:(h + 1) * HD] for h in heads]
            )  # [QK] along partitions: [P, MQK]
            m["b_qk"] = np.ascontiguousarray(
                b_qk_rows.reshape(h_core, P).T
            )
        if "b_v" in flags:
            m["b_v"] = np.concatenate(
                [bv[h * HD:(h + 1) * HD] for h in heads]
            ).reshape(1, -1).copy()
        if "b_ap" in flags:
            m["b_ap"] = (b_aproj / tpg).reshape(1, -1).copy()
        if "b_fc" in flags:
            m["b_fc"] = b_fc.reshape(1, -1).copy()
        if "b_mp" in flags:
            m["b_mp"] = b_mproj.reshape(1, -1).copy()
        in_maps.append(m)
    return in_maps, frozenset(flags)


# ---------------------------------------------------------------------------
# runner

_module_cache = {}


def run(inputs, trace=False, trace_kwargs=None, tmpdir=None):
    in_maps, flags = make_core_inputs(inputs)
    key = (flags, trace)
    if key not in _module_cache:
        _module_cache[key] = build_module(flags=flags)
    nc = _module_cache[key]
    if trace:
        _install_prof_hook()
    res = run_bass_kernel_spmd(
        nc,
        in_maps,
        core_ids=list(range(N_CORES)),
        trace=trace,
        tmpdir=tmpdir,
        **(trace_kwargs or {}),
    )
    # reassemble: core g*TPG+s provides token tiles (qc*TPG + s) of batch g
    NQC = T // QCH
    out = np.empty((B, T, C), np.float32)
    for g in range(len(GROUPS)):
        for s in range(TPG):
            o = res.results[g * TPG + s]["out"]
            for qc in range(NQC):
                tt = qc * TPG + s
                out[g, tt * P:(tt + 1) * P, :] = o[qc * P:(qc + 1) * P, :]
    return out, res


def kernel(**inputs) -> np.ndarray:
    out, _ = run(inputs, trace=False)
    return out


# revision 22
# speedup vs baseline: 1.0459x; 1.0073x over previous
"""Fused causal-transformer block (LN1 -> attn -> LN2 -> MLP, residuals) on
8 Trainium2 NeuronCores.

Sharding: 2 groups of 4 cores; group g handles batch element b=g (data
parallel).  Within a group:
  - Attention is Megatron head-parallel: core s owns 4 heads, computes
    partial y = attn(x) @ W_aproj_s for ALL tokens, chunked over four
    512-token chunks.  Each chunk's partials are summed with an in-group
    ReduceScatter, leaving core s with the summed attention output for
    token tile (chunk*4 + s) -- its 128-token slice of each chunk.
  - The MLP is token-parallel with REPLICATED weights: core s runs the
    full 4C-hidden MLP for its 4 owned token tiles (512 tokens total).
    No second collective is needed; the host reassembles token slices.
This cuts the collective count from 8 AllReduces to 4 ReduceScatters,
all hidden behind attention compute (the single CC core was the
bottleneck of the AllReduce design).

Compute dtype: bf16 matmul inputs, fp32 PSUM accumulation, fp32 residual
stream and softmax statistics.

Layouts (per core, all prepared host-side in kernel()):
  h1T/h2T  : [128, C/128, t]  activations transposed (contraction dim on
             partitions) produced on-device via PE transposes.
  qkT      : [128, H_core, T] rows = [q heads | k heads] * 64-dim each,
             two heads stacked per 128-partition tile.  Scores are
             computed directly in S^T [k, q] layout, so softmax
             normalization arrives as a PSUM row via a ones-column in v.
  v_aug    : [128, T/128, H_core, 65]  v token-major per head + ones col.
  w_fcT    : [128, C/128, 4C]  full W_fc^T (replicated), moving operand.
  w_mpT    : [128, 4C/128, C]  full W_mproj^T (replicated), moving.
"""

import contextlib
import ctypes
import math
import sys
import types

import numpy as np
import ml_dtypes

import bass_rust
import concourse.bass as bass
import concourse.mybir as mybir
import concourse.tile as tile
from concourse import library_config
from concourse.bass_utils import run_bass_kernel_spmd
from concourse.masks import make_identity
from concourse.tile import TileContext
from concourse.vector_clock import ScopedClock

# ---------------------------------------------------------------------------
# problem constants (hardcoded per the harness contract)
B, T, C, H = 2, 2048, 1024, 16
HD = C // H                 # 64
N_CORES = 8
TPG = 4                     # tensor-parallel group size
H_CORE = H // TPG           # heads per core = 4
DH = H_CORE * HD            # per-core attention dim = 256
FH = 4 * C                  # full MLP hidden (replicated) = 4096
P = 128
EPS = 1e-5
QCH = 512                   # q-chunk width
GROUPS = [[0, 1, 2, 3], [4, 5, 6, 7]]

F32 = mybir.dt.float32
BF16 = mybir.dt.bfloat16

# ---------------------------------------------------------------------------
# workaround 1: the container's walrus accepts a single sync-wait command per
# instruction; move extra semaphore waits onto inserted EventSemaphore
# instructions on the same engine (program order preserves semantics).

_waitfix_counter = [0]


def _legalize_waits(nc, cap=1):
    fn = nc.m.functions[0]
    n_split = 0
    for bb in fn.blocks:
        out = []
        changed = False
        for inst in bb.instructions:
            si = inst.sync_info
            waits = list(si.on_wait) if si is not None else []
            if len(waits) > cap:
                movable = [w for w in waits if w.sync_type == "semaphore"]
                fixed = [w for w in waits if w.sync_type != "semaphore"]
                n_keep = max(cap - len(fixed), 0)
                keep = fixed + (movable[len(movable) - n_keep:] if n_keep else [])
                extra = movable[: len(movable) - n_keep] if n_keep else movable
                for w in extra:
                    _waitfix_counter[0] += 1
                    ev = mybir.InstEventSemaphore(
                        name=f"I-waitfix-{_waitfix_counter[0]}",
                        engine=inst.engine,
                        ins=[],
                        outs=[],
                        sync_info=bass_rust.SyncInfo(on_wait=[w], on_update=[]),
                    )
                    out.append(ev)
                    n_split += 1
                inst.sync_info = bass_rust.SyncInfo(
                    on_wait=keep, on_update=list(si.on_update)
                )
                changed = True
            out.append(inst)
        if changed:
            bb.instructions = out
    return n_split


# workaround 2: same issue for the Tile kernel-tail Drain — emit one wait-nop
# per live proc ahead of a wait-less drain instead of stacking waits on it.


def _drain_and_barrier_split(self, tick_clock, wait_clock):
    gc = tick_clock.global_clock
    sems_alloc = wait_clock.sems.allocated()
    for proc in sorted(sems_alloc):
        tick = gc.peek_next(proc) - 1
        if tick <= 0:
            continue
        vc1 = bass_rust.VectorClock()
        vc1.require_at_least(proc, tick)
        nop = self.nc.sync.nop()
        wait_clock.add_sem_waits(nop.ins, ScopedClock({None: vc1}))
    self.nc.sync.drain()
    self.nc.all_engine_barrier()
    assert self.sems is not None
    popped = self.nc._tile_sem_poison_stack.pop()
    assert popped is self._sem_poison
    self.nc.clear_and_free_semaphores(list(self.sems.allocated().values()))
    self.nc.all_engine_barrier()


TileContext._drain_and_barrier = _drain_and_barrier_split


# workaround 3 (profiling only): register the NTFF hook the trimmed antenv
# lacks so run_bass_kernel_spmd(trace=True) works under axon.


def _install_prof_hook():
    if "antenv.axon_hooks" in sys.modules:
        return
    so_path = "/opt/axon/libaxon_pjrt.so"
    hook = None
    try:
        lib = ctypes.CDLL(so_path)
        if hasattr(lib, "axon_start_nrt_profile"):
            lib.axon_start_nrt_profile.argtypes = [
                ctypes.POINTER(ctypes.c_int64),
                ctypes.c_size_t,
            ]
            lib.axon_start_nrt_profile.restype = ctypes.c_int64
            lib.axon_stop_nrt_profile.argtypes = [ctypes.c_char_p]
            lib.axon_stop_nrt_profile.restype = ctypes.c_int64

            @contextlib.contextmanager
            def _hook_cm(output_dir, device_ids):
                import jax

                jax.devices()
                if device_ids:
                    ids = (ctypes.c_int64 * len(device_ids))(*device_ids)
                    rc = lib.axon_start_nrt_profile(ids, len(device_ids))
                else:
                    rc = lib.axon_start_nrt_profile(None, 0)
                if rc != 0:
                    raise RuntimeError(f"axon_start_nrt_profile rc={rc}")
                try:
                    yield
                finally:
                    n = lib.axon_stop_nrt_profile(str(output_dir).encode())
                    if n < 0:
                        raise RuntimeError(f"axon_stop_nrt_profile rc={n}")

            hook = _hook_cm
    except OSError:
        pass
    mod = types.ModuleType("antenv.axon_hooks")
    mod.get_axon_ntff_profile_hook = lambda: hook
    mod.set_axon_ntff_profile_hook = lambda h: None
    sys.modules["antenv.axon_hooks"] = mod
    from concourse import bass_utils

    bass_utils.upload_artifacts = lambda tmpdir: tmpdir


# ---------------------------------------------------------------------------
# device kernel builder


def build_module(
    t_len=T,
    c_dim=C,
    h_core=H_CORE,
    fh=FH,
    flags=frozenset(),
    replica_groups=GROUPS,
    local_reduce=False,
    legalize=True,
):
    """Build the per-core SPMD Bass module.

    flags: subset of {"ln1_g","ln1_b","ln2_g","ln2_b","b_qk","b_v","b_ap",
    "b_fc","b_mp"} enabling the non-trivial affine/bias paths.
    local_reduce: replace the in-group ReduceScatter with a local strided
    copy (single core test mode: takes this rank-0 slice).
    """
    KO = c_dim // P             # c-tiles
    NT = t_len // P             # token tiles
    NQC = t_len // QCH          # q chunks
    KPQ = QCH // P              # token tiles per chunk (= group size 4)
    QK = h_core * P             # stacked q+k dims
    MQK = h_core                # m-tiles of qkT
    DKO = (h_core * HD) // P    # d-tiles of y/aproj  (h_core/2)
    FKO = fh // P               # hidden tiles (32)
    NPAIR = h_core // 2
    NCC = c_dim // QCH          # 512-chunks of C
    NHC = fh // QCH             # 512-chunks of hidden (8)
    assert h_core % 2 == 0 and c_dim % P == 0 and t_len % QCH == 0

    nc = bass.Bass(num_devices=N_CORES)

    x_tm = nc.dram_tensor("x_tm", (t_len, c_dim), F32, kind="ExternalInput")
    x_own = nc.dram_tensor("x_own", (NQC * P, c_dim), F32, kind="ExternalInput")
    w_qk = nc.dram_tensor("w_qk", (P, KO, QK), BF16, kind="ExternalInput")
    w_v = nc.dram_tensor("w_v", (P, KO, h_core * HD), BF16, kind="ExternalInput")
    w_ap = nc.dram_tensor("w_ap", (P, DKO, c_dim), BF16, kind="ExternalInput")
    w_fcT = nc.dram_tensor("w_fcT", (P, KO, fh), BF16, kind="ExternalInput")
    w_mpT = nc.dram_tensor("w_mpT", (P, FKO, c_dim), BF16, kind="ExternalInput")
    tri = nc.dram_tensor("tri", (P, P), F32, kind="ExternalInput")
    opt_in = {}
    for name, shape in [
        ("ln1_g", (1, c_dim)), ("ln1_b", (1, c_dim)),
        ("ln2_g", (1, c_dim)), ("ln2_b", (1, c_dim)),
        ("b_qk", (P, MQK)), ("b_v", (1, h_core * HD)), ("b_ap", (1, c_dim)),
        ("b_fc", (1, fh)), ("b_mp", (1, c_dim)),
    ]:
        if name in flags:
            opt_in[name] = nc.dram_tensor(name, shape, F32, kind="ExternalInput")

    # per-core output: its 4 owned token tiles, row qc*128+p = token
    # tile (qc*4 + rank), host reassembles.
    out_y = nc.dram_tensor("out", (NQC * P, c_dim), F32, kind="ExternalOutput")

    # collective payloads travel in bf16: halves the wire time; the partial
    # projections are O(1)-magnitude so the rounding is ~1e-3 relative.
    ARDT = BF16
    rs_in = [nc.dram_tensor(f"rs_in{i}", (QCH, c_dim), ARDT) for i in range(NQC)]
    rs_out = [nc.dram_tensor(f"rs_out{i}", (P, c_dim), ARDT) for i in range(NQC)]
    # DRAM bounce rows for the softmax-denominator partition broadcast
    recip_d = nc.dram_tensor("recip_d", (NQC * h_core, QCH), F32)

    with TileContext(nc) as tc, contextlib.ExitStack() as ctx:
        const = ctx.enter_context(tc.tile_pool(name="const", bufs=1))
        workb = ctx.enter_context(tc.tile_pool(name="workb", bufs=2))
        works = ctx.enter_context(tc.tile_pool(name="works", bufs=3))
        stats = ctx.enter_context(tc.tile_pool(name="stats", bufs=6))

        ident = const.tile([P, P], BF16)
        make_identity(nc, ident)
        eps_t = const.tile([P, 1], F32)
        nc.vector.memset(eps_t[:], EPS)
        tri_sb = const.tile([P, P], F32)
        nc.sync.dma_start(tri_sb[:], tri[:])

        # optional affine operands, broadcast to 128 partitions once
        def _bcast_row(name, width):
            if name not in opt_in:
                return None
            bc = const.tile([P, width], F32, name=f"bc_{name}", tag=f"bc_{name}")
            nc.sync.dma_start(bc[:], opt_in[name][:].to_broadcast((P, width)))
            return bc

        def _col(name):
            if name not in opt_in:
                return None
            t_ = const.tile(list(opt_in[name].shape), F32, name=f"col_{name}", tag=f"col_{name}")
            nc.sync.dma_start(t_[:], opt_in[name][:])
            return t_

        ln1_g_bc = _bcast_row("ln1_g", c_dim)
        ln1_b_bc = _bcast_row("ln1_b", c_dim)
        ln2_g_bc = _bcast_row("ln2_g", c_dim)
        ln2_b_bc = _bcast_row("ln2_b", c_dim)
        b_v_bc = _bcast_row("b_v", h_core * HD)
        b_ap_bc = _bcast_row("b_ap", c_dim)
        b_fc_bc = _bcast_row("b_fc", fh)
        b_mp_bc = _bcast_row("b_mp", c_dim)
        b_qk_col = _col("b_qk")

        ps_tr = ctx.enter_context(tc.tile_pool(name="ps_tr", bufs=2, space="PSUM"))
        ps_mm = ctx.enter_context(tc.tile_pool(name="ps_mm", bufs=4, space="PSUM"))
        ps_y = ctx.enter_context(tc.tile_pool(name="ps_y", bufs=2, space="PSUM"))

        def ln_tile(x_f32, g_bc, b_bc):
            """LayerNorm of a [P, c_dim] fp32 AP -> new [P, c_dim] bf16 tile.
            rsqrt via Ln+Exp: shares the Scalar activation table with the
            softmax Exp, so no table reloads between LN and attention."""
            nsub = c_dim // 512
            st = stats.tile([P, nsub, 6], F32)
            for j in range(nsub):
                nc.vector.bn_stats(st[:, j, :], x_f32[:, j * 512:(j + 1) * 512])
            mv = stats.tile([P, 2], F32)
            nc.vector.bn_aggr(mv[:], st[:])
            r = stats.tile([P, 1], F32)
            nc.scalar.activation(
                r[:], mv[:, 1:2], mybir.ActivationFunctionType.Ln, bias=eps_t[:]
            )
            nc.scalar.activation(
                r[:], r[:], mybir.ActivationFunctionType.Exp, scale=-0.5
            )
            if g_bc is None and b_bc is None:
                h_bf = works.tile([P, c_dim], BF16, tag="ln_out", name="h_bf")
                nc.vector.tensor_scalar(
                    out=h_bf[:], in0=x_f32[:], scalar1=mv[:, 0:1], scalar2=r[:],
                    op0=mybir.AluOpType.subtract, op1=mybir.AluOpType.mult,
                )
            else:
                h_f = workb.tile([P, c_dim], F32, tag="ln_f32", name="h_f")
                nc.vector.tensor_scalar(
                    out=h_f[:], in0=x_f32[:], scalar1=mv[:, 0:1], scalar2=r[:],
                    op0=mybir.AluOpType.subtract, op1=mybir.AluOpType.mult,
                )
                if g_bc is not None:
                    nc.vector.tensor_mul(h_f[:], h_f[:], g_bc[:])
                if b_bc is not None:
                    nc.vector.tensor_add(h_f[:], h_f[:], b_bc[:])
                h_bf = works.tile([P, c_dim], BF16, tag="ln_out", name="h_bf")
                nc.vector.tensor_copy(h_bf[:], h_f[:])
            return h_bf

        def transpose_into(dstT, src_bf, tl, n_k):
            """PE-transpose [P, n_k*128] bf16 into dstT[:, :, tl*P:(tl+1)*P].
            Four 128x128 transposes share one PSUM bank so a single DVE copy
            evacuates them."""
            for kg in range(0, n_k, 4):
                nb = min(4, n_k - kg)
                pst = ps_tr.tile([P, 4 * P], BF16, tag="pst", name="pst")
                for j in range(nb):
                    nc.tensor.transpose(
                        pst[:, j * P:(j + 1) * P],
                        src_bf[:, (kg + j) * P:(kg + j + 1) * P],
                        ident[:],
                    )
                nc.vector.tensor_copy(
                    dstT[:, kg:kg + nb, tl * P:(tl + 1) * P],
                    pst[:, 0:nb * P].rearrange("p (a b) -> p a b", a=nb),
                )

        # =============== the MLP weight pools (persistent) =================
        # w_fcT preloads during attention (SBUF has room); w_mpT loads into
        # the space the attention pools free, overlapping the fc stages.
        wfc_pool = ctx.enter_context(tc.tile_pool(name="wfc", bufs=1))
        h2p = ctx.enter_context(tc.tile_pool(name="h2p", bufs=4))

        # ======================= attention phase ===========================
        with contextlib.ExitStack() as attn_ctx:
            # weights go on the Activation engine's DMA queue so the big
            # transfers never head-of-line-block the latency-critical
            # activation loads on the sync queue.
            wa = attn_ctx.enter_context(tc.tile_pool(name="wa", bufs=1))
            w_qk_sb = wa.tile([P, KO, QK], BF16)
            nc.scalar.dma_start(w_qk_sb[:], w_qk[:])
            w_v_sb = wa.tile([P, KO, h_core * HD], BF16)
            nc.scalar.dma_start(w_v_sb[:], w_v[:])
            w_ap_sb = wa.tile([P, DKO, c_dim], BF16)
            nc.scalar.dma_start(w_ap_sb[:], w_ap[:])

            big = attn_ctx.enter_context(tc.tile_pool(name="big", bufs=1))
            qkT = big.tile([P, MQK, t_len], BF16)
            vaug = big.tile([P, NT, h_core, 65], BF16)
            nc.vector.memset(vaug[:, :, :, 64:65], 1.0)

            h1p = attn_ctx.enter_context(tc.tile_pool(name="h1p", bufs=2))
            yp = attn_ctx.enter_context(tc.tile_pool(name="yp", bufs=2))
            pt_pool = attn_ctx.enter_context(tc.tile_pool(name="pt", bufs=6))
            rowp = attn_ctx.enter_context(tc.tile_pool(name="rows", bufs=2))
            arp = attn_ctx.enter_context(tc.tile_pool(name="arp", bufs=2))

            w_fc_sb = wfc_pool.tile([P, KO, fh], BF16)

            def stage_A_pre(qc):
                """LN1 + transposed activations for chunk qc (DVE/Scalar
                heavy; emitted a chunk ahead so the PE never waits on it)."""
                h1T = h1p.tile([P, KO, QCH], BF16, tag="h1T", name="h1T")
                for tl in range(KPQ):
                    tt = qc * KPQ + tl
                    xt = workb.tile([P, c_dim], F32, tag="x_in", name="xt")
                    nc.sync.dma_start(xt[:], x_tm[tt * P:(tt + 1) * P, :])
                    h_bf = ln_tile(xt, ln1_g_bc, ln1_b_bc)
                    transpose_into(h1T, h_bf, tl, KO)
                return h1T

            def stage_A(qc, h1T):
                # qkT chunk (transposed-output form)
                for mt in range(MQK):
                    ps = ps_mm.tile([P, QCH], F32, tag="ps", name="ps")
                    for ko in range(KO):
                        nc.tensor.matmul(
                            ps[:],
                            w_qk_sb[:, ko, mt * P:(mt + 1) * P],
                            h1T[:, ko, :],
                            start=(ko == 0),
                            stop=(ko == KO - 1),
                        )
                    dst = qkT[:, mt, qc * QCH:(qc + 1) * QCH]
                    if b_qk_col is not None:
                        nc.vector.tensor_scalar_add(dst, ps[:], b_qk_col[:, mt:mt + 1])
                    else:
                        nc.vector.tensor_copy(dst, ps[:])

                # v token-major for this chunk's tiles
                for tl in range(KPQ):
                    tt = qc * KPQ + tl
                    ps = ps_mm.tile([P, QCH], F32, tag="ps", name="ps")
                    for ko in range(KO):
                        nc.tensor.matmul(
                            ps[:, 0:h_core * HD],
                            h1T[:, ko, tl * P:(tl + 1) * P],
                            w_v_sb[:, ko, :],
                            start=(ko == 0),
                            stop=(ko == KO - 1),
                        )
                    if b_v_bc is not None:
                        nc.vector.tensor_add(
                            ps[:, 0:h_core * HD], ps[:, 0:h_core * HD], b_v_bc[:]
                        )
                    for h in range(h_core):
                        nc.vector.tensor_copy(
                            vaug[:, tt, h, 0:64], ps[:, h * HD:(h + 1) * HD]
                        )

                # causal attention, one head-pair at a time
                yT = yp.tile([P, DKO, QCH], BF16, tag="yT", name="yT")
                for pr in range(NPAIR):
                    heads = ((0, 2 * pr), (64, 2 * pr + 1))
                    psy = {}
                    for sub, h in heads:
                        psy[h] = ps_y.tile([P, QCH], F32, tag="psy", name=f"psy_{h}")
                    nkt = (qc + 1) * KPQ
                    pts = {}

                    def emit_s_exp(kt, heads=heads, pts=pts, qc=qc, pr=pr):
                        i = kt - qc * KPQ  # >=0 on the diagonal band
                        for sub, h in heads:
                            pss = ps_mm.tile([P, QCH], F32, tag="ps", name=f"pss_{h}")
                            nc.tensor.matmul(
                                pss[:],
                                qkT[sub:sub + 64, DKO + pr, kt * P:(kt + 1) * P],
                                qkT[sub:sub + 64, pr, qc * QCH:(qc + 1) * QCH],
                                start=True,
                                stop=True,
                            )
                            pt = pt_pool.tile([P, QCH], BF16, tag="pt", name=f"pt_{h}")
                            if i >= 0:
                                if i > 0:
                                    nc.vector.memset(pt[:, 0:i * P], 0.0)
                                nc.vector.tensor_add(
                                    pss[:, i * P:(i + 1) * P],
                                    pss[:, i * P:(i + 1) * P],
                                    tri_sb[:],
                                )
                                nc.scalar.activation(
                                    pt[:, i * P:QCH],
                                    pss[:, i * P:QCH],
                                    mybir.ActivationFunctionType.Exp,
                                )
                            else:
                                nc.scalar.activation(
                                    pt[:], pss[:], mybir.ActivationFunctionType.Exp
                                )
                            pts[(kt, h)] = pt

                    def emit_pv(kt, heads=heads, psy=psy, pts=pts, nkt=nkt):
                        for sub, h in heads:
                            nc.tensor.matmul(
                                psy[h][0:65, :],
                                vaug[:, kt, h, :],
                                pts.pop((kt, h))[:],
                                start=(kt == 0),
                                stop=(kt == nkt - 1),
                            )

                    for kt in range(nkt):
                        emit_s_exp(kt)
                        if kt > 0:
                            emit_pv(kt - 1)
                    emit_pv(nkt - 1)
                    for sub, h in heads:
                        # evacuate the whole psy bank once via DVE (frees the
                        # PSUM bank for the next pair immediately, and keeps
                        # the Scalar engine free for the softmax exps)
                        psy_sb = rowp.tile([65, QCH], F32, tag="psy_sb", name="psy_sb")
                        nc.vector.tensor_copy(psy_sb[:], psy[h][0:65, :])
                        bc_sb = rowp.tile([64, QCH], F32, tag="bc_sb", name="bc_sb")
                        rd = recip_d[qc * h_core + h:qc * h_core + h + 1, :]
                        nc.sync.dma_start(rd, psy_sb[64:65, :])
                        nc.sync.dma_start(bc_sb[:], rd.to_broadcast((64, QCH)))
                        # reciprocal on the 64-partition broadcast: ~25x
                        # faster on DVE than on the 1-partition row
                        nc.vector.reciprocal(bc_sb[:], bc_sb[:])
                        nc.vector.tensor_tensor(
                            yT[sub:sub + 64, pr, :],
                            psy_sb[0:64, :],
                            bc_sb[:],
                            mybir.AluOpType.mult,
                        )

                # out-projection partials -> one staging tile -> one DMA
                ar_st = arp.tile([P, KPQ, c_dim], ARDT, tag="ar_st", name="ar_st")
                for tl in range(KPQ):
                    for nch in range(NCC):
                        ps = ps_mm.tile([P, QCH], F32, tag="ps", name="ps")
                        for dk in range(DKO):
                            nc.tensor.matmul(
                                ps[:],
                                yT[:, dk, tl * P:(tl + 1) * P],
                                w_ap_sb[:, dk, nch * QCH:(nch + 1) * QCH],
                                start=(dk == 0),
                                stop=(dk == DKO - 1),
                            )
                        dst = ar_st[:, tl, nch * QCH:(nch + 1) * QCH]
                        if b_ap_bc is not None:
                            nc.vector.tensor_add(
                                dst, ps[:], b_ap_bc[:, nch * QCH:(nch + 1) * QCH]
                            )
                        else:
                            nc.vector.tensor_copy(dst, ps[:])
                nc.sync.dma_start(
                    rs_in[qc][:].rearrange("(a p) c -> p a c", p=P), ar_st[:]
                )
                if local_reduce:
                    nc.sync.dma_start(rs_out[qc][:], rs_in[qc][0:P, :])
                else:
                    nc.gpsimd.collective_compute(
                        "ReduceScatter",
                        mybir.AluOpType.add,
                        replica_groups=replica_groups,
                        ins=[rs_in[qc][:]],
                        outs=[rs_out[qc][:]],
                    )

            def read_rs(qc):
                """Pre-read the chunk's ReduceScatter output.  Placement
                matters: this is a DEPENDENT dma on the in-order sync queue,
                so it must be emitted after every latency-critical load that
                could sit behind it."""
                at = workb.tile([P, c_dim], ARDT, tag="ar_rd", name="at")
                nc.sync.dma_start(at[:], rs_out[qc][:])
                return at

            def x1_tile(qc, at=None):
                """x1 = x_own + attn for owned tile of chunk qc, fp32."""
                xt = workb.tile([P, c_dim], F32, tag="x1f", name="xt")
                nc.sync.dma_start(xt[:], x_own[qc * P:(qc + 1) * P, :])
                if at is None:
                    at = read_rs(qc)
                nc.vector.tensor_add(xt[:], xt[:], at[:])
                return xt

            def stage_F_pre(qc, at=None):
                """x1 + LN2 + transposed h2 for the owned tile of chunk qc."""
                x1 = x1_tile(qc, at)
                h_bf = ln_tile(x1, ln2_g_bc, ln2_b_bc)
                h2T = h2p.tile([P, KO, P], BF16, tag="h2T", name="h2T")
                transpose_into(h2T, h_bf, 0, KO)
                return h2T

            h1s = {}
            h2s = {}
            ats = {}
            h1s[0] = stage_A_pre(0)
            h1s[1] = stage_A_pre(1)
            stage_A(0, h1s.pop(0))
            h1s[2] = stage_A_pre(2)
            stage_A(1, h1s.pop(1))
            # preload the full fc weight mid-attention: late enough not to
            # contend for HBM with the startup x/weight loads, early enough
            # to land long before the fc stages need it.
            nc.scalar.dma_start(w_fc_sb[:], w_fcT[:])
            h1s[3] = stage_A_pre(3)
            # all attention x loads are now emitted; dependent rs reads are
            # safe to queue, and the LN2 stages overlap attention compute
            ats[0] = read_rs(0)
            stage_A(2, h1s.pop(2))
            h2s[0] = stage_F_pre(0, ats.pop(0))
            ats[1] = read_rs(1)
            stage_A(3, h1s.pop(3))
            h2s[1] = stage_F_pre(1, ats.pop(1))

        # ======================== MLP phase ================================
        # token-parallel over the 4 owned tiles; full weights, no collective
        wmp_pool = ctx.enter_context(tc.tile_pool(name="wmp", bufs=1))
        w_mp_sb = wmp_pool.tile([P, FKO, c_dim], BF16)
        nc.scalar.dma_start(w_mp_sb[:], w_mpT[:])

        gsp = ctx.enter_context(tc.tile_pool(name="gsp", bufs=2))
        gtp = ctx.enter_context(tc.tile_pool(name="gtp", bufs=2))

        def stage_F(qc, h2T):
            """fc + gelu + transpose for the owned tile of chunk qc."""
            g_s = gsp.tile([P, NHC, QCH], BF16, tag="g_s", name="g_s")
            # two PSUM half-rounds of 4 h-chunks: stationary h2T[ko] is
            # amortized over 4 moving-512 matmuls per load
            for half in range(2):
                pss = [
                    ps_mm.tile([P, QCH], F32, tag="ps", name=f"psf_{i}")
                    for i in range(4)
                ]
                for ko in range(KO):
                    for i in range(4):
                        hc = half * 4 + i
                        nc.tensor.matmul(
                            pss[i][:],
                            h2T[:, ko, :],
                            w_fc_sb[:, ko, hc * QCH:(hc + 1) * QCH],
                            start=(ko == 0),
                            stop=(ko == KO - 1),
                        )
                for i in range(4):
                    hc = half * 4 + i
                    if b_fc_bc is not None:
                        nc.vector.tensor_add(
                            pss[i][:], pss[i][:],
                            b_fc_bc[:, hc * QCH:(hc + 1) * QCH],
                        )
                    nc.scalar.activation(
                        g_s[:, hc, :], pss[i][:],
                        mybir.ActivationFunctionType.Gelu_apprx_tanh,
                    )

            gT = gtp.tile([P, FKO, P], BF16, tag="gT", name="gT")
            for hc in range(NHC):
                transpose_into(gT[:, hc * 4:(hc + 1) * 4, :], g_s[:, hc, :], 0, 4)
            return gT

        def stage_M(qc, gT):
            """Down-projection + final residual + store for chunk qc."""
            x1 = x1_tile(qc)
            for nch in range(NCC):
                ps = ps_mm.tile([P, QCH], F32, tag="ps", name="ps")
                for hk in range(FKO):
                    nc.tensor.matmul(
                        ps[:],
                        gT[:, hk, :],
                        w_mp_sb[:, hk, nch * QCH:(nch + 1) * QCH],
                        start=(hk == 0),
                        stop=(hk == FKO - 1),
                    )
                if b_mp_bc is not None:
                    nc.vector.tensor_add(
                        ps[:], ps[:], b_mp_bc[:, nch * QCH:(nch + 1) * QCH]
                    )
                ev = works.tile([P, QCH], F32, tag="evac", name="ev")
                nc.vector.tensor_tensor(
                    ev[:], ps[:], x1[:, nch * QCH:(nch + 1) * QCH],
                    mybir.AluOpType.add,
                )
                nc.sync.dma_start(
                    out_y[qc * P:(qc + 1) * P, nch * QCH:(nch + 1) * QCH], ev[:]
                )

        gts = {}
        h2s[2] = stage_F_pre(2)
        gts[0] = stage_F(0, h2s.pop(0))
        h2s[3] = stage_F_pre(3)
        gts[1] = stage_F(1, h2s.pop(1))
        stage_M(0, gts.pop(0))
        gts[2] = stage_F(2, h2s.pop(2))
        stage_M(1, gts.pop(1))
        gts[3] = stage_F(3, h2s.pop(3))
        stage_M(2, gts.pop(2))
        stage_M(3, gts.pop(3))

    if legalize:
        _legalize_waits(nc)
    return nc


# ---------------------------------------------------------------------------
# host-side sharding / layout prep


def _tile_k(arr, width):
    """[K, M] -> [128, K//128, M] (contraction dim inner on partitions)."""
    k, m = arr.shape
    assert m == width and k % P == 0
    return np.ascontiguousarray(
        arr.reshape(k // P, P, m).transpose(1, 0, 2)
    )


def _bf(arr):
    return arr.astype(ml_dtypes.bfloat16)


def make_core_inputs(inputs, t_len=T, c_dim=C, h_core=H_CORE, fh=FH,
                     n_groups=len(GROUPS), tpg=TPG):
    """Shard + lay out the full inputs into per-core input dicts and the
    active-flag set."""
    f32 = np.float32
    x = np.asarray(inputs["x"], f32)
    W_attn = np.asarray(inputs["W_attn"], f32)
    W_aproj = np.asarray(inputs["W_aproj"], f32)
    W_fc = np.asarray(inputs["W_fc"], f32)
    W_mproj = np.asarray(inputs["W_mproj"], f32)
    ln1_g = np.asarray(inputs["ln1_g"], f32)
    ln1_b = np.asarray(inputs["ln1_b"], f32)
    ln2_g = np.asarray(inputs["ln2_g"], f32)
    ln2_b = np.asarray(inputs["ln2_b"], f32)
    b_attn = np.asarray(inputs["b_attn"], f32)
    b_aproj = np.asarray(inputs["b_aproj"], f32)
    b_fc = np.asarray(inputs["b_fc"], f32)
    b_mproj = np.asarray(inputs["b_mproj"], f32)

    Wq, Wk, Wv = W_attn[:c_dim], W_attn[c_dim:2 * c_dim], W_attn[2 * c_dim:]
    bq, bk, bv = b_attn[:c_dim], b_attn[c_dim:2 * c_dim], b_attn[2 * c_dim:]
    scale = 1.0 / math.sqrt(HD)

    flags = set()
    if not np.all(ln1_g == 1.0):
        flags.add("ln1_g")
    if np.any(ln1_b):
        flags.add("ln1_b")
    if not np.all(ln2_g == 1.0):
        flags.add("ln2_g")
    if np.any(ln2_b):
        flags.add("ln2_b")
    if np.any(b_attn[:2 * c_dim]):
        flags.add("b_qk")
    if np.any(bv):
        flags.add("b_v")
    if np.any(b_aproj):
        flags.add("b_ap")
    if np.any(b_fc):
        flags.add("b_fc")
    if np.any(b_mproj):
        flags.add("b_mp")

    tri = np.where(
        np.arange(P)[:, None] > np.arange(P)[None, :], f32(-1e30), f32(0.0)
    ).astype(f32)

    # replicated full MLP weights, transposed layouts (contraction inner)
    w_fcT_full = _tile_k(_bf(np.ascontiguousarray(W_fc.T)), fh)
    w_mpT_full = _tile_k(_bf(np.ascontiguousarray(W_mproj.T)), c_dim)

    NQC = t_len // QCH

    in_maps = []
    for core in range(n_groups * tpg):
        g, s = core // tpg, core % tpg
        heads = range(s * h_core, (s + 1) * h_core)
        # stacked [q heads | k heads] output dims, q pre-scaled by 1/sqrt(hd)
        w_qk_rows = np.concatenate(
            [Wq[h * HD:(h + 1) * HD] * scale for h in heads]
            + [Wk[h * HD:(h + 1) * HD] for h in heads], axis=0
        )  # [QK, C]
        w_v_rows = np.concatenate(
            [Wv[h * HD:(h + 1) * HD] for h in heads], axis=0
        )  # [DH, C]
        dsl = slice(s * h_core * HD, (s + 1) * h_core * HD)
        xg = x[g % x.shape[0]]
        x_own = np.concatenate(
            [xg[(qc * tpg + s) * P:(qc * tpg + s + 1) * P] for qc in range(NQC)],
            axis=0,
        )
        m = {
            "x_tm": np.ascontiguousarray(xg),
            "x_own": np.ascontiguousarray(x_own),
            "w_qk": _tile_k(_bf(w_qk_rows.T), h_core * P),
            "w_v": _tile_k(_bf(w_v_rows.T), h_core * HD),
            "w_ap": _tile_k(_bf(W_aproj[:, dsl].T.copy()), c_dim),
            "w_fcT": w_fcT_full,
            "w_mpT": w_mpT_full,
            "tri": tri,
        }
        if "ln1_g" in flags:
            m["ln1_g"] = ln1_g.reshape(1, -1).copy()
        if "ln1_b" in flags:
            m["ln1_b"] = ln1_b.reshape(1, -1).copy()
        if "ln2_g" in flags:
            m["ln2_g"] = ln2_g.reshape(1, -1).copy()
        if "ln2_b" in flags:
            m["ln2_b"] = ln2_b.reshape(1, -1).copy()
        if "b_qk" in flags:
            b_qk_rows = np.concatenate(
                [bq[h * HD:(h + 1) * HD] * scale for h in heads]
                + [bk[h * HD:(h + 1) * HD] for h in heads]
            )  # [QK] along partitions: [P, MQK]
            m["b_qk"] = np.ascontiguousarray(
                b_qk_rows.reshape(h_core, P).T
            )
        if "b_v" in flags:
            m["b_v"] = np.concatenate(
                [bv[h * HD:(h + 1) * HD] for h in heads]
            ).reshape(1, -1).copy()
        if "b_ap" in flags:
            m["b_ap"] = (b_aproj / tpg).reshape(1, -1).copy()
        if "b_fc" in flags:
            m["b_fc"] = b_fc.reshape(1, -1).copy()
        if "b_mp" in flags:
            m["b_mp"] = b_mproj.reshape(1, -1).copy()
        in_maps.append(m)
    return in_maps, frozenset(flags)


# ---------------------------------------------------------------------------
# runner

_module_cache = {}


def run(inputs, trace=False, trace_kwargs=None, tmpdir=None):
    in_maps, flags = make_core_inputs(inputs)
    key = (flags, trace)
    if key not in _module_cache:
        _module_cache[key] = build_module(flags=flags)
    nc = _module_cache[key]
    if trace:
        _install_prof_hook()
    res = run_bass_kernel_spmd(
        nc,
        in_maps,
        core_ids=list(range(N_CORES)),
        trace=trace,
        tmpdir=tmpdir,
        **(trace_kwargs or {}),
    )
    # reassemble: core g*TPG+s provides token tiles (qc*TPG + s) of batch g
    NQC = T // QCH
    out = np.empty((B, T, C), np.float32)
    for g in range(len(GROUPS)):
        for s in range(TPG):
            o = res.results[g * TPG + s]["out"]
            for qc in range(NQC):
                tt = qc * TPG + s
                out[g, tt * P:(tt + 1) * P, :] = o[qc * P:(qc + 1) * P, :]
    return out, res


def kernel(**inputs) -> np.ndarray:
    out, _ = run(inputs, trace=False)
    return out


# revision 33
# speedup vs baseline: 1.0654x; 1.0187x over previous
"""Fused causal-transformer block (LN1 -> attn -> LN2 -> MLP, residuals) on
8 Trainium2 NeuronCores.

Sharding: 2 groups of 4 cores; group g handles batch element b=g (data
parallel).  Within a group:
  - Attention is Megatron head-parallel: core s owns 4 heads, computes
    partial y = attn(x) @ W_aproj_s for ALL tokens, chunked over four
    512-token chunks.  Each chunk's partials are summed with an in-group
    ReduceScatter, leaving core s with the summed attention output for
    token tile (chunk*4 + s) -- its 128-token slice of each chunk.
  - The MLP is token-parallel with REPLICATED weights: core s runs the
    full 4C-hidden MLP for its 4 owned token tiles (512 tokens total).
    No second collective is needed; the host reassembles token slices.
This cuts the collective count from 8 AllReduces to 4 ReduceScatters,
all hidden behind attention compute (the single CC core was the
bottleneck of the AllReduce design).

Compute dtype: bf16 matmul inputs, fp32 PSUM accumulation, fp32 residual
stream and softmax statistics.

Layouts (per core, all prepared host-side in kernel()):
  h1T/h2T  : [128, C/128, t]  activations transposed (contraction dim on
             partitions) produced on-device via PE transposes.
  qkT      : [128, H_core, T] rows = [q heads | k heads] * 64-dim each,
             two heads stacked per 128-partition tile.  Scores are
             computed directly in S^T [k, q] layout, so softmax
             normalization arrives as a PSUM row via a ones-column in v.
  v_aug    : [128, T/128, H_core, 65]  v token-major per head + ones col.
  w_fcT    : [128, C/128, 4C]  full W_fc^T (replicated), moving operand.
  w_mpT    : [128, 4C/128, C]  full W_mproj^T (replicated), moving.
"""

import contextlib
import ctypes
import math
import sys
import types

import numpy as np
import ml_dtypes

import bass_rust
import concourse.bass as bass
import concourse.mybir as mybir
import concourse.tile as tile
from concourse import library_config
from concourse.bass_utils import run_bass_kernel_spmd
from concourse.masks import make_identity
from concourse.tile import TileContext
from concourse.vector_clock import ScopedClock

# ---------------------------------------------------------------------------
# problem constants (hardcoded per the harness contract)
B, T, C, H = 2, 2048, 1024, 16
HD = C // H                 # 64
N_CORES = 8
TPG = 4                     # tensor-parallel group size
H_CORE = H // TPG           # heads per core = 4
DH = H_CORE * HD            # per-core attention dim = 256
FH = 4 * C                  # full MLP hidden (replicated) = 4096
P = 128
EPS = 1e-5
QCH = 512                   # q-chunk width
GROUPS = [[0, 1, 2, 3], [4, 5, 6, 7]]

F32 = mybir.dt.float32
BF16 = mybir.dt.bfloat16

# ---------------------------------------------------------------------------
# workaround 1: the container's walrus accepts a single sync-wait command per
# instruction; move extra semaphore waits onto inserted EventSemaphore
# instructions on the same engine (program order preserves semantics).

_waitfix_counter = [0]


def _legalize_waits(nc, cap=1):
    fn = nc.m.functions[0]
    n_split = 0
    for bb in fn.blocks:
        out = []
        changed = False
        for inst in bb.instructions:
            si = inst.sync_info
            waits = list(si.on_wait) if si is not None else []
            if len(waits) > cap:
                movable = [w for w in waits if w.sync_type == "semaphore"]
                fixed = [w for w in waits if w.sync_type != "semaphore"]
                n_keep = max(cap - len(fixed), 0)
                keep = fixed + (movable[len(movable) - n_keep:] if n_keep else [])
                extra = movable[: len(movable) - n_keep] if n_keep else movable
                for w in extra:
                    _waitfix_counter[0] += 1
                    ev = mybir.InstEventSemaphore(
                        name=f"I-waitfix-{_waitfix_counter[0]}",
                        engine=inst.engine,
                        ins=[],
                        outs=[],
                        sync_info=bass_rust.SyncInfo(on_wait=[w], on_update=[]),
                    )
                    out.append(ev)
                    n_split += 1
                inst.sync_info = bass_rust.SyncInfo(
                    on_wait=keep, on_update=list(si.on_update)
                )
                changed = True
            out.append(inst)
        if changed:
            bb.instructions = out
    return n_split


# workaround 2: same issue for the Tile kernel-tail Drain — emit one wait-nop
# per live proc ahead of a wait-less drain instead of stacking waits on it.


def _drain_and_barrier_split(self, tick_clock, wait_clock):
    gc = tick_clock.global_clock
    sems_alloc = wait_clock.sems.allocated()
    for proc in sorted(sems_alloc):
        tick = gc.peek_next(proc) - 1
        if tick <= 0:
            continue
        vc1 = bass_rust.VectorClock()
        vc1.require_at_least(proc, tick)
        nop = self.nc.sync.nop()
        wait_clock.add_sem_waits(nop.ins, ScopedClock({None: vc1}))
    self.nc.sync.drain()
    self.nc.all_engine_barrier()
    assert self.sems is not None
    popped = self.nc._tile_sem_poison_stack.pop()
    assert popped is self._sem_poison
    self.nc.clear_and_free_semaphores(list(self.sems.allocated().values()))
    self.nc.all_engine_barrier()


TileContext._drain_and_barrier = _drain_and_barrier_split


# workaround 3 (profiling only): register the NTFF hook the trimmed antenv
# lacks so run_bass_kernel_spmd(trace=True) works under axon.


def _install_prof_hook():
    if "antenv.axon_hooks" in sys.modules:
        return
    so_path = "/opt/axon/libaxon_pjrt.so"
    hook = None
    try:
        lib = ctypes.CDLL(so_path)
        if hasattr(lib, "axon_start_nrt_profile"):
            lib.axon_start_nrt_profile.argtypes = [
                ctypes.POINTER(ctypes.c_int64),
                ctypes.c_size_t,
            ]
            lib.axon_start_nrt_profile.restype = ctypes.c_int64
            lib.axon_stop_nrt_profile.argtypes = [ctypes.c_char_p]
            lib.axon_stop_nrt_profile.restype = ctypes.c_int64

            @contextlib.contextmanager
            def _hook_cm(output_dir, device_ids):
                import jax

                jax.devices()
                if device_ids:
                    ids = (ctypes.c_int64 * len(device_ids))(*device_ids)
                    rc = lib.axon_start_nrt_profile(ids, len(device_ids))
                else:
                    rc = lib.axon_start_nrt_profile(None, 0)
                if rc != 0:
                    raise RuntimeError(f"axon_start_nrt_profile rc={rc}")
                try:
                    yield
                finally:
                    n = lib.axon_stop_nrt_profile(str(output_dir).encode())
                    if n < 0:
                        raise RuntimeError(f"axon_stop_nrt_profile rc={n}")

            hook = _hook_cm
    except OSError:
        pass
    mod = types.ModuleType("antenv.axon_hooks")
    mod.get_axon_ntff_profile_hook = lambda: hook
    mod.set_axon_ntff_profile_hook = lambda h: None
    sys.modules["antenv.axon_hooks"] = mod
    from concourse import bass_utils

    bass_utils.upload_artifacts = lambda tmpdir: tmpdir


# ---------------------------------------------------------------------------
# device kernel builder


def build_module(
    t_len=T,
    c_dim=C,
    h_core=H_CORE,
    fh=FH,
    flags=frozenset(),
    replica_groups=GROUPS,
    local_reduce=False,
    legalize=True,
):
    """Build the per-core SPMD Bass module.

    flags: subset of {"ln1_g","ln1_b","ln2_g","ln2_b","b_qk","b_v","b_ap",
    "b_fc","b_mp"} enabling the non-trivial affine/bias paths.
    local_reduce: replace the in-group ReduceScatter with a local strided
    copy (single core test mode: takes this rank-0 slice).
    """
    KO = c_dim // P             # c-tiles
    NT = t_len // P             # token tiles
    NQC = t_len // QCH          # q chunks
    KPQ = QCH // P              # token tiles per chunk (= group size 4)
    QK = h_core * P             # stacked q+k dims
    MQK = h_core                # m-tiles of qkT
    DKO = (h_core * HD) // P    # d-tiles of y/aproj  (h_core/2)
    FKO = fh // P               # hidden tiles (32)
    NPAIR = h_core // 2
    NCC = c_dim // QCH          # 512-chunks of C
    NHC = fh // QCH             # 512-chunks of hidden (8)
    assert h_core % 2 == 0 and c_dim % P == 0 and t_len % QCH == 0

    nc = bass.Bass(num_devices=N_CORES)

    x_tm = nc.dram_tensor("x_tm", (t_len, c_dim), F32, kind="ExternalInput")
    x_own = nc.dram_tensor("x_own", (NQC * P, c_dim), F32, kind="ExternalInput")
    w_qk = nc.dram_tensor("w_qk", (P, KO, QK), BF16, kind="ExternalInput")
    w_v = nc.dram_tensor("w_v", (P, KO, h_core * HD), BF16, kind="ExternalInput")
    w_ap = nc.dram_tensor("w_ap", (P, DKO, c_dim), BF16, kind="ExternalInput")
    w_fcT = nc.dram_tensor("w_fcT", (P, KO, fh), BF16, kind="ExternalInput")
    w_mpT = nc.dram_tensor("w_mpT", (P, FKO, c_dim), BF16, kind="ExternalInput")
    tri = nc.dram_tensor("tri", (P, P), F32, kind="ExternalInput")
    opt_in = {}
    for name, shape in [
        ("ln1_g", (1, c_dim)), ("ln1_b", (1, c_dim)),
        ("ln2_g", (1, c_dim)), ("ln2_b", (1, c_dim)),
        ("b_qk", (P, MQK)), ("b_v", (1, h_core * HD)), ("b_ap", (1, c_dim)),
        ("b_fc", (1, fh)), ("b_mp", (1, c_dim)),
    ]:
        if name in flags:
            opt_in[name] = nc.dram_tensor(name, shape, F32, kind="ExternalInput")

    # per-core output: its 4 owned token tiles, row qc*128+p = token
    # tile (qc*4 + rank), host reassembles.
    out_y = nc.dram_tensor("out", (NQC * P, c_dim), F32, kind="ExternalOutput")

    # collective payloads travel in bf16: halves the wire time; the partial
    # projections are O(1)-magnitude so the rounding is ~1e-3 relative.
    ARDT = BF16
    rs_in = [nc.dram_tensor(f"rs_in{i}", (QCH, c_dim), ARDT) for i in range(NQC)]
    rs_out = [nc.dram_tensor(f"rs_out{i}", (P, c_dim), ARDT) for i in range(NQC)]
    # DRAM bounce rows for the softmax-denominator partition broadcast
    recip_d = nc.dram_tensor("recip_d", (NQC * h_core, QCH), F32)

    with TileContext(nc) as tc, contextlib.ExitStack() as ctx:
        const = ctx.enter_context(tc.tile_pool(name="const", bufs=1))
        workb = ctx.enter_context(tc.tile_pool(name="workb", bufs=2))
        works = ctx.enter_context(tc.tile_pool(name="works", bufs=3))
        stats = ctx.enter_context(tc.tile_pool(name="stats", bufs=6))

        ident = const.tile([P, P], BF16)
        make_identity(nc, ident)
        ones64 = const.tile([1, 64], F32)
        nc.vector.memset(ones64[:], 1.0)
        eps_t = const.tile([P, 1], F32)
        nc.vector.memset(eps_t[:], EPS)
        tri_sb = const.tile([P, P], F32)
        nc.sync.dma_start(tri_sb[:], tri[:])

        # optional affine operands, broadcast to 128 partitions once
        def _bcast_row(name, width):
            if name not in opt_in:
                return None
            bc = const.tile([P, width], F32, name=f"bc_{name}", tag=f"bc_{name}")
            nc.sync.dma_start(bc[:], opt_in[name][:].to_broadcast((P, width)))
            return bc

        def _col(name):
            if name not in opt_in:
                return None
            t_ = const.tile(list(opt_in[name].shape), F32, name=f"col_{name}", tag=f"col_{name}")
            nc.sync.dma_start(t_[:], opt_in[name][:])
            return t_

        ln1_g_bc = _bcast_row("ln1_g", c_dim)
        ln1_b_bc = _bcast_row("ln1_b", c_dim)
        ln2_g_bc = _bcast_row("ln2_g", c_dim)
        ln2_b_bc = _bcast_row("ln2_b", c_dim)
        b_v_bc = _bcast_row("b_v", h_core * HD)
        b_ap_bc = _bcast_row("b_ap", c_dim)
        b_fc_bc = _bcast_row("b_fc", fh)
        b_mp_bc = _bcast_row("b_mp", c_dim)
        b_qk_col = _col("b_qk")

        ps_tr = ctx.enter_context(tc.tile_pool(name="ps_tr", bufs=2, space="PSUM"))
        ps_mm = ctx.enter_context(tc.tile_pool(name="ps_mm", bufs=4, space="PSUM"))
        ps_y = ctx.enter_context(tc.tile_pool(name="ps_y", bufs=2, space="PSUM"))

        def ln_tile(x_f32, g_bc, b_bc):
            """LayerNorm of a [P, c_dim] fp32 AP -> new [P, c_dim] bf16 tile.
            rsqrt via Ln+Exp: shares the Scalar activation table with the
            softmax Exp, so no table reloads between LN and attention."""
            nsub = c_dim // 512
            st = stats.tile([P, nsub, 6], F32)
            for j in range(nsub):
                nc.vector.bn_stats(st[:, j, :], x_f32[:, j * 512:(j + 1) * 512])
            mv = stats.tile([P, 2], F32)
            nc.vector.bn_aggr(mv[:], st[:])
            r = stats.tile([P, 1], F32)
            nc.scalar.activation(
                r[:], mv[:, 1:2], mybir.ActivationFunctionType.Ln, bias=eps_t[:]
            )
            nc.scalar.activation(
                r[:], r[:], mybir.ActivationFunctionType.Exp, scale=-0.5
            )
            if g_bc is None and b_bc is None:
                h_bf = works.tile([P, c_dim], BF16, tag="ln_out", name="h_bf")
                nc.vector.tensor_scalar(
                    out=h_bf[:], in0=x_f32[:], scalar1=mv[:, 0:1], scalar2=r[:],
                    op0=mybir.AluOpType.subtract, op1=mybir.AluOpType.mult,
                )
            else:
                h_f = workb.tile([P, c_dim], F32, tag="ln_f32", name="h_f")
                nc.vector.tensor_scalar(
                    out=h_f[:], in0=x_f32[:], scalar1=mv[:, 0:1], scalar2=r[:],
                    op0=mybir.AluOpType.subtract, op1=mybir.AluOpType.mult,
                )
                if g_bc is not None:
                    nc.vector.tensor_mul(h_f[:], h_f[:], g_bc[:])
                if b_bc is not None:
                    nc.vector.tensor_add(h_f[:], h_f[:], b_bc[:])
                h_bf = works.tile([P, c_dim], BF16, tag="ln_out", name="h_bf")
                nc.vector.tensor_copy(h_bf[:], h_f[:])
            return h_bf

        def transpose_into(dstT, src_bf, tl, n_k):
            """PE-transpose [P, n_k*128] bf16 into dstT[:, :, tl*P:(tl+1)*P].
            Four 128x128 transposes share one PSUM bank so a single DVE copy
            evacuates them."""
            for kg in range(0, n_k, 4):
                nb = min(4, n_k - kg)
                pst = ps_tr.tile([P, 4 * P], BF16, tag="pst", name="pst", bufs=1)
                for j in range(nb):
                    nc.tensor.transpose(
                        pst[:, j * P:(j + 1) * P],
                        src_bf[:, (kg + j) * P:(kg + j + 1) * P],
                        ident[:],
                    )
                nc.vector.tensor_copy(
                    dstT[:, kg:kg + nb, tl * P:(tl + 1) * P],
                    pst[:, 0:nb * P].rearrange("p (a b) -> p a b", a=nb),
                )

        # =============== the MLP weight pools (persistent) =================
        # w_fcT preloads during attention (SBUF has room); w_mpT loads into
        # the space the attention pools free, overlapping the fc stages.
        wfc_pool = ctx.enter_context(tc.tile_pool(name="wfc", bufs=1))
        h2p = ctx.enter_context(tc.tile_pool(name="h2p", bufs=4))

        # ======================= attention phase ===========================
        with contextlib.ExitStack() as attn_ctx:
            # weights go on the Activation engine's DMA queue so the big
            # transfers never head-of-line-block the latency-critical
            # activation loads on the sync queue.
            wa = attn_ctx.enter_context(tc.tile_pool(name="wa", bufs=1))
            w_qk_sb = wa.tile([P, KO, QK], BF16)
            nc.scalar.dma_start(w_qk_sb[:], w_qk[:])
            w_v_sb = wa.tile([P, KO, h_core * HD], BF16)
            nc.scalar.dma_start(w_v_sb[:], w_v[:])
            w_ap_sb = wa.tile([P, DKO, c_dim], BF16)
            nc.scalar.dma_start(w_ap_sb[:], w_ap[:])

            big = attn_ctx.enter_context(tc.tile_pool(name="big", bufs=1))
            qkT = big.tile([P, MQK, t_len], BF16)
            vaug = big.tile([P, NT, h_core, 65], BF16)
            nc.vector.memset(vaug[:, :, :, 64:65], 1.0)

            h1p = attn_ctx.enter_context(tc.tile_pool(name="h1p", bufs=2))
            yp = attn_ctx.enter_context(tc.tile_pool(name="yp", bufs=2))
            pt_pool = attn_ctx.enter_context(tc.tile_pool(name="pt", bufs=5))
            rowp = attn_ctx.enter_context(tc.tile_pool(name="rows", bufs=2))
            arp = attn_ctx.enter_context(tc.tile_pool(name="arp", bufs=2))

            w_fc_sb = wfc_pool.tile([P, KO, fh], BF16)

            def stage_A_pre(qc):
                """LN1 + transposed activations for chunk qc (DVE/Scalar
                heavy; emitted a chunk ahead so the PE never waits on it)."""
                h1T = h1p.tile([P, KO, QCH], BF16, tag="h1T", name="h1T")
                for tl in range(KPQ):
                    tt = qc * KPQ + tl
                    xt = workb.tile([P, c_dim], F32, tag="x_in", name="xt")
                    nc.sync.dma_start(xt[:], x_tm[tt * P:(tt + 1) * P, :])
                    h_bf = ln_tile(xt, ln1_g_bc, ln1_b_bc)
                    transpose_into(h1T, h_bf, tl, KO)
                return h1T

            def stage_A(qc, h1T):
                # qkT chunk (transposed-output form)
                for mt in range(MQK):
                    ps = ps_mm.tile([P, QCH], F32, tag="ps", name="ps")
                    for ko in range(KO):
                        nc.tensor.matmul(
                            ps[:],
                            w_qk_sb[:, ko, mt * P:(mt + 1) * P],
                            h1T[:, ko, :],
                            start=(ko == 0),
                            stop=(ko == KO - 1),
                        )
                    dst = qkT[:, mt, qc * QCH:(qc + 1) * QCH]
                    if b_qk_col is not None:
                        nc.vector.tensor_scalar_add(dst, ps[:], b_qk_col[:, mt:mt + 1])
                    else:
                        nc.vector.tensor_copy(dst, ps[:])

                # v token-major for this chunk's tiles
                for tl in range(KPQ):
                    tt = qc * KPQ + tl
                    ps = ps_mm.tile([P, QCH], F32, tag="ps", name="ps")
                    for ko in range(KO):
                        nc.tensor.matmul(
                            ps[:, 0:h_core * HD],
                            h1T[:, ko, tl * P:(tl + 1) * P],
                            w_v_sb[:, ko, :],
                            start=(ko == 0),
                            stop=(ko == KO - 1),
                        )
                    if b_v_bc is not None:
                        nc.vector.tensor_add(
                            ps[:, 0:h_core * HD], ps[:, 0:h_core * HD], b_v_bc[:]
                        )
                    for h in range(h_core):
                        nc.vector.tensor_copy(
                            vaug[:, tt, h, 0:64], ps[:, h * HD:(h + 1) * HD]
                        )

                # causal attention, one head-pair at a time
                yT = yp.tile([P, DKO, QCH], BF16, tag="yT", name="yT")
                for pr in range(NPAIR):
                    heads = ((0, 2 * pr), (64, 2 * pr + 1))
                    psy = {}
                    for sub, h in heads:
                        psy[h] = ps_y.tile([P, QCH], F32, tag="psy", name=f"psy_{h}")
                    nkt = (qc + 1) * KPQ
                    pts = {}

                    def emit_s_exp(kt, heads=heads, pts=pts, qc=qc, pr=pr):
                        i = kt - qc * KPQ  # >=0 on the diagonal band
                        for sub, h in heads:
                            pss = ps_mm.tile([P, QCH], F32, tag="ps", name=f"pss_{h}")
                            nc.tensor.matmul(
                                pss[:],
                                qkT[sub:sub + 64, DKO + pr, kt * P:(kt + 1) * P],
                                qkT[sub:sub + 64, pr, qc * QCH:(qc + 1) * QCH],
                                start=True,
                                stop=True,
                            )
                            pt = pt_pool.tile([P, QCH], BF16, tag="pt", name=f"pt_{h}")
                            if i >= 0:
                                if i > 0:
                                    nc.vector.memset(pt[:, 0:i * P], 0.0)
                                nc.vector.tensor_add(
                                    pss[:, i * P:(i + 1) * P],
                                    pss[:, i * P:(i + 1) * P],
                                    tri_sb[:],
                                )
                                nc.scalar.activation(
                                    pt[:, i * P:QCH],
                                    pss[:, i * P:QCH],
                                    mybir.ActivationFunctionType.Exp,
                                )
                            else:
                                nc.scalar.activation(
                                    pt[:], pss[:], mybir.ActivationFunctionType.Exp
                                )
                            pts[(kt, h)] = pt

                    def emit_pv(kt, heads=heads, psy=psy, pts=pts, nkt=nkt):
                        for sub, h in heads:
                            nc.tensor.matmul(
                                psy[h][0:65, :],
                                vaug[:, kt, h, :],
                                pts.pop((kt, h))[:],
                                start=(kt == 0),
                                stop=(kt == nkt - 1),
                            )

                    for kt in range(nkt):
                        emit_s_exp(kt)
                        if kt > 0:
                            emit_pv(kt - 1)
                    emit_pv(nkt - 1)
                    for sub, h in heads:
                        # evacuate the whole psy bank once via DVE (frees the
                        # PSUM bank for the next pair immediately, and keeps
                        # the Scalar engine free for the softmax exps)
                        psy_sb = rowp.tile([65, QCH], F32, tag="psy_sb", name="psy_sb")
                        nc.vector.tensor_copy(psy_sb[:], psy[h][0:65, :])
                        # reciprocal of the denominator row via exp(-ln(x)):
                        # two cheap Scalar ops on the activation table already
                        # loaded for the softmax (DVE reciprocal is ~6ns/elem
                        # per partition -- 3.2us for a 512-wide row).  The Ln
                        # also moves the row to partition 0 for the PE.
                        row = rowp.tile([1, QCH], F32, tag="rec", name="row", bufs=1)
                        nc.scalar.activation(
                            row[:], psy_sb[64:65, :],
                            mybir.ActivationFunctionType.Ln,
                        )
                        nc.scalar.activation(
                            row[:], row[:],
                            mybir.ActivationFunctionType.Exp, scale=-1.0,
                        )
                        # PE broadcast of the reciprocal row to 64 partitions;
                        # the DVE multiply reads it straight from PSUM
                        bc = ps_tr.tile([64, QCH], F32, tag="bc", name="bc",
                                        bufs=1)
                        nc.tensor.matmul(
                            bc[:], ones64[:], row[:],
                            start=True, stop=True,
                        )
                        nc.vector.tensor_tensor(
                            yT[sub:sub + 64, pr, :],
                            psy_sb[0:64, :],
                            bc[:],
                            mybir.AluOpType.mult,
                        )

                # out-projection partials -> one staging tile -> one DMA
                ar_st = arp.tile([P, KPQ, c_dim], ARDT, tag="ar_st", name="ar_st")
                for tl in range(KPQ):
                    for nch in range(NCC):
                        ps = ps_mm.tile([P, QCH], F32, tag="ps", name="ps")
                        for dk in range(DKO):
                            nc.tensor.matmul(
                                ps[:],
                                yT[:, dk, tl * P:(tl + 1) * P],
                                w_ap_sb[:, dk, nch * QCH:(nch + 1) * QCH],
                                start=(dk == 0),
                                stop=(dk == DKO - 1),
                            )
                        dst = ar_st[:, tl, nch * QCH:(nch + 1) * QCH]
                        if b_ap_bc is not None:
                            nc.vector.tensor_add(
                                dst, ps[:], b_ap_bc[:, nch * QCH:(nch + 1) * QCH]
                            )
                        else:
                            nc.vector.tensor_copy(dst, ps[:])
                nc.sync.dma_start(
                    rs_in[qc][:].rearrange("(a p) c -> p a c", p=P), ar_st[:]
                )
                if local_reduce:
                    nc.sync.dma_start(rs_out[qc][:], rs_in[qc][0:P, :])
                else:
                    nc.gpsimd.collective_compute(
                        "ReduceScatter",
                        mybir.AluOpType.add,
                        replica_groups=replica_groups,
                        ins=[rs_in[qc][:]],
                        outs=[rs_out[qc][:]],
                    )

            def x1_tile(qc):
                """x1 = x_own + attn for owned tile of chunk qc, fp32.
                The rs_out read is a DEPENDENT dma on the in-order sync
                queue: only emit once nothing latency-critical queues
                behind it."""
                xt = workb.tile([P, c_dim], F32, tag="x1f", name="xt",
                                bufs=4)
                nc.sync.dma_start(xt[:], x_own[qc * P:(qc + 1) * P, :])
                at = workb.tile([P, c_dim], ARDT, tag="ar_rd", name="at")
                nc.sync.dma_start(at[:], rs_out[qc][:])
                nc.vector.tensor_add(xt[:], xt[:], at[:])
                return xt

            def stage_F_pre(qc):
                """x1 + LN2 + transposed h2 for the owned tile of chunk qc.
                Returns (h2T, x1); x1 is kept for the final residual."""
                x1 = x1_tile(qc)
                h_bf = ln_tile(x1, ln2_g_bc, ln2_b_bc)
                h2T = h2p.tile([P, KO, P], BF16, tag="h2T", name="h2T")
                transpose_into(h2T, h_bf, 0, KO)
                return h2T, x1

            h1s = {}
            h2s = {}
            x1s = {}
            h1s[0] = stage_A_pre(0)
            h1s[1] = stage_A_pre(1)
            stage_A(0, h1s.pop(0))
            h1s[2] = stage_A_pre(2)
            stage_A(1, h1s.pop(1))
            # preload the full fc weight mid-attention: late enough not to
            # contend for HBM with the startup x/weight loads, early enough
            # to land long before the fc stages need it.
            nc.scalar.dma_start(w_fc_sb[:], w_fcT[:])
            h1s[3] = stage_A_pre(3)
            stage_A(2, h1s.pop(2))
            # all attention x loads and chunk<=2 sync-queue work emitted;
            # the LN2 stages overlap the remaining attention compute
            h2s[0], x1s[0] = stage_F_pre(0)
            stage_A(3, h1s.pop(3))
            h2s[1], x1s[1] = stage_F_pre(1)

        # ======================== MLP phase ================================
        # token-parallel over the 4 owned tiles; full weights, no collective
        wmp_pool = ctx.enter_context(tc.tile_pool(name="wmp", bufs=1))
        w_mp_sb = wmp_pool.tile([P, FKO, c_dim], BF16)
        nc.scalar.dma_start(w_mp_sb[:], w_mpT[:])

        gsp = ctx.enter_context(tc.tile_pool(name="gsp", bufs=2))
        gtp = ctx.enter_context(tc.tile_pool(name="gtp", bufs=2))

        def stage_F(qc, h2T):
            """fc + gelu + transpose for the owned tile of chunk qc."""
            g_s = gsp.tile([P, NHC, QCH], BF16, tag="g_s", name="g_s")
            # two PSUM half-rounds of 4 h-chunks: stationary h2T[ko] is
            # amortized over 4 moving-512 matmuls per load
            for half in range(2):
                pss = [
                    ps_mm.tile([P, QCH], F32, tag="ps", name=f"psf_{i}")
                    for i in range(4)
                ]
                for ko in range(KO):
                    for i in range(4):
                        hc = half * 4 + i
                        nc.tensor.matmul(
                            pss[i][:],
                            h2T[:, ko, :],
                            w_fc_sb[:, ko, hc * QCH:(hc + 1) * QCH],
                            start=(ko == 0),
                            stop=(ko == KO - 1),
                        )
                for i in range(4):
                    hc = half * 4 + i
                    if b_fc_bc is not None:
                        nc.vector.tensor_add(
                            pss[i][:], pss[i][:],
                            b_fc_bc[:, hc * QCH:(hc + 1) * QCH],
                        )
                    nc.scalar.activation(
                        g_s[:, hc, :], pss[i][:],
                        mybir.ActivationFunctionType.Gelu_apprx_tanh,
                    )

            gT = gtp.tile([P, FKO, P], BF16, tag="gT", name="gT")
            for hc in range(NHC):
                transpose_into(gT[:, hc * 4:(hc + 1) * 4, :], g_s[:, hc, :], 0, 4)
            return gT

        def stage_M(qc, gT, x1):
            """Down-projection + final residual + store for chunk qc."""
            for nch in range(NCC):
                ps = ps_mm.tile([P, QCH], F32, tag="ps", name="ps")
                for hk in range(FKO):
                    nc.tensor.matmul(
                        ps[:],
                        gT[:, hk, :],
                        w_mp_sb[:, hk, nch * QCH:(nch + 1) * QCH],
                        start=(hk == 0),
                        stop=(hk == FKO - 1),
                    )
                if b_mp_bc is not None:
                    nc.vector.tensor_add(
                        ps[:], ps[:], b_mp_bc[:, nch * QCH:(nch + 1) * QCH]
                    )
                ev = works.tile([P, QCH], F32, tag="evac", name="ev", bufs=2)
                nc.vector.tensor_tensor(
                    ev[:], ps[:], x1[:, nch * QCH:(nch + 1) * QCH],
                    mybir.AluOpType.add,
                )
                nc.sync.dma_start(
                    out_y[qc * P:(qc + 1) * P, nch * QCH:(nch + 1) * QCH], ev[:]
                )

        gts = {}
        h2s[2], x1s[2] = stage_F_pre(2)
        gts[0] = stage_F(0, h2s.pop(0))
        h2s[3], x1s[3] = stage_F_pre(3)
        gts[1] = stage_F(1, h2s.pop(1))
        stage_M(0, gts.pop(0), x1s.pop(0))
        gts[2] = stage_F(2, h2s.pop(2))
        stage_M(1, gts.pop(1), x1s.pop(1))
        gts[3] = stage_F(3, h2s.pop(3))
        stage_M(2, gts.pop(2), x1s.pop(2))
        stage_M(3, gts.pop(3), x1s.pop(3))

    if legalize:
        _legalize_waits(nc)
    return nc


# ---------------------------------------------------------------------------
# host-side sharding / layout prep


def _tile_k(arr, width):
    """[K, M] -> [128, K//128, M] (contraction dim inner on partitions)."""
    k, m = arr.shape
    assert m == width and k % P == 0
    return np.ascontiguousarray(
        arr.reshape(k // P, P, m).transpose(1, 0, 2)
    )


def _bf(arr):
    return arr.astype(ml_dtypes.bfloat16)


def make_core_inputs(inputs, t_len=T, c_dim=C, h_core=H_CORE, fh=FH,
                     n_groups=len(GROUPS), tpg=TPG):
    """Shard + lay out the full inputs into per-core input dicts and the
    active-flag set."""
    f32 = np.float32
    x = np.asarray(inputs["x"], f32)
    W_attn = np.asarray(inputs["W_attn"], f32)
    W_aproj = np.asarray(inputs["W_aproj"], f32)
    W_fc = np.asarray(inputs["W_fc"], f32)
    W_mproj = np.asarray(inputs["W_mproj"], f32)
    ln1_g = np.asarray(inputs["ln1_g"], f32)
    ln1_b = np.asarray(inputs["ln1_b"], f32)
    ln2_g = np.asarray(inputs["ln2_g"], f32)
    ln2_b = np.asarray(inputs["ln2_b"], f32)
    b_attn = np.asarray(inputs["b_attn"], f32)
    b_aproj = np.asarray(inputs["b_aproj"], f32)
    b_fc = np.asarray(inputs["b_fc"], f32)
    b_mproj = np.asarray(inputs["b_mproj"], f32)

    Wq, Wk, Wv = W_attn[:c_dim], W_attn[c_dim:2 * c_dim], W_attn[2 * c_dim:]
    bq, bk, bv = b_attn[:c_dim], b_attn[c_dim:2 * c_dim], b_attn[2 * c_dim:]
    scale = 1.0 / math.sqrt(HD)

    flags = set()
    if not np.all(ln1_g == 1.0):
        flags.add("ln1_g")
    if np.any(ln1_b):
        flags.add("ln1_b")
    if not np.all(ln2_g == 1.0):
        flags.add("ln2_g")
    if np.any(ln2_b):
        flags.add("ln2_b")
    if np.any(b_attn[:2 * c_dim]):
        flags.add("b_qk")
    if np.any(bv):
        flags.add("b_v")
    if np.any(b_aproj):
        flags.add("b_ap")
    if np.any(b_fc):
        flags.add("b_fc")
    if np.any(b_mproj):
        flags.add("b_mp")

    tri = np.where(
        np.arange(P)[:, None] > np.arange(P)[None, :], f32(-1e30), f32(0.0)
    ).astype(f32)

    # replicated full MLP weights, transposed layouts (contraction inner)
    w_fcT_full = _tile_k(_bf(np.ascontiguousarray(W_fc.T)), fh)
    w_mpT_full = _tile_k(_bf(np.ascontiguousarray(W_mproj.T)), c_dim)

    NQC = t_len // QCH

    in_maps = []
    for core in range(n_groups * tpg):
        g, s = core // tpg, core % tpg
        heads = range(s * h_core, (s + 1) * h_core)
        # stacked [q heads | k heads] output dims, q pre-scaled by 1/sqrt(hd)
        w_qk_rows = np.concatenate(
            [Wq[h * HD:(h + 1) * HD] * scale for h in heads]
            + [Wk[h * HD:(h + 1) * HD] for h in heads], axis=0
        )  # [QK, C]
        w_v_rows = np.concatenate(
            [Wv[h * HD:(h + 1) * HD] for h in heads], axis=0
        )  # [DH, C]
        dsl = slice(s * h_core * HD, (s + 1) * h_core * HD)
        xg = x[g % x.shape[0]]
        x_own = np.concatenate(
            [xg[(qc * tpg + s) * P:(qc * tpg + s + 1) * P] for qc in range(NQC)],
            axis=0,
        )
        m = {
            "x_tm": np.ascontiguousarray(xg),
            "x_own": np.ascontiguousarray(x_own),
            "w_qk": _tile_k(_bf(w_qk_rows.T), h_core * P),
            "w_v": _tile_k(_bf(w_v_rows.T), h_core * HD),
            "w_ap": _tile_k(_bf(W_aproj[:, dsl].T.copy()), c_dim),
            "w_fcT": w_fcT_full,
            "w_mpT": w_mpT_full,
            "tri": tri,
        }
        if "ln1_g" in flags:
            m["ln1_g"] = ln1_g.reshape(1, -1).copy()
        if "ln1_b" in flags:
            m["ln1_b"] = ln1_b.reshape(1, -1).copy()
        if "ln2_g" in flags:
            m["ln2_g"] = ln2_g.reshape(1, -1).copy()
        if "ln2_b" in flags:
            m["ln2_b"] = ln2_b.reshape(1, -1).copy()
        if "b_qk" in flags:
            b_qk_rows = np.concatenate(
                [bq[h * HD:(h + 1) * HD] * scale for h in heads]
                + [bk[h * HD:(h + 1) * HD] for h in heads]
            )  # [QK] along partitions: [P, MQK]
            m["b_qk"] = np.ascontiguousarray(
                b_qk_rows.reshape(h_core, P).T
            )
        if "b_v" in flags:
            m["b_v"] = np.concatenate(
                [bv[h * HD:(h + 1) * HD] for h in heads]
            ).reshape(1, -1).copy()
        if "b_ap" in flags:
            m["b_ap"] = (b_aproj / tpg).reshape(1, -1).copy()
        if "b_fc" in flags:
            m["b_fc"] = b_fc.reshape(1, -1).copy()
        if "b_mp" in flags:
            m["b_mp"] = b_mproj.reshape(1, -1).copy()
        in_maps.append(m)
    return in_maps, frozenset(flags)


# ---------------------------------------------------------------------------
# runner

_module_cache = {}


def run(inputs, trace=False, trace_kwargs=None, tmpdir=None):
    in_maps, flags = make_core_inputs(inputs)
    key = (flags, trace)
    if key not in _module_cache:
        _module_cache[key] = build_module(flags=flags)
    nc = _module_cache[key]
    if trace:
        _install_prof_hook()
    res = run_bass_kernel_spmd(
        nc,
        in_maps,
        core_ids=list(range(N_CORES)),
        trace=trace,
        tmpdir=tmpdir,
        **(trace_kwargs or {}),
    )
    # reassemble: core g*TPG+s provides token tiles (qc*TPG + s) of batch g
    NQC = T // QCH
    out = np.empty((B, T, C), np.float32)
    for g in range(len(GROUPS)):
        for s in range(TPG):
            o = res.results[g * TPG + s]["out"]
            for qc in range(NQC):
                tt = qc * TPG + s
                out[g, tt * P:(tt + 1) * P, :] = o[qc * P:(qc + 1) * P, :]
    return out, res


def kernel(**inputs) -> np.ndarray:
    out, _ = run(inputs, trace=False)
    return out


# revision 39
# speedup vs baseline: 1.1515x; 1.0808x over previous
"""Fused causal-transformer block (LN1 -> attn -> LN2 -> MLP, residuals) on
8 Trainium2 NeuronCores.

Sharding: 2 groups of 4 cores; group g handles batch element b=g (data
parallel).  Within a group:
  - Attention is Megatron head-parallel: core s owns 4 heads, computes
    partial y = attn(x) @ W_aproj_s for ALL tokens, chunked over four
    512-token chunks.  Each chunk's partials are summed with an in-group
    ReduceScatter, leaving core s with the summed attention output for
    token tile (chunk*4 + s) -- its 128-token slice of each chunk.
  - The MLP is token-parallel with REPLICATED weights: core s runs the
    full 4C-hidden MLP for its 4 owned token tiles (512 tokens total).
    No second collective is needed; the host reassembles token slices.
This cuts the collective count from 8 AllReduces to 4 ReduceScatters,
all hidden behind attention compute (the single CC core was the
bottleneck of the AllReduce design).

Compute dtype: bf16 matmul inputs, fp32 PSUM accumulation, fp32 residual
stream and softmax statistics.

Layouts (per core, all prepared host-side in kernel()):
  h1T/h2T  : [128, C/128, t]  activations transposed (contraction dim on
             partitions) produced on-device via PE transposes.
  qkT      : [128, H_core, T] rows = [q heads | k heads] * 64-dim each,
             two heads stacked per 128-partition tile.  Scores are
             computed directly in S^T [k, q] layout, so softmax
             normalization arrives as a PSUM row via a ones-column in v.
  v_aug    : [128, T/128, H_core, 65]  v token-major per head + ones col.
  w_fcT    : [128, C/128, 4C]  full W_fc^T (replicated), moving operand.
  w_mpT    : [128, 4C/128, C]  full W_mproj^T (replicated), moving.
"""

import contextlib
import ctypes
import math
import sys
import types

import numpy as np
import ml_dtypes

import bass_rust
import concourse.bass as bass
import concourse.mybir as mybir
import concourse.tile as tile
from concourse import library_config
from concourse.bass_utils import run_bass_kernel_spmd
from concourse.masks import make_identity
from concourse.tile import TileContext
from concourse.vector_clock import ScopedClock

# ---------------------------------------------------------------------------
# problem constants (hardcoded per the harness contract)
B, T, C, H = 2, 2048, 1024, 16
HD = C // H                 # 64
N_CORES = 8
TPG = 4                     # tensor-parallel group size
H_CORE = H // TPG           # heads per core = 4
DH = H_CORE * HD            # per-core attention dim = 256
FH = 4 * C                  # full MLP hidden (replicated) = 4096
P = 128
EPS = 1e-5
QCH = 512                   # q-chunk width
GROUPS = [[0, 1, 2, 3], [4, 5, 6, 7]]

F32 = mybir.dt.float32
BF16 = mybir.dt.bfloat16

# ---------------------------------------------------------------------------
# workaround 1: the container's walrus accepts a single sync-wait command per
# instruction; move extra semaphore waits onto inserted EventSemaphore
# instructions on the same engine (program order preserves semantics).

_waitfix_counter = [0]


def _legalize_waits(nc, cap=1):
    fn = nc.m.functions[0]
    n_split = 0
    for bb in fn.blocks:
        out = []
        changed = False
        for inst in bb.instructions:
            si = inst.sync_info
            waits = list(si.on_wait) if si is not None else []
            if len(waits) > cap:
                movable = [w for w in waits if w.sync_type == "semaphore"]
                fixed = [w for w in waits if w.sync_type != "semaphore"]
                n_keep = max(cap - len(fixed), 0)
                keep = fixed + (movable[len(movable) - n_keep:] if n_keep else [])
                extra = movable[: len(movable) - n_keep] if n_keep else movable
                for w in extra:
                    _waitfix_counter[0] += 1
                    ev = mybir.InstEventSemaphore(
                        name=f"I-waitfix-{_waitfix_counter[0]}",
                        engine=inst.engine,
                        ins=[],
                        outs=[],
                        sync_info=bass_rust.SyncInfo(on_wait=[w], on_update=[]),
                    )
                    out.append(ev)
                    n_split += 1
                inst.sync_info = bass_rust.SyncInfo(
                    on_wait=keep, on_update=list(si.on_update)
                )
                changed = True
            out.append(inst)
        if changed:
            bb.instructions = out
    return n_split


# workaround 2: same issue for the Tile kernel-tail Drain — emit one wait-nop
# per live proc ahead of a wait-less drain instead of stacking waits on it.


def _drain_and_barrier_split(self, tick_clock, wait_clock):
    gc = tick_clock.global_clock
    sems_alloc = wait_clock.sems.allocated()
    for proc in sorted(sems_alloc):
        tick = gc.peek_next(proc) - 1
        if tick <= 0:
            continue
        vc1 = bass_rust.VectorClock()
        vc1.require_at_least(proc, tick)
        nop = self.nc.sync.nop()
        wait_clock.add_sem_waits(nop.ins, ScopedClock({None: vc1}))
    self.nc.sync.drain()
    self.nc.all_engine_barrier()
    assert self.sems is not None
    popped = self.nc._tile_sem_poison_stack.pop()
    assert popped is self._sem_poison
    self.nc.clear_and_free_semaphores(list(self.sems.allocated().values()))
    self.nc.all_engine_barrier()


TileContext._drain_and_barrier = _drain_and_barrier_split


# workaround 3 (profiling only): register the NTFF hook the trimmed antenv
# lacks so run_bass_kernel_spmd(trace=True) works under axon.


def _install_prof_hook():
    if "antenv.axon_hooks" in sys.modules:
        return
    so_path = "/opt/axon/libaxon_pjrt.so"
    hook = None
    try:
        lib = ctypes.CDLL(so_path)
        if hasattr(lib, "axon_start_nrt_profile"):
            lib.axon_start_nrt_profile.argtypes = [
                ctypes.POINTER(ctypes.c_int64),
                ctypes.c_size_t,
            ]
            lib.axon_start_nrt_profile.restype = ctypes.c_int64
            lib.axon_stop_nrt_profile.argtypes = [ctypes.c_char_p]
            lib.axon_stop_nrt_profile.restype = ctypes.c_int64

            @contextlib.contextmanager
            def _hook_cm(output_dir, device_ids):
                import jax

                jax.devices()
                if device_ids:
                    ids = (ctypes.c_int64 * len(device_ids))(*device_ids)
                    rc = lib.axon_start_nrt_profile(ids, len(device_ids))
                else:
                    rc = lib.axon_start_nrt_profile(None, 0)
                if rc != 0:
                    raise RuntimeError(f"axon_start_nrt_profile rc={rc}")
                try:
                    yield
                finally:
                    n = lib.axon_stop_nrt_profile(str(output_dir).encode())
                    if n < 0:
                        raise RuntimeError(f"axon_stop_nrt_profile rc={n}")

            hook = _hook_cm
    except OSError:
        pass
    mod = types.ModuleType("antenv.axon_hooks")
    mod.get_axon_ntff_profile_hook = lambda: hook
    mod.set_axon_ntff_profile_hook = lambda h: None
    sys.modules["antenv.axon_hooks"] = mod
    from concourse import bass_utils

    bass_utils.upload_artifacts = lambda tmpdir: tmpdir


# ---------------------------------------------------------------------------
# device kernel builder


def build_module(
    t_len=T,
    c_dim=C,
    h_core=H_CORE,
    fh=FH,
    flags=frozenset(),
    replica_groups=GROUPS,
    local_reduce=False,
    legalize=True,
):
    """Build the per-core SPMD Bass module.

    flags: subset of {"ln1_g","ln1_b","ln2_g","ln2_b","b_qk","b_v","b_ap",
    "b_fc","b_mp"} enabling the non-trivial affine/bias paths.
    local_reduce: replace the in-group ReduceScatter with a local strided
    copy (single core test mode: takes this rank-0 slice).
    """
    KO = c_dim // P             # c-tiles
    NT = t_len // P             # token tiles
    NQC = t_len // QCH          # q chunks
    KPQ = QCH // P              # token tiles per chunk (= group size 4)
    QK = h_core * P             # stacked q+k dims
    MQK = h_core                # m-tiles of qkT
    DKO = (h_core * HD) // P    # d-tiles of y/aproj  (h_core/2)
    FKO = fh // P               # hidden tiles (32)
    NPAIR = h_core // 2
    NCC = c_dim // QCH          # 512-chunks of C
    NHC = fh // QCH             # 512-chunks of hidden (8)
    assert h_core % 2 == 0 and c_dim % P == 0 and t_len % QCH == 0

    nc = bass.Bass(num_devices=N_CORES)

    x_tm = nc.dram_tensor("x_tm", (t_len, c_dim), F32, kind="ExternalInput")
    x_own = nc.dram_tensor("x_own", (NQC * P, c_dim), F32, kind="ExternalInput")
    w_qk = nc.dram_tensor("w_qk", (P, KO, QK), BF16, kind="ExternalInput")
    w_v = nc.dram_tensor("w_v", (P, KO, h_core * HD), BF16, kind="ExternalInput")
    w_ap = nc.dram_tensor("w_ap", (P, DKO, c_dim), BF16, kind="ExternalInput")
    w_fcT = nc.dram_tensor("w_fcT", (P, KO, fh), BF16, kind="ExternalInput")
    w_mpT = nc.dram_tensor("w_mpT", (P, FKO, c_dim), BF16, kind="ExternalInput")
    tri = nc.dram_tensor("tri", (P, P), F32, kind="ExternalInput")
    opt_in = {}
    for name, shape in [
        ("ln1_g", (1, c_dim)), ("ln1_b", (1, c_dim)),
        ("ln2_g", (1, c_dim)), ("ln2_b", (1, c_dim)),
        ("b_qk", (P, MQK)), ("b_v", (1, h_core * HD)), ("b_ap", (1, c_dim)),
        ("b_fc", (1, fh)), ("b_mp", (1, c_dim)),
    ]:
        if name in flags:
            opt_in[name] = nc.dram_tensor(name, shape, F32, kind="ExternalInput")

    # per-core output: its 4 owned token tiles, row qc*128+p = token
    # tile (qc*4 + rank), host reassembles.
    out_y = nc.dram_tensor("out", (NQC * P, c_dim), F32, kind="ExternalOutput")

    # collective payloads travel in bf16: halves the wire time; the partial
    # projections are O(1)-magnitude so the rounding is ~1e-3 relative.
    ARDT = BF16
    rs_in = [nc.dram_tensor(f"rs_in{i}", (QCH, c_dim), ARDT) for i in range(NQC)]
    rs_out = [nc.dram_tensor(f"rs_out{i}", (P, c_dim), ARDT) for i in range(NQC)]
    # DRAM bounce rows for the softmax-denominator partition broadcast
    recip_d = nc.dram_tensor("recip_d", (NQC * h_core, QCH), F32)

    with TileContext(nc) as tc, contextlib.ExitStack() as ctx:
        const = ctx.enter_context(tc.tile_pool(name="const", bufs=1))
        workb = ctx.enter_context(tc.tile_pool(name="workb", bufs=2))
        works = ctx.enter_context(tc.tile_pool(name="works", bufs=3))
        stats = ctx.enter_context(tc.tile_pool(name="stats", bufs=6))

        ident = const.tile([P, P], BF16)
        make_identity(nc, ident)
        ones64 = const.tile([1, 64], F32)
        nc.vector.memset(ones64[:], 1.0)
        eps_t = const.tile([P, 1], F32)
        nc.vector.memset(eps_t[:], EPS)
        tri_sb = const.tile([P, P], F32)
        nc.sync.dma_start(tri_sb[:], tri[:])

        # optional affine operands, broadcast to 128 partitions once
        def _bcast_row(name, width):
            if name not in opt_in:
                return None
            bc = const.tile([P, width], F32, name=f"bc_{name}", tag=f"bc_{name}")
            nc.sync.dma_start(bc[:], opt_in[name][:].to_broadcast((P, width)))
            return bc

        def _col(name):
            if name not in opt_in:
                return None
            t_ = const.tile(list(opt_in[name].shape), F32, name=f"col_{name}", tag=f"col_{name}")
            nc.sync.dma_start(t_[:], opt_in[name][:])
            return t_

        ln1_g_bc = _bcast_row("ln1_g", c_dim)
        ln1_b_bc = _bcast_row("ln1_b", c_dim)
        ln2_g_bc = _bcast_row("ln2_g", c_dim)
        ln2_b_bc = _bcast_row("ln2_b", c_dim)
        b_v_bc = _bcast_row("b_v", h_core * HD)
        b_ap_bc = _bcast_row("b_ap", c_dim)
        b_fc_bc = _bcast_row("b_fc", fh)
        b_mp_bc = _bcast_row("b_mp", c_dim)
        b_qk_col = _col("b_qk")

        ps_tr = ctx.enter_context(tc.tile_pool(name="ps_tr", bufs=2, space="PSUM"))
        ps_mm = ctx.enter_context(tc.tile_pool(name="ps_mm", bufs=5, space="PSUM"))
        ps_y = ctx.enter_context(tc.tile_pool(name="ps_y", bufs=2, space="PSUM"))

        def ln_tile(x_f32, g_bc, b_bc):
            """LayerNorm of a [P, c_dim] fp32 AP -> new [P, c_dim] bf16 tile.
            rsqrt via Ln+Exp: shares the Scalar activation table with the
            softmax Exp, so no table reloads between LN and attention."""
            nsub = c_dim // 512
            st = stats.tile([P, nsub, 6], F32)
            for j in range(nsub):
                nc.vector.bn_stats(st[:, j, :], x_f32[:, j * 512:(j + 1) * 512])
            mv = stats.tile([P, 2], F32)
            nc.vector.bn_aggr(mv[:], st[:])
            r = stats.tile([P, 1], F32)
            nc.scalar.activation(
                r[:], mv[:, 1:2], mybir.ActivationFunctionType.Ln, bias=eps_t[:]
            )
            nc.scalar.activation(
                r[:], r[:], mybir.ActivationFunctionType.Exp, scale=-0.5
            )
            if g_bc is None and b_bc is None:
                h_bf = works.tile([P, c_dim], BF16, tag="ln_out", name="h_bf")
                nc.vector.tensor_scalar(
                    out=h_bf[:], in0=x_f32[:], scalar1=mv[:, 0:1], scalar2=r[:],
                    op0=mybir.AluOpType.subtract, op1=mybir.AluOpType.mult,
                )
            else:
                h_f = workb.tile([P, c_dim], F32, tag="ln_f32", name="h_f")
                nc.vector.tensor_scalar(
                    out=h_f[:], in0=x_f32[:], scalar1=mv[:, 0:1], scalar2=r[:],
                    op0=mybir.AluOpType.subtract, op1=mybir.AluOpType.mult,
                )
                if g_bc is not None:
                    nc.vector.tensor_mul(h_f[:], h_f[:], g_bc[:])
                if b_bc is not None:
                    nc.vector.tensor_add(h_f[:], h_f[:], b_bc[:])
                h_bf = works.tile([P, c_dim], BF16, tag="ln_out", name="h_bf")
                nc.vector.tensor_copy(h_bf[:], h_f[:])
            return h_bf

        def transpose_into(dstT, src_bf, tl, n_k):
            """PE-transpose [P, n_k*128] bf16 into dstT[:, :, tl*P:(tl+1)*P].
            Four 128x128 transposes share one PSUM bank so a single DVE copy
            evacuates them."""
            for kg in range(0, n_k, 4):
                nb = min(4, n_k - kg)
                pst = ps_tr.tile([P, 4 * P], BF16, tag="pst", name="pst", bufs=1)
                for j in range(nb):
                    nc.tensor.transpose(
                        pst[:, j * P:(j + 1) * P],
                        src_bf[:, (kg + j) * P:(kg + j + 1) * P],
                        ident[:],
                    )
                nc.vector.tensor_copy(
                    dstT[:, kg:kg + nb, tl * P:(tl + 1) * P],
                    pst[:, 0:nb * P].rearrange("p (a b) -> p a b", a=nb),
                )

        # =============== the MLP weight pools (persistent) =================
        # w_fcT preloads during attention (SBUF has room); w_mpT loads into
        # the space the attention pools free, overlapping the fc stages.
        wfc_pool = ctx.enter_context(tc.tile_pool(name="wfc", bufs=1))
        h2p = ctx.enter_context(tc.tile_pool(name="h2p", bufs=4))

        # ======================= attention phase ===========================
        with contextlib.ExitStack() as attn_ctx:
            # weights go on the Activation engine's DMA queue so the big
            # transfers never head-of-line-block the latency-critical
            # activation loads on the sync queue.
            wa = attn_ctx.enter_context(tc.tile_pool(name="wa", bufs=1))
            w_qk_sb = wa.tile([P, KO, QK], BF16)
            nc.scalar.dma_start(w_qk_sb[:], w_qk[:])
            w_v_sb = wa.tile([P, KO, h_core * HD], BF16)
            nc.scalar.dma_start(w_v_sb[:], w_v[:])
            w_ap_sb = wa.tile([P, DKO, c_dim], BF16)
            nc.scalar.dma_start(w_ap_sb[:], w_ap[:])

            big = attn_ctx.enter_context(tc.tile_pool(name="big", bufs=1))
            qkT = big.tile([P, MQK, t_len], BF16)
            vaug = big.tile([P, NT, h_core, 65], BF16)
            nc.vector.memset(vaug[:, :, :, 64:65], 1.0)

            h1p = attn_ctx.enter_context(tc.tile_pool(name="h1p", bufs=2))
            yp = attn_ctx.enter_context(tc.tile_pool(name="yp", bufs=2))
            pt_pool = attn_ctx.enter_context(tc.tile_pool(name="pt", bufs=5))
            rowp = attn_ctx.enter_context(tc.tile_pool(name="rows", bufs=2))
            arp = attn_ctx.enter_context(tc.tile_pool(name="arp", bufs=2))

            w_fc_sb = wfc_pool.tile([P, KO, fh], BF16)

            def stage_A_pre(qc):
                """LN1 + transposed activations for chunk qc (DVE/Scalar
                heavy; emitted a chunk ahead so the PE never waits on it)."""
                h1T = h1p.tile([P, KO, QCH], BF16, tag="h1T", name="h1T")
                for tl in range(KPQ):
                    tt = qc * KPQ + tl
                    xt = workb.tile([P, c_dim], F32, tag="x_in", name="xt")
                    nc.sync.dma_start(xt[:], x_tm[tt * P:(tt + 1) * P, :])
                    h_bf = ln_tile(xt, ln1_g_bc, ln1_b_bc)
                    transpose_into(h1T, h_bf, tl, KO)
                return h1T

            def stage_A(qc, h1T, interject=None):
                # qkT chunk (transposed-output form)
                for mt in range(MQK):
                    ps = ps_mm.tile([P, QCH], F32, tag="ps", name="ps")
                    for ko in range(KO):
                        nc.tensor.matmul(
                            ps[:],
                            w_qk_sb[:, ko, mt * P:(mt + 1) * P],
                            h1T[:, ko, :],
                            start=(ko == 0),
                            stop=(ko == KO - 1),
                        )
                    dst = qkT[:, mt, qc * QCH:(qc + 1) * QCH]
                    if b_qk_col is not None:
                        nc.vector.tensor_scalar_add(dst, ps[:], b_qk_col[:, mt:mt + 1])
                    else:
                        nc.vector.tensor_copy(dst, ps[:])

                # v token-major for this chunk's tiles
                for tl in range(KPQ):
                    tt = qc * KPQ + tl
                    ps = ps_mm.tile([P, QCH], F32, tag="ps", name="ps")
                    for ko in range(KO):
                        nc.tensor.matmul(
                            ps[:, 0:h_core * HD],
                            h1T[:, ko, tl * P:(tl + 1) * P],
                            w_v_sb[:, ko, :],
                            start=(ko == 0),
                            stop=(ko == KO - 1),
                        )
                    if b_v_bc is not None:
                        nc.vector.tensor_add(
                            ps[:, 0:h_core * HD], ps[:, 0:h_core * HD], b_v_bc[:]
                        )
                    for h in range(h_core):
                        nc.vector.tensor_copy(
                            vaug[:, tt, h, 0:64], ps[:, h * HD:(h + 1) * HD]
                        )

                # the previous chunk's out-projection slots in here: the
                # qk/v matmuls above give the PE independent work while the
                # previous softmax-normalize chain drains
                if interject is not None:
                    interject()

                # causal attention, one head-pair at a time
                yT = yp.tile([P, DKO, QCH], BF16, tag="yT", name="yT")
                for pr in range(NPAIR):
                    heads = ((0, 2 * pr), (64, 2 * pr + 1))
                    psy = {}
                    for sub, h in heads:
                        psy[h] = ps_y.tile([P, QCH], F32, tag="psy", name=f"psy_{h}")
                    nkt = (qc + 1) * KPQ
                    pts = {}

                    def emit_s_exp(kt, heads=heads, pts=pts, qc=qc, pr=pr):
                        i = kt - qc * KPQ  # >=0 on the diagonal band
                        for sub, h in heads:
                            pss = ps_mm.tile([P, QCH], F32, tag="ps", name=f"pss_{h}")
                            nc.tensor.matmul(
                                pss[:],
                                qkT[sub:sub + 64, DKO + pr, kt * P:(kt + 1) * P],
                                qkT[sub:sub + 64, pr, qc * QCH:(qc + 1) * QCH],
                                start=True,
                                stop=True,
                            )
                            pt = pt_pool.tile([P, QCH], BF16, tag="pt", name=f"pt_{h}")
                            if i >= 0:
                                if i > 0:
                                    nc.vector.memset(pt[:, 0:i * P], 0.0)
                                nc.vector.tensor_add(
                                    pss[:, i * P:(i + 1) * P],
                                    pss[:, i * P:(i + 1) * P],
                                    tri_sb[:],
                                )
                                nc.scalar.activation(
                                    pt[:, i * P:QCH],
                                    pss[:, i * P:QCH],
                                    mybir.ActivationFunctionType.Exp,
                                )
                            else:
                                nc.scalar.activation(
                                    pt[:], pss[:], mybir.ActivationFunctionType.Exp
                                )
                            pts[(kt, h)] = pt

                    def emit_pv(kt, heads=heads, psy=psy, pts=pts, nkt=nkt):
                        for sub, h in heads:
                            nc.tensor.matmul(
                                psy[h][0:65, :],
                                vaug[:, kt, h, :],
                                pts.pop((kt, h))[:],
                                start=(kt == 0),
                                stop=(kt == nkt - 1),
                            )

                    for kt in range(nkt):
                        emit_s_exp(kt)
                        if kt > 0:
                            emit_pv(kt - 1)
                    emit_pv(nkt - 1)
                    for sub, h in heads:
                        # evacuate the whole psy bank once via DVE (frees the
                        # PSUM bank for the next pair immediately, and keeps
                        # the Scalar engine free for the softmax exps)
                        psy_sb = rowp.tile([65, QCH], F32, tag="psy_sb", name="psy_sb")
                        nc.vector.tensor_copy(psy_sb[:], psy[h][0:65, :])
                        # reciprocal of the denominator row via exp(-ln(x)):
                        # two cheap Scalar ops on the activation table already
                        # loaded for the softmax (DVE reciprocal is ~6ns/elem
                        # per partition -- 3.2us for a 512-wide row)
                        row = rowp.tile([1, QCH], F32, tag="rec", name="row", bufs=2)
                        nc.scalar.activation(
                            row[:], psy_sb[64:65, :],
                            mybir.ActivationFunctionType.Ln,
                        )
                        nc.scalar.activation(
                            row[:], row[:],
                            mybir.ActivationFunctionType.Exp, scale=-1.0,
                        )
                        # broadcast to 64 partitions via a DRAM bounce: keeps
                        # the PE stream free of dependent instructions (a PE
                        # stall resets the clock p-state, costing ~3x the
                        # stall itself)
                        bc_sb = rowp.tile([64, QCH], F32, tag="bc_sb", name="bc_sb")
                        rd = recip_d[qc * h_core + h:qc * h_core + h + 1, :]
                        nc.sync.dma_start(rd, row[:])
                        nc.sync.dma_start(bc_sb[:], rd.to_broadcast((64, QCH)))
                        nc.vector.tensor_tensor(
                            yT[sub:sub + 64, pr, :],
                            psy_sb[0:64, :],
                            bc_sb[:],
                            mybir.AluOpType.mult,
                        )
                return yT

            def stage_A_proj(qc, yT):
                """Out-projection partials + ReduceScatter for chunk qc.
                Emitted AFTER the next chunk's qk/v projections, so the PE
                has independent work while the softmax normalize finishes."""
                ar_st = arp.tile([P, KPQ, c_dim], ARDT, tag="ar_st", name="ar_st")
                for tl in range(KPQ):
                    for nch in range(NCC):
                        ps = ps_mm.tile([P, QCH], F32, tag="ps", name="ps")
                        for dk in range(DKO):
                            nc.tensor.matmul(
                                ps[:],
                                yT[:, dk, tl * P:(tl + 1) * P],
                                w_ap_sb[:, dk, nch * QCH:(nch + 1) * QCH],
                                start=(dk == 0),
                                stop=(dk == DKO - 1),
                            )
                        dst = ar_st[:, tl, nch * QCH:(nch + 1) * QCH]
                        if b_ap_bc is not None:
                            nc.vector.tensor_add(
                                dst, ps[:], b_ap_bc[:, nch * QCH:(nch + 1) * QCH]
                            )
                        else:
                            nc.vector.tensor_copy(dst, ps[:])
                nc.sync.dma_start(
                    rs_in[qc][:].rearrange("(a p) c -> p a c", p=P), ar_st[:]
                )
                if local_reduce:
                    nc.sync.dma_start(rs_out[qc][:], rs_in[qc][0:P, :])
                else:
                    nc.gpsimd.collective_compute(
                        "ReduceScatter",
                        mybir.AluOpType.add,
                        replica_groups=replica_groups,
                        ins=[rs_in[qc][:]],
                        outs=[rs_out[qc][:]],
                    )

            def x1_tile(qc):
                """x1 = x_own + attn for owned tile of chunk qc, fp32.
                The rs_out read is a DEPENDENT dma on the in-order sync
                queue: only emit once nothing latency-critical queues
                behind it."""
                xt = workb.tile([P, c_dim], F32, tag="x1f", name="xt",
                                bufs=4)
                nc.sync.dma_start(xt[:], x_own[qc * P:(qc + 1) * P, :])
                at = workb.tile([P, c_dim], ARDT, tag="ar_rd", name="at")
                nc.sync.dma_start(at[:], rs_out[qc][:])
                nc.vector.tensor_add(xt[:], xt[:], at[:])
                return xt

            def stage_F_pre(qc):
                """x1 + LN2 + transposed h2 for the owned tile of chunk qc.
                Returns (h2T, x1); x1 is kept for the final residual."""
                x1 = x1_tile(qc)
                h_bf = ln_tile(x1, ln2_g_bc, ln2_b_bc)
                h2T = h2p.tile([P, KO, P], BF16, tag="h2T", name="h2T")
                transpose_into(h2T, h_bf, 0, KO)
                return h2T, x1

            h1s = {}
            h2s = {}
            x1s = {}
            yts = {}
            h1s[0] = stage_A_pre(0)
            h1s[1] = stage_A_pre(1)
            yts[0] = stage_A(0, h1s.pop(0))
            h1s[2] = stage_A_pre(2)
            yts[1] = stage_A(
                1, h1s.pop(1),
                interject=lambda: stage_A_proj(0, yts.pop(0)),
            )
            # preload the full fc weight mid-attention: late enough not to
            # contend for HBM with the startup x/weight loads, early enough
            # to land long before the fc stages need it.
            nc.scalar.dma_start(w_fc_sb[:], w_fcT[:])
            h1s[3] = stage_A_pre(3)
            yts[2] = stage_A(
                2, h1s.pop(2),
                interject=lambda: stage_A_proj(1, yts.pop(1)),
            )
            yts[3] = stage_A(
                3, h1s.pop(3),
                interject=lambda: stage_A_proj(2, yts.pop(2)),
            )
            stage_A_proj(3, yts.pop(3))

        # ======================== MLP phase ================================
        # token-parallel over the 4 owned tiles; full weights, no collective
        wmp_pool = ctx.enter_context(tc.tile_pool(name="wmp", bufs=1))
        w_mp_sb = wmp_pool.tile([P, FKO, c_dim], BF16)
        nc.scalar.dma_start(w_mp_sb[:], w_mpT[:])

        gsp = ctx.enter_context(tc.tile_pool(name="gsp", bufs=2))
        gtp = ctx.enter_context(tc.tile_pool(name="gtp", bufs=2))

        def stage_F(qc, h2T):
            """fc + gelu + transpose for the owned tile of chunk qc."""
            g_s = gsp.tile([P, NHC, QCH], BF16, tag="g_s", name="g_s")
            # two PSUM half-rounds of 4 h-chunks: stationary h2T[ko] is
            # amortized over 4 moving-512 matmuls per load
            for half in range(2):
                pss = [
                    ps_mm.tile([P, QCH], F32, tag="ps", name=f"psf_{i}")
                    for i in range(4)
                ]
                for ko in range(KO):
                    for i in range(4):
                        hc = half * 4 + i
                        nc.tensor.matmul(
                            pss[i][:],
                            h2T[:, ko, :],
                            w_fc_sb[:, ko, hc * QCH:(hc + 1) * QCH],
                            start=(ko == 0),
                            stop=(ko == KO - 1),
                        )
                for i in range(4):
                    hc = half * 4 + i
                    if b_fc_bc is not None:
                        nc.vector.tensor_add(
                            pss[i][:], pss[i][:],
                            b_fc_bc[:, hc * QCH:(hc + 1) * QCH],
                        )
                    nc.scalar.activation(
                        g_s[:, hc, :], pss[i][:],
                        mybir.ActivationFunctionType.Gelu_apprx_tanh,
                    )

            gT = gtp.tile([P, FKO, P], BF16, tag="gT", name="gT")
            for hc in range(NHC):
                transpose_into(gT[:, hc * 4:(hc + 1) * 4, :], g_s[:, hc, :], 0, 4)
            return gT

        def stage_M(qc, gT, x1):
            """Down-projection + final residual + store for chunk qc."""
            for nch in range(NCC):
                ps = ps_mm.tile([P, QCH], F32, tag="ps", name="ps")
                for hk in range(FKO):
                    nc.tensor.matmul(
                        ps[:],
                        gT[:, hk, :],
                        w_mp_sb[:, hk, nch * QCH:(nch + 1) * QCH],
                        start=(hk == 0),
                        stop=(hk == FKO - 1),
                    )
                if b_mp_bc is not None:
                    nc.vector.tensor_add(
                        ps[:], ps[:], b_mp_bc[:, nch * QCH:(nch + 1) * QCH]
                    )
                ev = works.tile([P, QCH], F32, tag="evac", name="ev", bufs=2)
                nc.vector.tensor_tensor(
                    ev[:], ps[:], x1[:, nch * QCH:(nch + 1) * QCH],
                    mybir.AluOpType.add,
                )
                nc.sync.dma_start(
                    out_y[qc * P:(qc + 1) * P, nch * QCH:(nch + 1) * QCH], ev[:]
                )

        gts = {}
        h2s[0], x1s[0] = stage_F_pre(0)
        h2s[1], x1s[1] = stage_F_pre(1)
        gts[0] = stage_F(0, h2s.pop(0))
        h2s[2], x1s[2] = stage_F_pre(2)
        gts[1] = stage_F(1, h2s.pop(1))
        stage_M(0, gts.pop(0), x1s.pop(0))
        h2s[3], x1s[3] = stage_F_pre(3)
        gts[2] = stage_F(2, h2s.pop(2))
        stage_M(1, gts.pop(1), x1s.pop(1))
        gts[3] = stage_F(3, h2s.pop(3))
        stage_M(2, gts.pop(2), x1s.pop(2))
        stage_M(3, gts.pop(3), x1s.pop(3))

    if legalize:
        _legalize_waits(nc)
    return nc


# ---------------------------------------------------------------------------
# host-side sharding / layout prep


def _tile_k(arr, width):
    """[K, M] -> [128, K//128, M] (contraction dim inner on partitions)."""
    k, m = arr.shape
    assert m == width and k % P == 0
    return np.ascontiguousarray(
        arr.reshape(k // P, P, m).transpose(1, 0, 2)
    )


def _bf(arr):
    return arr.astype(ml_dtypes.bfloat16)


def make_core_inputs(inputs, t_len=T, c_dim=C, h_core=H_CORE, fh=FH,
                     n_groups=len(GROUPS), tpg=TPG):
    """Shard + lay out the full inputs into per-core input dicts and the
    active-flag set."""
    f32 = np.float32
    x = np.asarray(inputs["x"], f32)
    W_attn = np.asarray(inputs["W_attn"], f32)
    W_aproj = np.asarray(inputs["W_aproj"], f32)
    W_fc = np.asarray(inputs["W_fc"], f32)
    W_mproj = np.asarray(inputs["W_mproj"], f32)
    ln1_g = np.asarray(inputs["ln1_g"], f32)
    ln1_b = np.asarray(inputs["ln1_b"], f32)
    ln2_g = np.asarray(inputs["ln2_g"], f32)
    ln2_b = np.asarray(inputs["ln2_b"], f32)
    b_attn = np.asarray(inputs["b_attn"], f32)
    b_aproj = np.asarray(inputs["b_aproj"], f32)
    b_fc = np.asarray(inputs["b_fc"], f32)
    b_mproj = np.asarray(inputs["b_mproj"], f32)

    Wq, Wk, Wv = W_attn[:c_dim], W_attn[c_dim:2 * c_dim], W_attn[2 * c_dim:]
    bq, bk, bv = b_attn[:c_dim], b_attn[c_dim:2 * c_dim], b_attn[2 * c_dim:]
    scale = 1.0 / math.sqrt(HD)

    flags = set()
    if not np.all(ln1_g == 1.0):
        flags.add("ln1_g")
    if np.any(ln1_b):
        flags.add("ln1_b")
    if not np.all(ln2_g == 1.0):
        flags.add("ln2_g")
    if np.any(ln2_b):
        flags.add("ln2_b")
    if np.any(b_attn[:2 * c_dim]):
        flags.add("b_qk")
    if np.any(bv):
        flags.add("b_v")
    if np.any(b_aproj):
        flags.add("b_ap")
    if np.any(b_fc):
        flags.add("b_fc")
    if np.any(b_mproj):
        flags.add("b_mp")

    tri = np.where(
        np.arange(P)[:, None] > np.arange(P)[None, :], f32(-1e30), f32(0.0)
    ).astype(f32)

    # replicated full MLP weights, transposed layouts (contraction inner)
    w_fcT_full = _tile_k(_bf(np.ascontiguousarray(W_fc.T)), fh)
    w_mpT_full = _tile_k(_bf(np.ascontiguousarray(W_mproj.T)), c_dim)

    NQC = t_len // QCH

    in_maps = []
    for core in range(n_groups * tpg):
        g, s = core // tpg, core % tpg
        heads = range(s * h_core, (s + 1) * h_core)
        # stacked [q heads | k heads] output dims, q pre-scaled by 1/sqrt(hd)
        w_qk_rows = np.concatenate(
            [Wq[h * HD:(h + 1) * HD] * scale for h in heads]
            + [Wk[h * HD:(h + 1) * HD] for h in heads], axis=0
        )  # [QK, C]
        w_v_rows = np.concatenate(
            [Wv[h * HD:(h + 1) * HD] for h in heads], axis=0
        )  # [DH, C]
        dsl = slice(s * h_core * HD, (s + 1) * h_core * HD)
        xg = x[g % x.shape[0]]
        x_own = np.concatenate(
            [xg[(qc * tpg + s) * P:(qc * tpg + s + 1) * P] for qc in range(NQC)],
            axis=0,
        )
        m = {
            "x_tm": np.ascontiguousarray(xg),
            "x_own": np.ascontiguousarray(x_own),
            "w_qk": _tile_k(_bf(w_qk_rows.T), h_core * P),
            "w_v": _tile_k(_bf(w_v_rows.T), h_core * HD),
            "w_ap": _tile_k(_bf(W_aproj[:, dsl].T.copy()), c_dim),
            "w_fcT": w_fcT_full,
            "w_mpT": w_mpT_full,
            "tri": tri,
        }
        if "ln1_g" in flags:
            m["ln1_g"] = ln1_g.reshape(1, -1).copy()
        if "ln1_b" in flags:
            m["ln1_b"] = ln1_b.reshape(1, -1).copy()
        if "ln2_g" in flags:
            m["ln2_g"] = ln2_g.reshape(1, -1).copy()
        if "ln2_b" in flags:
            m["ln2_b"] = ln2_b.reshape(1, -1).copy()
        if "b_qk" in flags:
            b_qk_rows = np.concatenate(
                [bq[h * HD:(h + 1) * HD] * scale for h in heads]
                + [bk[h * HD:(h + 1) * HD] for h in heads]
            )  # [QK] along partitions: [P, MQK]
            m["b_qk"] = np.ascontiguousarray(
                b_qk_rows.reshape(h_core, P).T
            )
        if "b_v" in flags:
            m["b_v"] = np.concatenate(
                [bv[h * HD:(h + 1) * HD] for h in heads]
            ).reshape(1, -1).copy()
        if "b_ap" in flags:
            m["b_ap"] = (b_aproj / tpg).reshape(1, -1).copy()
        if "b_fc" in flags:
            m["b_fc"] = b_fc.reshape(1, -1).copy()
        if "b_mp" in flags:
            m["b_mp"] = b_mproj.reshape(1, -1).copy()
        in_maps.append(m)
    return in_maps, frozenset(flags)


# ---------------------------------------------------------------------------
# runner

_module_cache = {}


def run(inputs, trace=False, trace_kwargs=None, tmpdir=None):
    in_maps, flags = make_core_inputs(inputs)
    key = (flags, trace)
    if key not in _module_cache:
        _module_cache[key] = build_module(flags=flags)
    nc = _module_cache[key]
    if trace:
        _install_prof_hook()
    res = run_bass_kernel_spmd(
        nc,
        in_maps,
        core_ids=list(range(N_CORES)),
        trace=trace,
        tmpdir=tmpdir,
        **(trace_kwargs or {}),
    )
    # reassemble: core g*TPG+s provides token tiles (qc*TPG + s) of batch g
    NQC = T // QCH
    out = np.empty((B, T, C), np.float32)
    for g in range(len(GROUPS)):
        for s in range(TPG):
            o = res.results[g * TPG + s]["out"]
            for qc in range(NQC):
                tt = qc * TPG + s
                out[g, tt * P:(tt + 1) * P, :] = o[qc * P:(qc + 1) * P, :]
    return out, res


def kernel(**inputs) -> np.ndarray:
    out, _ = run(inputs, trace=False)
    return out


# revision 42
# speedup vs baseline: 1.2285x; 1.0669x over previous
"""Fused causal-transformer block (LN1 -> attn -> LN2 -> MLP, residuals) on
8 Trainium2 NeuronCores.

Sharding: 2 groups of 4 cores; group g handles batch element b=g (data
parallel).  Within a group:
  - Attention is Megatron head-parallel: core s owns 4 heads, computes
    partial y = attn(x) @ W_aproj_s for ALL tokens, chunked over four
    512-token chunks.  Each chunk's partials are summed with an in-group
    ReduceScatter, leaving core s with the summed attention output for
    token tile (chunk*4 + s) -- its 128-token slice of each chunk.
  - The MLP is token-parallel with REPLICATED weights: core s runs the
    full 4C-hidden MLP for its 4 owned token tiles (512 tokens total).
    No second collective is needed; the host reassembles token slices.
This cuts the collective count from 8 AllReduces to 4 ReduceScatters,
all hidden behind attention compute (the single CC core was the
bottleneck of the AllReduce design).

Compute dtype: bf16 matmul inputs, fp32 PSUM accumulation, fp32 residual
stream and softmax statistics.

Layouts (per core, all prepared host-side in kernel()):
  h1T/h2T  : [128, C/128, t]  activations transposed (contraction dim on
             partitions) produced on-device via PE transposes.
  qkT      : [128, H_core, T] rows = [q heads | k heads] * 64-dim each,
             two heads stacked per 128-partition tile.  Scores are
             computed directly in S^T [k, q] layout, so softmax
             normalization arrives as a PSUM row via a ones-column in v.
  v_aug    : [128, T/128, H_core, 65]  v token-major per head + ones col.
  w_fcT    : [128, C/128, 4C]  full W_fc^T (replicated), moving operand.
  w_mpT    : [128, 4C/128, C]  full W_mproj^T (replicated), moving.
"""

import contextlib
import ctypes
import math
import sys
import types

import numpy as np
import ml_dtypes

import bass_rust
import concourse.bass as bass
import concourse.mybir as mybir
import concourse.tile as tile
from concourse import library_config
from concourse.bass_utils import run_bass_kernel_spmd
from concourse.masks import make_identity
from concourse.tile import TileContext
from concourse.vector_clock import ScopedClock

# ---------------------------------------------------------------------------
# problem constants (hardcoded per the harness contract)
B, T, C, H = 2, 2048, 1024, 16
HD = C // H                 # 64
N_CORES = 8
TPG = 4                     # tensor-parallel group size
H_CORE = H // TPG           # heads per core = 4
DH = H_CORE * HD            # per-core attention dim = 256
FH = 4 * C                  # full MLP hidden (replicated) = 4096
P = 128
EPS = 1e-5
QCH = 512                   # q-chunk width
GROUPS = [[0, 1, 2, 3], [4, 5, 6, 7]]

F32 = mybir.dt.float32
BF16 = mybir.dt.bfloat16

# ---------------------------------------------------------------------------
# workaround 1: the container's walrus accepts a single sync-wait command per
# instruction; move extra semaphore waits onto inserted EventSemaphore
# instructions on the same engine (program order preserves semantics).

_waitfix_counter = [0]


def _legalize_waits(nc, cap=1):
    fn = nc.m.functions[0]
    n_split = 0
    for bb in fn.blocks:
        out = []
        changed = False
        for inst in bb.instructions:
            si = inst.sync_info
            waits = list(si.on_wait) if si is not None else []
            if len(waits) > cap:
                movable = [w for w in waits if w.sync_type == "semaphore"]
                fixed = [w for w in waits if w.sync_type != "semaphore"]
                n_keep = max(cap - len(fixed), 0)
                keep = fixed + (movable[len(movable) - n_keep:] if n_keep else [])
                extra = movable[: len(movable) - n_keep] if n_keep else movable
                for w in extra:
                    _waitfix_counter[0] += 1
                    ev = mybir.InstEventSemaphore(
                        name=f"I-waitfix-{_waitfix_counter[0]}",
                        engine=inst.engine,
                        ins=[],
                        outs=[],
                        sync_info=bass_rust.SyncInfo(on_wait=[w], on_update=[]),
                    )
                    out.append(ev)
                    n_split += 1
                inst.sync_info = bass_rust.SyncInfo(
                    on_wait=keep, on_update=list(si.on_update)
                )
                changed = True
            out.append(inst)
        if changed:
            bb.instructions = out
    return n_split


# workaround 2: same issue for the Tile kernel-tail Drain — emit one wait-nop
# per live proc ahead of a wait-less drain instead of stacking waits on it.


def _drain_and_barrier_split(self, tick_clock, wait_clock):
    gc = tick_clock.global_clock
    sems_alloc = wait_clock.sems.allocated()
    for proc in sorted(sems_alloc):
        tick = gc.peek_next(proc) - 1
        if tick <= 0:
            continue
        vc1 = bass_rust.VectorClock()
        vc1.require_at_least(proc, tick)
        nop = self.nc.sync.nop()
        wait_clock.add_sem_waits(nop.ins, ScopedClock({None: vc1}))
    self.nc.sync.drain()
    self.nc.all_engine_barrier()
    assert self.sems is not None
    popped = self.nc._tile_sem_poison_stack.pop()
    assert popped is self._sem_poison
    self.nc.clear_and_free_semaphores(list(self.sems.allocated().values()))
    self.nc.all_engine_barrier()


TileContext._drain_and_barrier = _drain_and_barrier_split


# workaround 3 (profiling only): register the NTFF hook the trimmed antenv
# lacks so run_bass_kernel_spmd(trace=True) works under axon.


def _install_prof_hook():
    if "antenv.axon_hooks" in sys.modules:
        return
    so_path = "/opt/axon/libaxon_pjrt.so"
    hook = None
    try:
        lib = ctypes.CDLL(so_path)
        if hasattr(lib, "axon_start_nrt_profile"):
            lib.axon_start_nrt_profile.argtypes = [
                ctypes.POINTER(ctypes.c_int64),
                ctypes.c_size_t,
            ]
            lib.axon_start_nrt_profile.restype = ctypes.c_int64
            lib.axon_stop_nrt_profile.argtypes = [ctypes.c_char_p]
            lib.axon_stop_nrt_profile.restype = ctypes.c_int64

            @contextlib.contextmanager
            def _hook_cm(output_dir, device_ids):
                import jax

                jax.devices()
                if device_ids:
                    ids = (ctypes.c_int64 * len(device_ids))(*device_ids)
                    rc = lib.axon_start_nrt_profile(ids, len(device_ids))
                else:
                    rc = lib.axon_start_nrt_profile(None, 0)
                if rc != 0:
                    raise RuntimeError(f"axon_start_nrt_profile rc={rc}")
                try:
                    yield
                finally:
                    n = lib.axon_stop_nrt_profile(str(output_dir).encode())
                    if n < 0:
                        raise RuntimeError(f"axon_stop_nrt_profile rc={n}")

            hook = _hook_cm
    except OSError:
        pass
    mod = types.ModuleType("antenv.axon_hooks")
    mod.get_axon_ntff_profile_hook = lambda: hook
    mod.set_axon_ntff_profile_hook = lambda h: None
    sys.modules["antenv.axon_hooks"] = mod
    from concourse import bass_utils

    bass_utils.upload_artifacts = lambda tmpdir: tmpdir


# ---------------------------------------------------------------------------
# device kernel builder


def build_module(
    t_len=T,
    c_dim=C,
    h_core=H_CORE,
    fh=FH,
    flags=frozenset(),
    replica_groups=GROUPS,
    local_reduce=False,
    legalize=True,
):
    """Build the per-core SPMD Bass module.

    flags: subset of {"ln1_g","ln1_b","ln2_g","ln2_b","b_qk","b_v","b_ap",
    "b_fc","b_mp"} enabling the non-trivial affine/bias paths.
    local_reduce: replace the in-group ReduceScatter with a local strided
    copy (single core test mode: takes this rank-0 slice).
    """
    KO = c_dim // P             # c-tiles
    NT = t_len // P             # token tiles
    NQC = t_len // QCH          # q chunks
    KPQ = QCH // P              # token tiles per chunk (= group size 4)
    QK = h_core * P             # stacked q+k dims
    MQK = h_core                # m-tiles of qkT
    DKO = (h_core * HD) // P    # d-tiles of y/aproj  (h_core/2)
    FKO = fh // P               # hidden tiles (32)
    NPAIR = h_core // 2
    NCC = c_dim // QCH          # 512-chunks of C
    NHC = fh // QCH             # 512-chunks of hidden (8)
    assert h_core % 2 == 0 and c_dim % P == 0 and t_len % QCH == 0

    nc = bass.Bass(num_devices=N_CORES)

    x_tm = nc.dram_tensor("x_tm", (t_len, c_dim), BF16, kind="ExternalInput")
    x_own = nc.dram_tensor("x_own", (NQC * P, c_dim), BF16, kind="ExternalInput")
    w_qk = nc.dram_tensor("w_qk", (P, KO, QK), BF16, kind="ExternalInput")
    w_v = nc.dram_tensor("w_v", (P, KO, h_core * HD), BF16, kind="ExternalInput")
    w_ap = nc.dram_tensor("w_ap", (P, DKO, c_dim), BF16, kind="ExternalInput")
    w_fcT = nc.dram_tensor("w_fcT", (P, KO, fh), BF16, kind="ExternalInput")
    w_mpT = nc.dram_tensor("w_mpT", (P, FKO, c_dim), BF16, kind="ExternalInput")
    tri = nc.dram_tensor("tri", (P, P), F32, kind="ExternalInput")
    opt_in = {}
    for name, shape in [
        ("ln1_g", (1, c_dim)), ("ln1_b", (1, c_dim)),
        ("ln2_g", (1, c_dim)), ("ln2_b", (1, c_dim)),
        ("b_qk", (P, MQK)), ("b_v", (1, h_core * HD)), ("b_ap", (1, c_dim)),
        ("b_fc", (1, fh)), ("b_mp", (1, c_dim)),
    ]:
        if name in flags:
            opt_in[name] = nc.dram_tensor(name, shape, F32, kind="ExternalInput")

    # per-core output: its 4 owned token tiles, row qc*128+p = token
    # tile (qc*4 + rank), host reassembles.
    out_y = nc.dram_tensor("out", (NQC * P, c_dim), F32, kind="ExternalOutput")

    # collective payloads travel in bf16: halves the wire time; the partial
    # projections are O(1)-magnitude so the rounding is ~1e-3 relative.
    ARDT = BF16
    rs_in = [nc.dram_tensor(f"rs_in{i}", (QCH, c_dim), ARDT) for i in range(NQC)]
    rs_out = [nc.dram_tensor(f"rs_out{i}", (P, c_dim), ARDT) for i in range(NQC)]
    # DRAM bounce rows for the softmax-denominator partition broadcast
    recip_d = nc.dram_tensor("recip_d", (NQC * h_core, QCH), F32)

    with TileContext(nc) as tc, contextlib.ExitStack() as ctx:
        const = ctx.enter_context(tc.tile_pool(name="const", bufs=1))
        workb = ctx.enter_context(tc.tile_pool(name="workb", bufs=2))
        works = ctx.enter_context(tc.tile_pool(name="works", bufs=3))
        stats = ctx.enter_context(tc.tile_pool(name="stats", bufs=6))

        ident = const.tile([P, P], BF16)
        make_identity(nc, ident)
        ones64 = const.tile([1, 64], F32)
        nc.vector.memset(ones64[:], 1.0)
        eps_t = const.tile([P, 1], F32)
        nc.vector.memset(eps_t[:], EPS)
        tri_sb = const.tile([P, P], F32)
        nc.sync.dma_start(tri_sb[:], tri[:])

        # optional affine operands, broadcast to 128 partitions once
        def _bcast_row(name, width):
            if name not in opt_in:
                return None
            bc = const.tile([P, width], F32, name=f"bc_{name}", tag=f"bc_{name}")
            nc.sync.dma_start(bc[:], opt_in[name][:].to_broadcast((P, width)))
            return bc

        def _col(name):
            if name not in opt_in:
                return None
            t_ = const.tile(list(opt_in[name].shape), F32, name=f"col_{name}", tag=f"col_{name}")
            nc.sync.dma_start(t_[:], opt_in[name][:])
            return t_

        ln1_g_bc = _bcast_row("ln1_g", c_dim)
        ln1_b_bc = _bcast_row("ln1_b", c_dim)
        ln2_g_bc = _bcast_row("ln2_g", c_dim)
        ln2_b_bc = _bcast_row("ln2_b", c_dim)
        b_v_bc = _bcast_row("b_v", h_core * HD)
        b_ap_bc = _bcast_row("b_ap", c_dim)
        b_fc_bc = _bcast_row("b_fc", fh)
        b_mp_bc = _bcast_row("b_mp", c_dim)
        b_qk_col = _col("b_qk")

        ps_tr = ctx.enter_context(tc.tile_pool(name="ps_tr", bufs=2, space="PSUM"))
        ps_mm = ctx.enter_context(tc.tile_pool(name="ps_mm", bufs=5, space="PSUM"))
        ps_y = ctx.enter_context(tc.tile_pool(name="ps_y", bufs=2, space="PSUM"))

        def ln_tile(x_f32, g_bc, b_bc):
            """LayerNorm of a [P, c_dim] fp32 AP -> new [P, c_dim] bf16 tile.
            rsqrt via Ln+Exp: shares the Scalar activation table with the
            softmax Exp, so no table reloads between LN and attention."""
            nsub = c_dim // 512
            st = stats.tile([P, nsub, 6], F32)
            for j in range(nsub):
                nc.vector.bn_stats(st[:, j, :], x_f32[:, j * 512:(j + 1) * 512])
            mv = stats.tile([P, 2], F32)
            nc.vector.bn_aggr(mv[:], st[:])
            r = stats.tile([P, 1], F32)
            nc.scalar.activation(
                r[:], mv[:, 1:2], mybir.ActivationFunctionType.Ln, bias=eps_t[:]
            )
            nc.scalar.activation(
                r[:], r[:], mybir.ActivationFunctionType.Exp, scale=-0.5
            )
            if g_bc is None and b_bc is None:
                h_bf = works.tile([P, c_dim], BF16, tag="ln_out", name="h_bf")
                nc.vector.tensor_scalar(
                    out=h_bf[:], in0=x_f32[:], scalar1=mv[:, 0:1], scalar2=r[:],
                    op0=mybir.AluOpType.subtract, op1=mybir.AluOpType.mult,
                )
            else:
                h_f = workb.tile([P, c_dim], F32, tag="ln_f32", name="h_f")
                nc.vector.tensor_scalar(
                    out=h_f[:], in0=x_f32[:], scalar1=mv[:, 0:1], scalar2=r[:],
                    op0=mybir.AluOpType.subtract, op1=mybir.AluOpType.mult,
                )
                if g_bc is not None:
                    nc.vector.tensor_mul(h_f[:], h_f[:], g_bc[:])
                if b_bc is not None:
                    nc.vector.tensor_add(h_f[:], h_f[:], b_bc[:])
                h_bf = works.tile([P, c_dim], BF16, tag="ln_out", name="h_bf")
                nc.vector.tensor_copy(h_bf[:], h_f[:])
            return h_bf

        def transpose_into(dstT, src_bf, tl, n_k):
            """PE-transpose [P, n_k*128] bf16 into dstT[:, :, tl*P:(tl+1)*P].
            Four 128x128 transposes share one PSUM bank so a single DVE copy
            evacuates them."""
            for kg in range(0, n_k, 4):
                nb = min(4, n_k - kg)
                pst = ps_tr.tile([P, 4 * P], BF16, tag="pst", name="pst", bufs=1)
                for j in range(nb):
                    nc.tensor.transpose(
                        pst[:, j * P:(j + 1) * P],
                        src_bf[:, (kg + j) * P:(kg + j + 1) * P],
                        ident[:],
                    )
                nc.vector.tensor_copy(
                    dstT[:, kg:kg + nb, tl * P:(tl + 1) * P],
                    pst[:, 0:nb * P].rearrange("p (a b) -> p a b", a=nb),
                )

        # =============== the MLP weight pools (persistent) =================
        # w_fcT preloads during attention (SBUF has room); w_mpT loads into
        # the space the attention pools free, overlapping the fc stages.
        wfc_pool = ctx.enter_context(tc.tile_pool(name="wfc", bufs=1))
        h2p = ctx.enter_context(tc.tile_pool(name="h2p", bufs=4))

        # ======================= attention phase ===========================
        with contextlib.ExitStack() as attn_ctx:
            # weights go on the Activation engine's DMA queue so the big
            # transfers never head-of-line-block the latency-critical
            # activation loads on the sync queue.
            wa = attn_ctx.enter_context(tc.tile_pool(name="wa", bufs=1))
            w_qk_sb = wa.tile([P, KO, QK], BF16)
            nc.scalar.dma_start(w_qk_sb[:], w_qk[:])
            w_v_sb = wa.tile([P, KO, h_core * HD], BF16)
            nc.scalar.dma_start(w_v_sb[:], w_v[:])
            w_ap_sb = wa.tile([P, DKO, c_dim], BF16)
            nc.scalar.dma_start(w_ap_sb[:], w_ap[:])

            big = attn_ctx.enter_context(tc.tile_pool(name="big", bufs=1))
            qkT = big.tile([P, MQK, t_len], BF16)
            vaug = big.tile([P, NT, h_core, 65], BF16)
            nc.vector.memset(vaug[:, :, :, 64:65], 1.0)

            h1p = attn_ctx.enter_context(tc.tile_pool(name="h1p", bufs=2))
            yp = attn_ctx.enter_context(tc.tile_pool(name="yp", bufs=2))
            pt_pool = attn_ctx.enter_context(tc.tile_pool(name="pt", bufs=5))
            rowp = attn_ctx.enter_context(tc.tile_pool(name="rows", bufs=2))
            arp = attn_ctx.enter_context(tc.tile_pool(name="arp", bufs=2))

            w_fc_sb = wfc_pool.tile([P, KO, fh], BF16)

            def stage_A_pre(qc):
                """LN1 + transposed activations for chunk qc (DVE/Scalar
                heavy; emitted a chunk ahead so the PE never waits on it)."""
                h1T = h1p.tile([P, KO, QCH], BF16, tag="h1T", name="h1T")
                for tl in range(KPQ):
                    tt = qc * KPQ + tl
                    xt = workb.tile([P, c_dim], BF16, tag="x_in", name="xt")
                    nc.sync.dma_start(xt[:], x_tm[tt * P:(tt + 1) * P, :])
                    h_bf = ln_tile(xt, ln1_g_bc, ln1_b_bc)
                    transpose_into(h1T, h_bf, tl, KO)
                return h1T

            def stage_A(qc, h1T, interject=None):
                # qkT chunk (transposed-output form)
                for mt in range(MQK):
                    ps = ps_mm.tile([P, QCH], F32, tag="ps", name="ps")
                    for ko in range(KO):
                        nc.tensor.matmul(
                            ps[:],
                            w_qk_sb[:, ko, mt * P:(mt + 1) * P],
                            h1T[:, ko, :],
                            start=(ko == 0),
                            stop=(ko == KO - 1),
                        )
                    dst = qkT[:, mt, qc * QCH:(qc + 1) * QCH]
                    if b_qk_col is not None:
                        nc.vector.tensor_scalar_add(dst, ps[:], b_qk_col[:, mt:mt + 1])
                    else:
                        nc.vector.tensor_copy(dst, ps[:])

                # v token-major for this chunk's tiles
                for tl in range(KPQ):
                    tt = qc * KPQ + tl
                    ps = ps_mm.tile([P, QCH], F32, tag="ps", name="ps")
                    for ko in range(KO):
                        nc.tensor.matmul(
                            ps[:, 0:h_core * HD],
                            h1T[:, ko, tl * P:(tl + 1) * P],
                            w_v_sb[:, ko, :],
                            start=(ko == 0),
                            stop=(ko == KO - 1),
                        )
                    if b_v_bc is not None:
                        nc.vector.tensor_add(
                            ps[:, 0:h_core * HD], ps[:, 0:h_core * HD], b_v_bc[:]
                        )
                    for h in range(h_core):
                        nc.vector.tensor_copy(
                            vaug[:, tt, h, 0:64], ps[:, h * HD:(h + 1) * HD]
                        )

                # the previous chunk's out-projection slots in here: the
                # qk/v matmuls above give the PE independent work while the
                # previous softmax-normalize chain drains
                if interject is not None:
                    interject()

                # causal attention, one head-pair at a time
                yT = yp.tile([P, DKO, QCH], BF16, tag="yT", name="yT")
                for pr in range(NPAIR):
                    heads = ((0, 2 * pr), (64, 2 * pr + 1))
                    psy = {}
                    for sub, h in heads:
                        psy[h] = ps_y.tile([P, QCH], F32, tag="psy", name=f"psy_{h}")
                    nkt = (qc + 1) * KPQ
                    pts = {}

                    def emit_s_exp(kt, heads=heads, pts=pts, qc=qc, pr=pr):
                        i = kt - qc * KPQ  # >=0 on the diagonal band
                        for sub, h in heads:
                            pss = ps_mm.tile([P, QCH], F32, tag="ps", name=f"pss_{h}")
                            nc.tensor.matmul(
                                pss[:],
                                qkT[sub:sub + 64, DKO + pr, kt * P:(kt + 1) * P],
                                qkT[sub:sub + 64, pr, qc * QCH:(qc + 1) * QCH],
                                start=True,
                                stop=True,
                            )
                            pt = pt_pool.tile([P, QCH], BF16, tag="pt", name=f"pt_{h}")
                            if i >= 0:
                                if i > 0:
                                    nc.vector.memset(pt[:, 0:i * P], 0.0)
                                nc.vector.tensor_add(
                                    pss[:, i * P:(i + 1) * P],
                                    pss[:, i * P:(i + 1) * P],
                                    tri_sb[:],
                                )
                                nc.scalar.activation(
                                    pt[:, i * P:QCH],
                                    pss[:, i * P:QCH],
                                    mybir.ActivationFunctionType.Exp,
                                )
                            else:
                                nc.scalar.activation(
                                    pt[:], pss[:], mybir.ActivationFunctionType.Exp
                                )
                            pts[(kt, h)] = pt

                    def emit_pv(kt, heads=heads, psy=psy, pts=pts, nkt=nkt):
                        for sub, h in heads:
                            nc.tensor.matmul(
                                psy[h][0:65, :],
                                vaug[:, kt, h, :],
                                pts.pop((kt, h))[:],
                                start=(kt == 0),
                                stop=(kt == nkt - 1),
                            )

                    for kt in range(nkt):
                        emit_s_exp(kt)
                        if kt > 0:
                            emit_pv(kt - 1)
                    emit_pv(nkt - 1)
                    for sub, h in heads:
                        # evacuate the whole psy bank once via DVE (frees the
                        # PSUM bank for the next pair immediately, and keeps
                        # the Scalar engine free for the softmax exps)
                        psy_sb = rowp.tile([65, QCH], F32, tag="psy_sb", name="psy_sb")
                        nc.vector.tensor_copy(psy_sb[:], psy[h][0:65, :])
                        # reciprocal of the denominator row via exp(-ln(x)):
                        # two cheap Scalar ops on the activation table already
                        # loaded for the softmax (DVE reciprocal is ~6ns/elem
                        # per partition -- 3.2us for a 512-wide row)
                        row = rowp.tile([1, QCH], F32, tag="rec", name="row", bufs=2)
                        nc.scalar.activation(
                            row[:], psy_sb[64:65, :],
                            mybir.ActivationFunctionType.Ln,
                        )
                        nc.scalar.activation(
                            row[:], row[:],
                            mybir.ActivationFunctionType.Exp, scale=-1.0,
                        )
                        # broadcast to 64 partitions via a DRAM bounce: keeps
                        # the PE stream free of dependent instructions (a PE
                        # stall resets the clock p-state, costing ~3x the
                        # stall itself)
                        bc_sb = rowp.tile([64, QCH], F32, tag="bc_sb", name="bc_sb")
                        rd = recip_d[qc * h_core + h:qc * h_core + h + 1, :]
                        nc.sync.dma_start(rd, row[:])
                        nc.sync.dma_start(bc_sb[:], rd.to_broadcast((64, QCH)))
                        nc.vector.tensor_tensor(
                            yT[sub:sub + 64, pr, :],
                            psy_sb[0:64, :],
                            bc_sb[:],
                            mybir.AluOpType.mult,
                        )
                return yT

            def stage_A_proj(qc, yT):
                """Out-projection partials + ReduceScatter for chunk qc.
                Emitted AFTER the next chunk's qk/v projections, so the PE
                has independent work while the softmax normalize finishes."""
                ar_st = arp.tile([P, KPQ, c_dim], ARDT, tag="ar_st", name="ar_st")
                for tl in range(KPQ):
                    for nch in range(NCC):
                        ps = ps_mm.tile([P, QCH], F32, tag="ps", name="ps")
                        for dk in range(DKO):
                            nc.tensor.matmul(
                                ps[:],
                                yT[:, dk, tl * P:(tl + 1) * P],
                                w_ap_sb[:, dk, nch * QCH:(nch + 1) * QCH],
                                start=(dk == 0),
                                stop=(dk == DKO - 1),
                            )
                        dst = ar_st[:, tl, nch * QCH:(nch + 1) * QCH]
                        if b_ap_bc is not None:
                            nc.vector.tensor_add(
                                dst, ps[:], b_ap_bc[:, nch * QCH:(nch + 1) * QCH]
                            )
                        else:
                            nc.vector.tensor_copy(dst, ps[:])
                nc.sync.dma_start(
                    rs_in[qc][:].rearrange("(a p) c -> p a c", p=P), ar_st[:]
                )
                if local_reduce:
                    nc.sync.dma_start(rs_out[qc][:], rs_in[qc][0:P, :])
                else:
                    nc.gpsimd.collective_compute(
                        "ReduceScatter",
                        mybir.AluOpType.add,
                        replica_groups=replica_groups,
                        ins=[rs_in[qc][:]],
                        outs=[rs_out[qc][:]],
                    )
                return ar_st

            def x1_tile(qc, anchor=None):
                """x1 = x_own + attn for owned tile of chunk qc, fp32.
                The rs_out read is a DEPENDENT dma on the in-order sync
                queue; the anchor is a 1-element WAW dep that stops the
                scheduler from parking the queue on it before the
                ReduceScatter is actually near completion (the scheduler's
                collective cost model is ~4x optimistic)."""
                xt_b = workb.tile([P, c_dim], BF16, tag="xb", name="xt_b")
                at = workb.tile([P, c_dim], ARDT, tag="ar_rd", name="at")
                if anchor is not None:
                    nc.vector.tensor_copy(at[0:1, 0:1], anchor)
                nc.sync.dma_start(at[:], rs_out[qc][:])
                nc.sync.dma_start(xt_b[:], x_own[qc * P:(qc + 1) * P, :])
                x1 = workb.tile([P, c_dim], F32, tag="x1f", name="x1", bufs=4)
                nc.vector.tensor_tensor(x1[:], xt_b[:], at[:], mybir.AluOpType.add)
                return x1

            def stage_F_pre(qc, anchor=None):
                """x1 + LN2 + transposed h2 for the owned tile of chunk qc.
                Returns (h2T, x1); x1 is kept for the final residual."""
                x1 = x1_tile(qc, anchor)
                h_bf = ln_tile(x1, ln2_g_bc, ln2_b_bc)
                h2T = h2p.tile([P, KO, P], BF16, tag="h2T", name="h2T")
                transpose_into(h2T, h_bf, 0, KO)
                return h2T, x1

            h1s = {}
            h2s = {}
            x1s = {}
            yts = {}
            h1s[0] = stage_A_pre(0)
            h1s[1] = stage_A_pre(1)
            arsts = {}

            def _proj(qc):
                arsts[qc] = stage_A_proj(qc, yts.pop(qc))

            yts[0] = stage_A(0, h1s.pop(0))
            h1s[2] = stage_A_pre(2)
            yts[1] = stage_A(1, h1s.pop(1), interject=lambda: _proj(0))
            # preload the full fc weight mid-attention: late enough not to
            # contend for HBM with the startup x/weight loads, early enough
            # to land long before the fc stages need it.
            nc.scalar.dma_start(w_fc_sb[:], w_fcT[:])
            h1s[3] = stage_A_pre(3)
            yts[2] = stage_A(2, h1s.pop(2), interject=lambda: _proj(1))
            yts[3] = stage_A(3, h1s.pop(3), interject=lambda: _proj(2))
            _proj(3)

        # ======================== MLP phase ================================
        # token-parallel over the 4 owned tiles; full weights, no collective
        wmp_pool = ctx.enter_context(tc.tile_pool(name="wmp", bufs=1))
        w_mp_sb = wmp_pool.tile([P, FKO, c_dim], BF16)
        nc.scalar.dma_start(w_mp_sb[:], w_mpT[:])

        gsp = ctx.enter_context(tc.tile_pool(name="gsp", bufs=2))
        gtp = ctx.enter_context(tc.tile_pool(name="gtp", bufs=2))

        def stage_F(qc, h2T):
            """fc + gelu + transpose for the owned tile of chunk qc."""
            g_s = gsp.tile([P, NHC, QCH], BF16, tag="g_s", name="g_s")
            # two PSUM half-rounds of 4 h-chunks: stationary h2T[ko] is
            # amortized over 4 moving-512 matmuls per load
            for half in range(2):
                pss = [
                    ps_mm.tile([P, QCH], F32, tag="ps", name=f"psf_{i}")
                    for i in range(4)
                ]
                for ko in range(KO):
                    for i in range(4):
                        hc = half * 4 + i
                        nc.tensor.matmul(
                            pss[i][:],
                            h2T[:, ko, :],
                            w_fc_sb[:, ko, hc * QCH:(hc + 1) * QCH],
                            start=(ko == 0),
                            stop=(ko == KO - 1),
                        )
                for i in range(4):
                    hc = half * 4 + i
                    if b_fc_bc is not None:
                        nc.vector.tensor_add(
                            pss[i][:], pss[i][:],
                            b_fc_bc[:, hc * QCH:(hc + 1) * QCH],
                        )
                    nc.scalar.activation(
                        g_s[:, hc, :], pss[i][:],
                        mybir.ActivationFunctionType.Gelu_apprx_tanh,
                    )

            gT = gtp.tile([P, FKO, P], BF16, tag="gT", name="gT")
            for hc in range(NHC):
                transpose_into(gT[:, hc * 4:(hc + 1) * 4, :], g_s[:, hc, :], 0, 4)
            return gT

        def stage_M(qc, gT, x1):
            """Down-projection + final residual + store for chunk qc."""
            for nch in range(NCC):
                ps = ps_mm.tile([P, QCH], F32, tag="ps", name="ps")
                for hk in range(FKO):
                    nc.tensor.matmul(
                        ps[:],
                        gT[:, hk, :],
                        w_mp_sb[:, hk, nch * QCH:(nch + 1) * QCH],
                        start=(hk == 0),
                        stop=(hk == FKO - 1),
                    )
                if b_mp_bc is not None:
                    nc.vector.tensor_add(
                        ps[:], ps[:], b_mp_bc[:, nch * QCH:(nch + 1) * QCH]
                    )
                ev = works.tile([P, QCH], F32, tag="evac", name="ev", bufs=2)
                nc.vector.tensor_tensor(
                    ev[:], ps[:], x1[:, nch * QCH:(nch + 1) * QCH],
                    mybir.AluOpType.add,
                )
                nc.sync.dma_start(
                    out_y[qc * P:(qc + 1) * P, nch * QCH:(nch + 1) * QCH], ev[:]
                )

        gts = {}
        h2s[0], x1s[0] = stage_F_pre(0, arsts[1][0:1, KPQ - 1, c_dim - 1:c_dim])
        h2s[1], x1s[1] = stage_F_pre(1, arsts[2][0:1, KPQ - 1, c_dim - 1:c_dim])
        gts[0] = stage_F(0, h2s.pop(0))
        h2s[2], x1s[2] = stage_F_pre(2, arsts[3][0:1, KPQ - 1, c_dim - 1:c_dim])
        gts[1] = stage_F(1, h2s.pop(1))
        stage_M(0, gts.pop(0), x1s.pop(0))
        gts[2] = stage_F(2, h2s.pop(2))
        stage_M(1, gts.pop(1), x1s.pop(1))
        h2s[3], x1s[3] = stage_F_pre(3, gts[2][0:1, FKO - 1, P - 1:P])
        gts[3] = stage_F(3, h2s.pop(3))
        stage_M(2, gts.pop(2), x1s.pop(2))
        stage_M(3, gts.pop(3), x1s.pop(3))

    if legalize:
        _legalize_waits(nc)
    return nc


# ---------------------------------------------------------------------------
# host-side sharding / layout prep


def _tile_k(arr, width):
    """[K, M] -> [128, K//128, M] (contraction dim inner on partitions)."""
    k, m = arr.shape
    assert m == width and k % P == 0
    return np.ascontiguousarray(
        arr.reshape(k // P, P, m).transpose(1, 0, 2)
    )


def _bf(arr):
    return arr.astype(ml_dtypes.bfloat16)


def make_core_inputs(inputs, t_len=T, c_dim=C, h_core=H_CORE, fh=FH,
                     n_groups=len(GROUPS), tpg=TPG):
    """Shard + lay out the full inputs into per-core input dicts and the
    active-flag set."""
    f32 = np.float32
    x = np.asarray(inputs["x"], f32)
    W_attn = np.asarray(inputs["W_attn"], f32)
    W_aproj = np.asarray(inputs["W_aproj"], f32)
    W_fc = np.asarray(inputs["W_fc"], f32)
    W_mproj = np.asarray(inputs["W_mproj"], f32)
    ln1_g = np.asarray(inputs["ln1_g"], f32)
    ln1_b = np.asarray(inputs["ln1_b"], f32)
    ln2_g = np.asarray(inputs["ln2_g"], f32)
    ln2_b = np.asarray(inputs["ln2_b"], f32)
    b_attn = np.asarray(inputs["b_attn"], f32)
    b_aproj = np.asarray(inputs["b_aproj"], f32)
    b_fc = np.asarray(inputs["b_fc"], f32)
    b_mproj = np.asarray(inputs["b_mproj"], f32)

    Wq, Wk, Wv = W_attn[:c_dim], W_attn[c_dim:2 * c_dim], W_attn[2 * c_dim:]
    bq, bk, bv = b_attn[:c_dim], b_attn[c_dim:2 * c_dim], b_attn[2 * c_dim:]
    scale = 1.0 / math.sqrt(HD)

    flags = set()
    if not np.all(ln1_g == 1.0):
        flags.add("ln1_g")
    if np.any(ln1_b):
        flags.add("ln1_b")
    if not np.all(ln2_g == 1.0):
        flags.add("ln2_g")
    if np.any(ln2_b):
        flags.add("ln2_b")
    if np.any(b_attn[:2 * c_dim]):
        flags.add("b_qk")
    if np.any(bv):
        flags.add("b_v")
    if np.any(b_aproj):
        flags.add("b_ap")
    if np.any(b_fc):
        flags.add("b_fc")
    if np.any(b_mproj):
        flags.add("b_mp")

    tri = np.where(
        np.arange(P)[:, None] > np.arange(P)[None, :], f32(-1e30), f32(0.0)
    ).astype(f32)

    # replicated full MLP weights, transposed layouts (contraction inner)
    w_fcT_full = _tile_k(_bf(np.ascontiguousarray(W_fc.T)), fh)
    w_mpT_full = _tile_k(_bf(np.ascontiguousarray(W_mproj.T)), c_dim)

    NQC = t_len // QCH

    in_maps = []
    for core in range(n_groups * tpg):
        g, s = core // tpg, core % tpg
        heads = range(s * h_core, (s + 1) * h_core)
        # stacked [q heads | k heads] output dims, q pre-scaled by 1/sqrt(hd)
        w_qk_rows = np.concatenate(
            [Wq[h * HD:(h + 1) * HD] * scale for h in heads]
            + [Wk[h * HD:(h + 1) * HD] for h in heads], axis=0
        )  # [QK, C]
        w_v_rows = np.concatenate(
            [Wv[h * HD:(h + 1) * HD] for h in heads], axis=0
        )  # [DH, C]
        dsl = slice(s * h_core * HD, (s + 1) * h_core * HD)
        xg = x[g % x.shape[0]]
        x_own = np.concatenate(
            [xg[(qc * tpg + s) * P:(qc * tpg + s + 1) * P] for qc in range(NQC)],
            axis=0,
        )
        m = {
            "x_tm": _bf(np.ascontiguousarray(xg)),
            "x_own": _bf(np.ascontiguousarray(x_own)),
            "w_qk": _tile_k(_bf(w_qk_rows.T), h_core * P),
            "w_v": _tile_k(_bf(w_v_rows.T), h_core * HD),
            "w_ap": _tile_k(_bf(W_aproj[:, dsl].T.copy()), c_dim),
            "w_fcT": w_fcT_full,
            "w_mpT": w_mpT_full,
            "tri": tri,
        }
        if "ln1_g" in flags:
            m["ln1_g"] = ln1_g.reshape(1, -1).copy()
        if "ln1_b" in flags:
            m["ln1_b"] = ln1_b.reshape(1, -1).copy()
        if "ln2_g" in flags:
            m["ln2_g"] = ln2_g.reshape(1, -1).copy()
        if "ln2_b" in flags:
            m["ln2_b"] = ln2_b.reshape(1, -1).copy()
        if "b_qk" in flags:
            b_qk_rows = np.concatenate(
                [bq[h * HD:(h + 1) * HD] * scale for h in heads]
                + [bk[h * HD:(h + 1) * HD] for h in heads]
            )  # [QK] along partitions: [P, MQK]
            m["b_qk"] = np.ascontiguousarray(
                b_qk_rows.reshape(h_core, P).T
            )
        if "b_v" in flags:
            m["b_v"] = np.concatenate(
                [bv[h * HD:(h + 1) * HD] for h in heads]
            ).reshape(1, -1).copy()
        if "b_ap" in flags:
            m["b_ap"] = (b_aproj / tpg).reshape(1, -1).copy()
        if "b_fc" in flags:
            m["b_fc"] = b_fc.reshape(1, -1).copy()
        if "b_mp" in flags:
            m["b_mp"] = b_mproj.reshape(1, -1).copy()
        in_maps.append(m)
    return in_maps, frozenset(flags)


# ---------------------------------------------------------------------------
# runner

_module_cache = {}


def run(inputs, trace=False, trace_kwargs=None, tmpdir=None):
    in_maps, flags = make_core_inputs(inputs)
    key = (flags, trace)
    if key not in _module_cache:
        _module_cache[key] = build_module(flags=flags)
    nc = _module_cache[key]
    if trace:
        _install_prof_hook()
    res = run_bass_kernel_spmd(
        nc,
        in_maps,
        core_ids=list(range(N_CORES)),
        trace=trace,
        tmpdir=tmpdir,
        **(trace_kwargs or {}),
    )
    # reassemble: core g*TPG+s provides token tiles (qc*TPG + s) of batch g
    NQC = T // QCH
    out = np.empty((B, T, C), np.float32)
    for g in range(len(GROUPS)):
        for s in range(TPG):
            o = res.results[g * TPG + s]["out"]
            for qc in range(NQC):
                tt = qc * TPG + s
                out[g, tt * P:(tt + 1) * P, :] = o[qc * P:(qc + 1) * P, :]
    return out, res


def kernel(**inputs) -> np.ndarray:
    out, _ = run(inputs, trace=False)
    return out
